# revision 37
# baseline (speedup 1.0000x reference)
"""Trainium2 Bass kernel for nn_Decoder (dense transformer decoder layer).

Problem (hardcoded): B=4, S=T=1024, D=512, H=8 heads, fp32.
  h  = MHA_self(x, causal) ; x1 = LN(h + x)
  h  = MHA_cross(x1, encod_out) ; x2 = LN(h + x1)
  ff = relu(x2 @ fc1) @ fc2 ; out = LN(ff + x2)

Sharding (8 cores = 4 batch groups x 2-core pairs), same as the f32r
baseline: self-attention tensor-parallel over heads (4 heads/core, full
S); one pair ReduceScatter (bf16) combines partial head-sums and splits
the sequence; LN1, cross-attn (all 8 heads, redundant K/V), LN2, FFN,
LN3 run sequence-parallel on the core's 512-row half.

Speed: nearly all matmuls run as fp8(e4m3) DoubleRow (contraction 256
per instruction at 0.5 cycles/row = 4x the f32r rate in the cost
model).  Numerics (validated against the fp32 reference in numpy):
  - projections/scores/AV/W2 fp8 with power-of-2 scales folded into
    ACT epilogues (x*8, w*512, Q/K/V*32, e*8, w2*2048)
  - bk dropped (exact softmax invariance); bv/bo/bf folded into acc
  - causal tail fix: attention output for early tokens is dominated by
    a single V row, so V for keys 0-255 is recomputed with bf16
    operands and query-tile 0 runs its AV/W2 path in bf16
  - FFN entirely bf16 (fp8 FFN alone costs ~1.5e-2 rel err)
  - residuals/LN in f32; ReduceScatter in bf16
Attention is query-tile-outer / head-inner so the W2 output accumulates
across all heads in one pinned PSUM tile (single epilogue per tile).
"""
import math
import numpy as np

B, S, T, D, H = 4, 1024, 1024, 512, 8
P = 128
NC = 8
DC = D // P    # 4 feature chunks
TC = T // P    # 8 time chunks
SW = 512       # per-core sequence half
QW = 256       # query tile width (DoubleRow moving limit)
MC = 2048 // P  # 16 FFN hidden chunks
EPS = 1e-5
PAIRS = [[0, 1], [2, 3], [4, 5], [6, 7]]

# fp8 scales (powers of 2; folded into f32 epilogue constants)
SX = 8.0       # x / x1 / enc quant
SWQ = 512.0    # wq/wk/wv quant
SQ = 32.0      # Q requant
SK = 32.0      # K requant
SV = 32.0      # V requant
SE = 8.0       # exp output
SW2 = 2048.0   # folded w2 quant
RSQD = 1.0 / math.sqrt(D)

_CACHE = {}


def _host_prep(inputs):
    import ml_dtypes
    F8 = ml_dtypes.float8_e4m3
    BF = ml_dtypes.bfloat16
    x = np.asarray(inputs["x"], np.float32)
    enc = np.asarray(inputs["encod_out"], np.float32)

    def q8(a, s):
        return (np.asarray(a, np.float32) * s).astype(F8)

    per_phase = {}
    for p in ("sa", "ca"):
        wq = np.asarray(inputs[p + "_wq"], np.float32)
        bq = np.asarray(inputs[p + "_bq"], np.float32)
        wk = np.asarray(inputs[p + "_wk"], np.float32)
        wv = np.asarray(inputs[p + "_wv"], np.float32)
        bv = np.asarray(inputs[p + "_bv"], np.float32)
        wo = np.asarray(inputs[p + "_wo"], np.float32)
        bo = np.asarray(inputs[p + "_bo"], np.float32)
        wf = np.asarray(inputs[p + "_wf"], np.float32).reshape(H, D, D)
        bf = np.asarray(inputs[p + "_bf"], np.float32)
        w2 = np.einsum("hfg,hgk->hfk", wo.astype(np.float64),
                       wf.astype(np.float64)).astype(np.float32)
        acc = bf.astype(np.float64).copy()
        for h in range(H):
            acc += (bv[h].astype(np.float64) @ wo[h].astype(np.float64)
                    + bo[h].astype(np.float64)) @ wf[h].astype(np.float64)
        per_phase[p] = dict(
            wq8=q8(wq, SWQ), wk8=q8(wk, SWQ), wv8=q8(wv, SWQ),
            w28=q8(w2, SW2), wv_bf=wv.astype(BF),
            bq=bq * SQ, acc=acc.astype(np.float32))

    fc1_w = np.asarray(inputs["fc1_w"], np.float32)
    fc1_b = np.asarray(inputs["fc1_b"], np.float32)
    fc2_w = np.asarray(inputs["fc2_w"], np.float32)
    fc2_b = np.asarray(inputs["fc2_b"], np.float32)
    lns = {f"ln{i}_{k}": np.asarray(inputs[f"ln{i}_{k}"], np.float32)
           for i in (1, 2, 3) for k in ("g", "b")}

    # causal masks for the two diagonal key-blocks of each 256-query tile:
    # kb 2i: keep p <= c ; kb 2i+1: keep 128+p <= c   (c in 0..255)
    pp_ = np.arange(P)[:, None]
    cc = np.arange(QW)[None, :]
    mpair = np.stack([(pp_ <= cc), (P + pp_ <= cc)], axis=1)
    ones_pair = np.ones((P, 2, 1), np.float32)

    in_maps = []
    for c in range(NC):
        b, half = c // 2, c % 2
        hs = slice(4 * half, 4 * half + 4)
        ssl = slice(half * SW, (half + 1) * SW)
        xt = x[b].T
        m = {
            "xt8": np.ascontiguousarray((xt * SX)).astype(F8),
            "xbf": np.ascontiguousarray(xt[:, :QW]).astype(BF),
            "x_res": np.ascontiguousarray(xt[:, ssl]),
            "et8": np.ascontiguousarray(enc[b].T * SX).astype(F8),
            "masks8": mpair.astype(F8),
            "ones8": ones_pair.astype(F8),
            "onesbf": np.ones((P, 1), BF),
            "ones_row": np.ones((1, P), np.float32),
            "fc1bf": fc1_w.astype(BF), "fc1_b": fc1_b,
            "fc2bf": fc2_w.astype(BF), "ffn_bias": fc2_b,
        }
        pp = per_phase["sa"]
        for k in ("wq8", "wk8", "wv8", "w28", "wv_bf", "bq"):
            m["sa_" + k] = np.ascontiguousarray(pp[k][hs])
        m["sa_acc"] = pp["acc"] / 2.0
        pp = per_phase["ca"]
        for k in ("wq8", "wk8", "wv8", "w28", "bq"):
            m["ca_" + k] = pp[k]
        m["ca_acc"] = pp["acc"]
        for k, v in lns.items():
            m[k] = v
        in_maps.append(m)
    return in_maps


def build_program():
    import concourse.bacc as bacc
    import concourse.mybir as mybir
    import concourse.tile as tile

    F32 = mybir.dt.float32
    F32R = mybir.dt.float32r
    BF16 = mybir.dt.bfloat16
    F8 = mybir.dt.float8e4
    AF = mybir.ActivationFunctionType
    OP = mybir.AluOpType
    DR = mybir.MatmulPerfMode.DoubleRow

    nc = bacc.Bacc(None, target_bir_lowering=False, num_devices=NC)

    # ---- DRAM I/O ----
    xt8_d = nc.dram_tensor("xt8", [D, S], F8, kind="ExternalInput")
    xbf_d = nc.dram_tensor("xbf", [D, QW], BF16, kind="ExternalInput")
    xres_d = nc.dram_tensor("x_res", [D, SW], F32, kind="ExternalInput")
    et8_d = nc.dram_tensor("et8", [D, T], F8, kind="ExternalInput")
    masks8_d = nc.dram_tensor("masks8", [P, 2, QW], F8, kind="ExternalInput")
    ones8_d = nc.dram_tensor("ones8", [P, 2, 1], F8, kind="ExternalInput")
    onesbf_d = nc.dram_tensor("onesbf", [P, 1], BF16, kind="ExternalInput")
    onesrow_d = nc.dram_tensor("ones_row", [1, P], F32, kind="ExternalInput")
    sa_d = {k: nc.dram_tensor("sa_" + k, [4, D, D], F8, kind="ExternalInput")
            for k in ("wq8", "wk8", "wv8", "w28")}
    sa_d["wv_bf"] = nc.dram_tensor("sa_wv_bf", [4, D, D], BF16, kind="ExternalInput")
    sa_d["bq"] = nc.dram_tensor("sa_bq", [4, D], F32, kind="ExternalInput")
    sa_d["acc"] = nc.dram_tensor("sa_acc", [D], F32, kind="ExternalInput")
    ca_d = {k: nc.dram_tensor("ca_" + k, [H, D, D], F8, kind="ExternalInput")
            for k in ("wq8", "wk8", "wv8", "w28")}
    ca_d["bq"] = nc.dram_tensor("ca_bq", [H, D], F32, kind="ExternalInput")
    ca_d["acc"] = nc.dram_tensor("ca_acc", [D], F32, kind="ExternalInput")
    fc1_d = nc.dram_tensor("fc1bf", [D, 2048], BF16, kind="ExternalInput")
    fc1b_d = nc.dram_tensor("fc1_b", [2048], F32, kind="ExternalInput")
    fc2_d = nc.dram_tensor("fc2bf", [2048, D], BF16, kind="ExternalInput")
    ffnb_d = nc.dram_tensor("ffn_bias", [D], F32, kind="ExternalInput")
    ln_d = {f"ln{i}_{k}": nc.dram_tensor(f"ln{i}_{k}", [D], F32, kind="ExternalInput")
            for i in (1, 2, 3) for k in ("g", "b")}
    outt_d = nc.dram_tensor("outt", [D, SW], F32, kind="ExternalOutput")

    r32 = lambda ap: ap.bitcast(F32R)

    # epilogue constants
    C_Q = SQ / (SX * SWQ)
    C_K = SK / (SX * SWQ)
    C_V = SV / (SX * SWQ)
    C_EXP = RSQD / (SQ * SK)     # scale on score psum inside exp
    C_W2 = 1.0 / (SV * SW2)      # scale on fp8 W2 psum
    LN_SE = math.log(SE)

    with tile.TileContext(nc, pool_alloc_mode="queue") as tc:
        with tc.tile_pool(name="const", bufs=1) as constp, \
             tc.tile_pool(name="resid", bufs=2) as residp, \
             tc.tile_pool(name="smalls", bufs=3) as smallp, \
             tc.tile_pool(name="stats", bufs=4) as statp, \
             tc.tile_pool(name="pp", bufs=2, space="PSUM") as pp, \
             tc.tile_pool(name="sc", bufs=2, space="PSUM") as scp, \
             tc.tile_pool(name="po", bufs=1, space="PSUM") as pop, \
             tc.tile_pool(name="pw", bufs=1, space="PSUM") as pwp, \
             tc.tile_pool(name="dram", bufs=1, space="DRAM") as dramp:

            # ---- constants ----
            eps_sb = constp.tile([1, 1], F32, name="eps_sb")
            nc.vector.memset(eps_sb[:], EPS)
            ln8_col = constp.tile([P, 1], F32, name="ln8_col")
            nc.vector.memset(ln8_col[:], LN_SE)
            zero_col = constp.tile([P, 1], F32, name="zero_col")
            nc.vector.memset(zero_col[:], 0.0)
            ones_col = constp.tile([P, 1], F32R, name="ones_col")
            nc.vector.memset(ones_col[:], 1.0)

            xt8_sb = residp.tile([P, DC, S], F8, name="xt8_sb", tag="resid")
            nc.sync.dma_start(out=xt8_sb[:],
                              in_=xt8_d.ap().rearrange("(c p) s -> p c s", p=P))
            xbf_sb = constp.tile([P, DC, QW], BF16, name="xbf_sb")
            nc.scalar.dma_start(out=xbf_sb[:],
                                in_=xbf_d.ap().rearrange("(c p) s -> p c s", p=P))
            ca_kv_scope = tc.tile_pool(name="ca_kvp", bufs=4)
            kvpp = ca_kv_scope.__enter__()
            ca_kvw_scope = tc.tile_pool(name="ca_kvw", bufs=2)
            kvwp = ca_kvw_scope.__enter__()
            early_scope = tc.tile_pool(name="early", bufs=1)
            earlyp = early_scope.__enter__()
            xres_sb = earlyp.tile([P, DC, SW], F32, name="xres_sb", tag="xres")
            nc.scalar.dma_start(out=xres_sb[:],
                                in_=xres_d.ap().rearrange("(c p) s -> p c s", p=P))
            et8_sb = kvwp.tile([P, DC, T], F8, name="et8_sb", tag="et",
                               bufs=1)
            nc.scalar.dma_start(out=et8_sb[:],
                                in_=et8_d.ap().rearrange("(c p) s -> p c s", p=P))

            masks8_sb = constp.tile([P, 2, QW], F8, name="masks8_sb")
            nc.scalar.dma_start(out=masks8_sb[:], in_=masks8_d.ap())
            ones8_sb = constp.tile([P, 2, 1], F8, name="ones8_sb")
            nc.scalar.dma_start(out=ones8_sb[:], in_=ones8_d.ap())
            onesbf_sb = constp.tile([P, 1], BF16, name="onesbf_sb")
            nc.scalar.dma_start(out=onesbf_sb[:], in_=onesbf_d.ap())
            ones_row = constp.tile([1, P], F32R, name="ones_row")
            nc.scalar.dma_start(out=ones_row[:], in_=r32(onesrow_d.ap()))

            def vec_to_pc(dram_ap, name, nch):
                t = constp.tile([P, nch], F32, name=name)
                nc.scalar.dma_start(out=t[:],
                                    in_=dram_ap.rearrange("(c p) -> p c", p=P))
                return t

            bias_sb = {}
            for pn, dd, nh in (("sa", sa_d, 4), ("ca", ca_d, H)):
                t = constp.tile([P, nh, DC], F32, name=f"{pn}_bq_sb")
                nc.scalar.dma_start(
                    out=t[:], in_=dd["bq"].ap().rearrange("h (c p) -> p h c", p=P))
                bias_sb[pn, "bq"] = t
                bias_sb[pn, "acc"] = vec_to_pc(dd["acc"].ap(), f"{pn}_acc_sb", DC)
            grow_sb = constp.tile([1, 3, DC, P], F32R, name="ln_grow")
            for _i in (1, 2, 3):
                nc.scalar.dma_start(
                    out=grow_sb[:, _i - 1, :, :],
                    in_=r32(ln_d[f"ln{_i}_g"].ap().rearrange(
                        "(a c p) -> a c p", a=1, p=P)))
            fc1b_sb = vec_to_pc(fc1b_d.ap(), "fc1b_sb", MC)
            ffnb_sb = vec_to_pc(ffnb_d.ap(), "ffnb_sb", DC)
            ln_sb = {k: vec_to_pc(v.ap(), k + "_sb", DC) for k, v in ln_d.items()}

            cc_in = dramp.tile([2, D, SW], BF16, name="cc_in")
            cc_half = dramp.tile([D, SW], BF16, name="cc_half")

            def layernorm_half(src, resid_sb, dst, b_sb, gri, csl):
                """dst[:, :, csl] = LN(src + resid) on a column slice."""
                W = csl.stop - csl.start
                for c in range(DC):
                    nc.vector.tensor_add(dst[:, c, csl], src[:, c, csl],
                                         resid_sb[:, c, csl])
                psum_sum = pp.tile([1, SW], F32, name="ln_sum", tag="pp")
                psum_ssq = pp.tile([1, SW], F32, name="ln_ssq", tag="pp")
                for c in range(DC):
                    sq = smallp.tile([P, SW], F32R, name=f"ln_sq_{c}", tag="sm")
                    nc.scalar.activation(sq[:, :W], dst[:, c, csl], AF.Square)
                    nc.tensor.matmul(psum_sum[:, :W], ones_col[:],
                                     dst[:, c, csl],
                                     start=(c == 0), stop=(c == DC - 1))
                    nc.tensor.matmul(psum_ssq[:, :W], ones_col[:], sq[:, :W],
                                     start=(c == 0), stop=(c == DC - 1))
                mean = statp.tile([1, SW], F32R, name="ln_mean", tag="st")
                nc.scalar.activation(mean[:, :W], psum_sum[:, :W], AF.Copy,
                                     scale=1.0 / D)
                msq = statp.tile([1, SW], F32, name="ln_msq", tag="st")
                nc.scalar.activation(msq[:, :W], psum_ssq[:, :W], AF.Copy,
                                     scale=1.0 / D)
                var = statp.tile([1, SW], F32, name="ln_var", tag="st")
                nc.vector.tensor_tensor(var[:, :W], mean[:, :W], mean[:, :W],
                                        OP.mult)
                nc.vector.tensor_sub(var[:, :W], msq[:, :W], var[:, :W])
                std = statp.tile([1, SW], F32, name="ln_std", tag="st")
                nc.scalar.activation(std[:, :W], var[:, :W], AF.Sqrt,
                                     bias=eps_sb[:])
                rstd = statp.tile([1, SW], F32R, name="ln_rstd", tag="st")
                with nc.allow_low_precision(reason="f32r feed for bcast matmul"):
                    nc.vector.reciprocal(rstd[:, :W], std[:, :W])
                mr = statp.tile([1, SW], F32R, name="ln_mr", tag="st")
                nc.vector.tensor_tensor(mr[:, :W], mean[:, :W], rstd[:, :W],
                                        OP.mult)
                for c in range(DC):
                    psum_rb = pp.tile([P, SW], F32, name=f"ln_rb_{c}", tag="pp")
                    nc.tensor.matmul(psum_rb[:, :W], grow_sb[:, gri, c, :],
                                     rstd[:, :W], start=True, stop=True)
                    nc.tensor.matmul(psum_rb[:, W:2 * W],
                                     grow_sb[:, gri, c, :],
                                     mr[:, :W], start=True, stop=True)
                    tmp = smallp.tile([P, SW], F32, name=f"ln_t_{c}", tag="sm")
                    nc.vector.tensor_tensor(tmp[:, :W], dst[:, c, csl],
                                            psum_rb[:, :W], OP.mult)
                    nc.vector.scalar_tensor_tensor(
                        dst[:, c, csl], tmp[:, :W], b_sb[:, c:c + 1],
                        psum_rb[:, W:2 * W], OP.add, OP.subtract)

            def layernorm(src, resid_sb, dst, b_sb, gri):
                for hlf in range(2):
                    layernorm_half(src, resid_sb, dst, b_sb, gri,
                                   slice(hlf * QW, (hlf + 1) * QW))

            def epi_rr(eng, out_ap, ps_ap, cscale, bias_ap):
                """projection epilogue out = ps*c (+bias) on a chosen engine"""
                if eng is nc.scalar:
                    return nc.scalar.activation(
                        out_ap, ps_ap, AF.Identity, scale=cscale,
                        bias=bias_ap if bias_ap is not None else zero_col[:])
                if bias_ap is None:
                    return eng.tensor_scalar(out_ap, ps_ap, cscale, None,
                                             OP.mult)
                return eng.tensor_scalar(out_ap, ps_ap, cscale, bias_ap,
                                         OP.mult, OP.add)

            def proj_head(src_sb, wk_sb, wv_sb, wq_ap, bq_ap, kt8, v8, qt8,
                          nq, rr):
                """fp8 K^T [f,t], V [t,f], and optional Q^T [f,q] for one
                head.  Epilogues round-robin across Pool/ACT/DVE so all
                three drain in parallel."""
                last = None
                nrr = len(rr)
                ei = [0]
                def nxt():
                    e = rr[ei[0] % nrr]; ei[0] += 1
                    return e
                for fc in range(DC):
                    for th in range(2):
                        ps = pp.tile([P, SW], F32, name=f"kp_{fc}_{th}",
                                     tag="pp")
                        for nt in range(2):
                            for cp in range(2):
                                nc.tensor.matmul(
                                    ps[:, nt * QW:(nt + 1) * QW],
                                    wk_sb[:, 2 * cp:2 * cp + 2,
                                          fc * P:(fc + 1) * P],
                                    xt8_slice(src_sb, cp,
                                              th * SW + nt * QW),
                                    start=(cp == 0), stop=(cp == 1),
                                    perf_mode=DR)
                        epi_rr(nxt(), kt8[:, fc, th * SW:(th + 1) * SW],
                               ps[:], C_K, None)
                    for tci in (2 * fc, 2 * fc + 1):
                        ps = scp.tile([P, 2, QW], F32, name=f"vp_{tci}",
                                      tag="sc")
                        for fh in range(2):
                            for cp in range(2):
                                nc.tensor.matmul(
                                    ps[:, fh, :],
                                    src_sb[:, 2 * cp:2 * cp + 2,
                                           tci * P:(tci + 1) * P],
                                    wv_sb[:, 2 * cp:2 * cp + 2,
                                          fh * QW:(fh + 1) * QW],
                                    start=(cp == 0), stop=(cp == 1),
                                    perf_mode=DR)
                        last = epi_rr(nxt(), v8[:, tci, :], ps[:], C_V, None)
                    if wq_ap is not None:
                        for qh in range(nq):
                            ps = pp.tile([P, SW], F32, name=f"qp_{fc}_{qh}",
                                         tag="pp")
                            for nt in range(2):
                                for cp in range(2):
                                    nc.tensor.matmul(
                                        ps[:, nt * QW:(nt + 1) * QW],
                                        wq_ap[:, 2 * cp:2 * cp + 2,
                                              fc * P:(fc + 1) * P],
                                        xt8_slice(src_sb, cp,
                                                  qh * SW + nt * QW),
                                        start=(cp == 0), stop=(cp == 1),
                                        perf_mode=DR)
                            epi_rr(nxt(), qt8[:, fc, qh * SW:(qh + 1) * SW],
                                   ps[:], C_Q, bq_ap[:, fc:fc + 1])
                return last

            def xt8_slice(src_sb, cp, q0):
                return src_sb[:, 2 * cp:2 * cp + 2, q0:q0 + QW]

            # ============ self-attention (head-split, full S) ============
            with tc.tile_pool(name="sa_w2", bufs=1) as w2p, \
                 tc.tile_pool(name="sa_qkv", bufs=1) as qkvp, \
                 tc.tile_pool(name="sa_e", bufs=3) as ep, \
                 tc.tile_pool(name="sa_av", bufs=2) as avp, \
                 tc.tile_pool(name="sa_f", bufs=1) as fp:
                wkv_scope = tc.tile_pool(name="sa_wkv", bufs=2)
                wkvp = wkv_scope.__enter__()
                # projections for all 4 heads; per-head JIT weight DMA
                wq_sb, wk_sb, wv_sb, wvb_sb, w28_sb, w2b_sb = {}, {}, {}, {}, {}, {}
                kt8, v8, vbf, qt8 = {}, {}, {}, {}
                for h in range(4):
                    wk_sb[h] = wkvp.tile([P, DC, D], F8, name=f"sa_wk_{h}",
                                         tag="wk")
                    nc.sync.dma_start(out=wk_sb[h][:], in_=sa_d["wk8"].ap()[h]
                                      .rearrange("(c p) f -> p c f", p=P))
                    wv_sb[h] = wkvp.tile([P, DC, D], F8, name=f"sa_wv_{h}",
                                         tag="wv")
                    nc.sync.dma_start(out=wv_sb[h][:], in_=sa_d["wv8"].ap()[h]
                                      .rearrange("(c p) f -> p c f", p=P))
                    wq_sb[h] = wkvp.tile([P, DC, D], F8, name=f"sa_wq_{h}",
                                         tag="wq")
                    nc.sync.dma_start(out=wq_sb[h][:], in_=sa_d["wq8"].ap()[h]
                                      .rearrange("(c p) f -> p c f", p=P))
                    wvb_sb[h] = wkvp.tile([P, DC, D], BF16, name=f"sa_wvb_{h}",
                                          tag="wvb")
                    nc.sync.dma_start(out=wvb_sb[h][:], in_=sa_d["wv_bf"].ap()[h]
                                      .rearrange("(c p) f -> p c f", p=P))
                    kt8[h] = qkvp.tile([P, DC, S], F8, name=f"sa_kt_{h}",
                                       tag=f"kt{h}")
                    v8[h] = qkvp.tile([P, TC, D], F8, name=f"sa_v_{h}",
                                      tag=f"v{h}")
                    qt8[h] = qkvp.tile([P, DC, S], F8, name=f"sa_qt_{h}",
                                       tag=f"qt{h}")
                    proj_head(xt8_sb, wk_sb[h], wv_sb[h], wq_sb[h],
                              bias_sb["sa", "bq"][:, h, :], kt8[h], v8[h],
                              qt8[h], 2, [nc.gpsimd, nc.scalar, nc.vector])
                    # bf16 V for keys 0..255 (early-token numerics)
                    vbf[h] = qkvp.tile([P, 2, D], BF16, name=f"sa_vb_{h}",
                                       tag=f"vb{h}")
                    for tci in range(2):
                        ps = pp.tile([P, SW], F32, name=f"vbp_{h}_{tci}",
                                     tag="pp")
                        for c in range(DC):
                            nc.tensor.matmul(
                                ps[:], xbf_sb[:, c, tci * P:(tci + 1) * P],
                                wvb_sb[h][:, c, :],
                                start=(c == 0), stop=(c == DC - 1))
                        nc.vector.tensor_copy(vbf[h][:, tci, :], ps[:])
                wkv_scope.__exit__(None, None, None)
                # W2 weights arrive after the projection weights
                for h in range(4):
                    w28_sb[h] = w2p.tile([P, DC, D], F8, name=f"sa_w28_{h}",
                                         tag=f"w28{h}")
                    nc.sync.dma_start(out=w28_sb[h][:], in_=sa_d["w28"].ap()[h]
                                      .rearrange("(c p) f -> p c f", p=P))

                fbf = fp.tile([P, DC, S], BF16, name="sa_fbf")

                # attention: query-tile outer, head inner
                for qt in range(4):
                    qsl = slice(qt * QW, (qt + 1) * QW)
                    npair = qt + 1     # key-block pairs for this tile
                    bf = (qt == 0)     # bf16 AV/W2 path for queries 0..255
                    pw = pwp.tile([P, DC, QW], F32, name=f"sa_pw_{qt}", tag="pw")
                    rbs = {}
                    for h in range(4):
                        e8 = ep.tile([P, 2, QW] if bf else [P, TC, QW],
                                     BF16 if bf else F8,
                                     name=f"sa_e_{qt}_{h}",
                                     tag="ebf" if bf else "e",
                                     bufs=2 if bf else None)
                        dn = pp.tile([1, 2, QW], F32, name=f"sa_dn_{qt}_{h}",
                                     tag="pp")
                        for jp in range(npair):
                            sps = scp.tile([P, 2, QW], F32,
                                           name=f"sa_s_{qt}_{h}_{jp}", tag="sc")
                            for j2 in range(2):
                                kb = 2 * jp + j2
                                for fcp in range(2):
                                    nc.tensor.matmul(
                                        sps[:, j2, :],
                                        kt8[h][:, 2 * fcp:2 * fcp + 2,
                                               kb * P:(kb + 1) * P],
                                        qt8[h][:, 2 * fcp:2 * fcp + 2, qsl],
                                        start=(fcp == 0), stop=(fcp == 1),
                                        perf_mode=DR)
                            nc.scalar.activation(
                                e8[:, 2 * jp:2 * jp + 2, :], sps[:], AF.Exp,
                                scale=C_EXP,
                                bias=zero_col[:] if bf else ln8_col[:])
                            if jp == npair - 1:   # diagonal pair: apply mask
                                nc.gpsimd.tensor_tensor(
                                    e8[:, 2 * jp:2 * jp + 2, :],
                                    e8[:, 2 * jp:2 * jp + 2, :],
                                    masks8_sb[:], OP.mult)
                        if bf:
                            for j2 in range(2):
                                nc.tensor.matmul(
                                    dn[:, 0, :], onesbf_sb[:],
                                    e8[:, j2, :],
                                    start=(j2 == 0), stop=(j2 == 1))
                        else:
                            for jp in range(npair):
                                nc.tensor.matmul(
                                    dn[:, 0, :], ones8_sb[:],
                                    e8[:, 2 * jp:2 * jp + 2, :],
                                    start=(jp == 0), stop=(jp == npair - 1),
                                    perf_mode=DR)
                        rc = statp.tile([1, QW], F32R, name=f"sa_rc_{qt}_{h}",
                                        tag="st")
                        with nc.allow_low_precision(reason="recip bcast"):
                            nc.vector.reciprocal(rc[:], dn[:, 0, :])
                        rb = pp.tile([P, QW], F32, name=f"sa_rb_{qt}_{h}",
                                     tag="pp")
                        nc.tensor.matmul(rb[:], ones_row[:], rc[:],
                                         start=True, stop=True)
                        rbs[h] = rb

                        po = pop.tile([P, DC, QW], F32, name=f"sa_po_{qt}_{h}",
                                      tag="po")
                        avn = avp.tile([P, DC, QW], BF16 if bf else F8,
                                       name=f"sa_avn_{qt}_{h}",
                                       tag="avnbf" if bf else "avn",
                                       bufs=1 if bf else None)
                        for fc in range(DC):
                            if bf:
                                for j in range(2):
                                    nc.tensor.matmul(
                                        po[:, fc, :],
                                        vbf[h][:, j, fc * P:(fc + 1) * P],
                                        e8[:, j, :],
                                        start=(j == 0), stop=(j == 1))
                            else:
                                for jp in range(npair):
                                    nc.tensor.matmul(
                                        po[:, fc, :],
                                        v8[h][:, 2 * jp:2 * jp + 2,
                                              fc * P:(fc + 1) * P],
                                        e8[:, 2 * jp:2 * jp + 2, :],
                                        start=(jp == 0), stop=(jp == npair - 1),
                                        perf_mode=DR)
                            (nc.vector if fc < 2 else nc.gpsimd).tensor_tensor(
                                avn[:, fc, :], po[:, fc, :], rbs[h][:], OP.mult)
                        w2s = w28_sb[h]
                        for gc in range(DC):
                            for fcp in range(2):
                                if bf:
                                    # bf16 avn x fp8 w2 (no DoubleRow)
                                    for c2 in range(2):
                                        nc.tensor.matmul(
                                            pw[:, gc, :],
                                            w2s[:, 2 * fcp + c2,
                                                gc * P:(gc + 1) * P],
                                            avn[:, 2 * fcp + c2, :],
                                            start=(h == 0 and fcp == 0
                                                   and c2 == 0),
                                            stop=(h == 3 and fcp == 1
                                                  and c2 == 1))
                                else:
                                    nc.tensor.matmul(
                                        pw[:, gc, :],
                                        w2s[:, 2 * fcp:2 * fcp + 2,
                                            gc * P:(gc + 1) * P],
                                        avn[:, 2 * fcp:2 * fcp + 2, :],
                                        start=(h == 0 and fcp == 0),
                                        stop=(h == 3 and fcp == 1),
                                        perf_mode=DR)
                    # epilogue: all 4 heads accumulated; + acc bias -> bf16
                    cw = (1.0 / SW2) if bf else C_W2
                    for gc in range(DC):
                        nc.vector.tensor_scalar(
                            fbf[:, gc, qsl], pw[:, gc, :], cw,
                            bias_sb["sa", "acc"][:, gc:gc + 1],
                            OP.mult, OP.add)

                for half in range(2):
                    nc.sync.dma_start(
                        out=cc_in[half].rearrange("(c p) s -> p c s", p=P),
                        in_=fbf[:, :, half * SW:(half + 1) * SW])

            # one pair collective: reduce partial head-sums + scatter seq halves
            nc.gpsimd.collective_compute(
                "ReduceScatter", mybir.AluOpType.add, replica_groups=PAIRS,
                ins=[cc_in.opt()], outs=[cc_half.opt()])

            # ---- cross-attention K/V projections.  All 8 heads emitted into
            # a 4-deep rotating pool: the first ~3 run during the collective,
            # the rest pipeline as the head-outer attention loop frees slots.
            ln1_anchor = [None]
            ca_kt8, ca_v8 = {}, {}

            def ca_kv_proj(h, rr=None):
                wk_s = kvwp.tile([P, DC, D], F8, name=f"ca_wk_{h}", tag="wk")
                nc.sync.dma_start(out=wk_s[:], in_=ca_d["wk8"].ap()[h]
                                  .rearrange("(c p) f -> p c f", p=P))
                wv_s = kvwp.tile([P, DC, D], F8, name=f"ca_wv_{h}", tag="wv")
                nc.sync.dma_start(out=wv_s[:], in_=ca_d["wv8"].ap()[h]
                                  .rearrange("(c p) f -> p c f", p=P))
                ca_kt8[h] = kvpp.tile([P, DC, T], F8, name=f"ca_ktp_{h}",
                                      tag="cktp")
                ca_v8[h] = kvpp.tile([P, TC, D], F8, name=f"ca_vp_{h}",
                                     tag="cvp")
                return proj_head(et8_sb, wk_s, wv_s, None, None,
                                 ca_kt8[h], ca_v8[h], None, 0,
                                 rr or [nc.gpsimd, nc.scalar, nc.vector])

            for h in range(4):
                last = ca_kv_proj(h)
                if h == 2:
                    ln1_anchor[0] = last

            # ---- LN1 on my sequence half ----
            cch_sb = earlyp.tile([P, DC, SW], BF16, name="cch_sb",
                                 tag="cch")
            nc.sync.dma_start(
                out=cch_sb[:],
                in_=cc_half.opt().rearrange("(c p) s -> p c s", p=P))
            x1_sb = residp.tile([P, DC, SW], F32R, name="x1_sb", tag="resid")
            from concourse.tile import add_dep_helper as _adh
            _bb = nc.main_func.blocks[-1]
            _n0 = len(_bb.instructions)
            layernorm(cch_sb, xres_sb, x1_sb, ln_sb["ln1_b"], 0)
            x18_sb = residp.tile([P, DC, SW], F8, name="x18_sb", tag="x18",
                                 bufs=1)
            for c in range(DC):
                nc.scalar.activation(x18_sb[:, c, :], x1_sb[:, c, :],
                                     AF.Copy, scale=SX)
            if ln1_anchor[0] is not None:
                for _ins in list(_bb.instructions)[_n0:]:
                    _adh(_ins, ln1_anchor[0].ins, sync=False,
                         reason="order LN1 after CA-KV precompute h2")
            early_scope.__exit__(None, None, None)

            # FFN weights prefetch (transfers overlap CA attention)
            ffn_w_scope = tc.tile_pool(name="ffn_w", bufs=1)
            fwp = ffn_w_scope.__enter__()
            fc1_sb = fwp.tile([P, DC, 2048], BF16, name="fc1_sb")
            fc2_sb = fwp.tile([P, MC, D], BF16, name="fc2_sb")

            # ============ cross-attention (seq-split, all heads) ============
            with tc.tile_pool(name="ca_w", bufs=1) as cwp, \
                 tc.tile_pool(name="ca_qt", bufs=1) as cqtp, \
                 tc.tile_pool(name="ca_e", bufs=2) as ep, \
                 tc.tile_pool(name="ca_av", bufs=2) as avp, \
                 tc.tile_pool(name="ca_f", bufs=1) as fp:
                wq_c, w2_c = {}, {}
                for h in range(H):
                    w2_c[h] = cwp.tile([P, DC, D], F8, name=f"ca_w2_{h}",
                                       tag=f"cw2{h}")
                    nc.sync.dma_start(out=w2_c[h][:], in_=ca_d["w28"].ap()[h]
                                      .rearrange("(c p) f -> p c f", p=P))
                # wq tiles 2-buffered: freed as Q projections complete
                cwqp_scope = tc.tile_pool(name="ca_wq", bufs=2)
                cwqp = cwqp_scope.__enter__()
                for h in range(H):
                    wq_c[h] = cwqp.tile([P, DC, D], F8, name=f"ca_wq_{h}",
                                        tag="cwq")
                    nc.sync.dma_start(out=wq_c[h][:], in_=ca_d["wq8"].ap()[h]
                                      .rearrange("(c p) f -> p c f", p=P))
                for mg in range(4):
                    nc.sync.dma_start(
                        out=fc1_sb[:, :, mg * SW:(mg + 1) * SW],
                        in_=fc1_d.ap().rearrange("(c p) m -> p c m", p=P)
                        [:, :, mg * SW:(mg + 1) * SW])
                nc.sync.dma_start(out=fc2_sb[:], in_=fc2_d.ap()
                                  .rearrange("(c p) g -> p c g", p=P))
                qt8_c = {}
                for h in range(H):
                    qt8_c[h] = cqtp.tile([P, DC, SW], F8, name=f"ca_qt_{h}",
                                         tag=f"cq{h}")
                    _ = wq_c[h]  # DMA emitted above; tiles rotate via pool
                    for fc in range(DC):
                        ps = pp.tile([P, SW], F32, name=f"cqp_{h}_{fc}",
                                     tag="pp")
                        for nt in range(2):
                            for cp in range(2):
                                nc.tensor.matmul(
                                    ps[:, nt * QW:(nt + 1) * QW],
                                    wq_c[h][:, 2 * cp:2 * cp + 2,
                                            fc * P:(fc + 1) * P],
                                    x18_sb[:, 2 * cp:2 * cp + 2,
                                           nt * QW:(nt + 1) * QW],
                                    start=(cp == 0), stop=(cp == 1),
                                    perf_mode=DR)
                        (nc.gpsimd if h % 2 else nc.vector).tensor_scalar(
                            qt8_c[h][:, fc, :], ps[:], C_Q,
                            bias_sb["ca", "bq"][:, h, fc:fc + 1],
                            OP.mult, OP.add)
                cwqp_scope.__exit__(None, None, None)

                f2_sb = fp.tile([P, DC, SW], F32, name="ca_f2")
                for h in range(H):
                    e8 = ep.tile([P, 2 * TC, QW], F8, name=f"ca_e_{h}",
                                 tag="e")
                    dn = pp.tile([1, 2, QW], F32, name=f"ca_dn_{h}", tag="pp")
                    rb = pp.tile([P, 2, QW], F32, name=f"ca_rb_{h}", tag="pp")
                    for qt in range(2):
                        qsl = slice(qt * QW, (qt + 1) * QW)
                        for jp in range(4):
                            sps = scp.tile([P, 2, QW], F32,
                                           name=f"ca_s_{qt}_{h}_{jp}", tag="sc")
                            for j2 in range(2):
                                kb = 2 * jp + j2
                                for fcp in range(2):
                                    nc.tensor.matmul(
                                        sps[:, j2, :],
                                        ca_kt8[h][:, 2 * fcp:2 * fcp + 2,
                                                  kb * P:(kb + 1) * P],
                                        qt8_c[h][:, 2 * fcp:2 * fcp + 2, qsl],
                                        start=(fcp == 0), stop=(fcp == 1),
                                        perf_mode=DR)
                            nc.scalar.activation(
                                e8[:, 8 * qt + 2 * jp:8 * qt + 2 * jp + 2, :],
                                sps[:], AF.Exp, scale=C_EXP, bias=ln8_col[:])
                    # denominators after the scores loop (avoids PE
                    # head-of-line wait on each exp)
                    for qt in range(2):
                        for jp in range(4):
                            nc.tensor.matmul(
                                dn[:, qt, :], ones8_sb[:],
                                e8[:, 8 * qt + 2 * jp:8 * qt + 2 * jp + 2, :],
                                start=(jp == 0), stop=(jp == 3),
                                perf_mode=DR)
                    rc = statp.tile([1, 2, QW], F32R, name=f"ca_rc_{h}",
                                    tag="st")
                    with nc.allow_low_precision(reason="recip bcast"):
                        nc.vector.reciprocal(rc[:], dn[:])
                    for qt in range(2):
                        nc.tensor.matmul(rb[:, qt, :], ones_row[:],
                                         rc[:, qt, :], start=True, stop=True)
                    for qt in range(2):
                        qsl = slice(qt * QW, (qt + 1) * QW)
                        po = pop.tile([P, DC, QW], F32, name=f"ca_po_{qt}_{h}",
                                      tag="po")
                        avn = avp.tile([P, DC, QW], F8,
                                       name=f"ca_avn_{qt}_{h}", tag="avn")
                        for fc in range(DC):
                            for jp in range(4):
                                nc.tensor.matmul(
                                    po[:, fc, :],
                                    ca_v8[h][:, 2 * jp:2 * jp + 2,
                                             fc * P:(fc + 1) * P],
                                    e8[:, 8 * qt + 2 * jp:8 * qt + 2 * jp + 2, :],
                                    start=(jp == 0), stop=(jp == 3),
                                    perf_mode=DR)
                            (nc.vector if fc < 2 else nc.gpsimd).tensor_tensor(
                                avn[:, fc, :], po[:, fc, :], rb[:, qt, :],
                                OP.mult)
                        pw = pwp.tile([P, DC, QW], F32, name=f"ca_pw_{qt}_{h}",
                                      tag="pw")
                        for gc in range(DC):
                            for fcp in range(2):
                                nc.tensor.matmul(
                                    pw[:, gc, :],
                                    w2_c[h][:, 2 * fcp:2 * fcp + 2,
                                            gc * P:(gc + 1) * P],
                                    avn[:, 2 * fcp:2 * fcp + 2, :],
                                    start=(fcp == 0), stop=(fcp == 1),
                                    perf_mode=DR)
                        if h == 0:
                            for gc in range(DC):
                                nc.vector.tensor_scalar(
                                    f2_sb[:, gc, qsl], pw[:, gc, :], C_W2,
                                    bias_sb["ca", "acc"][:, gc:gc + 1],
                                    OP.mult, OP.add)
                        else:
                            nc.vector.scalar_tensor_tensor(
                                f2_sb[:, :, qsl], pw[:], C_W2,
                                f2_sb[:, :, qsl], OP.mult, OP.add)
                    if h + 4 < H:
                        ca_kv_proj(h + 4, rr=[nc.gpsimd, nc.scalar])

                # ---- LN2 ----
                x2_sb = residp.tile([P, DC, SW], F32R, name="x2_sb", tag="resid")
                layernorm(f2_sb, x1_sb, x2_sb, ln_sb["ln2_b"], 1)
                x2b_sb = residp.tile([P, DC, SW], BF16, name="x2b_sb",
                                     tag="x2b", bufs=1)
                for c in range(DC):
                    nc.gpsimd.tensor_copy(x2b_sb[:, c, :], x2_sb[:, c, :])
            # ============ FFN (bf16, seq-split) ============
            with tc.tile_pool(name="ffn_h", bufs=1) as fhp:
                h_sb = fhp.tile([P, MC, SW], BF16, name="h_sb")
                f3_sb = fhp.tile([P, DC, SW], F32, name="f3_sb")
                for mc in range(MC):
                    ps = pp.tile([P, SW], F32, name=f"f1_{mc}", tag="pp")
                    for c in range(DC):
                        nc.tensor.matmul(ps[:], fc1_sb[:, c, mc * P:(mc + 1) * P],
                                         x2b_sb[:, c, :],
                                         start=(c == 0), stop=(c == DC - 1))
                    nc.scalar.activation(h_sb[:, mc, :], ps[:], AF.Relu,
                                         bias=fc1b_sb[:, mc:mc + 1])
                for gc in range(DC):
                    ps = pp.tile([P, SW], F32, name=f"f2_{gc}", tag="pp")
                    for mc in range(MC):
                        nc.tensor.matmul(ps[:],
                                         fc2_sb[:, mc, gc * P:(gc + 1) * P],
                                         h_sb[:, mc, :],
                                         start=(mc == 0), stop=(mc == MC - 1))
                    nc.vector.tensor_scalar_add(f3_sb[:, gc, :], ps[:],
                                                ffnb_sb[:, gc:gc + 1])

                # ---- LN3 + output ----
                out_sb = residp.tile([P, DC, SW], F32R, name="out_sb",
                                     tag="resid")
                layernorm(f3_sb, x2_sb, out_sb, ln_sb["ln3_b"], 2)
                for c in range(DC):
                    nc.sync.dma_start(
                        out=outt_d.ap().rearrange("(c p) s -> p c s", p=P)[:, c, :],
                        in_=out_sb[:, c, :].bitcast(F32))

            ffn_w_scope.__exit__(None, None, None)
            ca_kvw_scope.__exit__(None, None, None)
            ca_kv_scope.__exit__(None, None, None)

    nc.compile()
    return nc


def get_program():
    if "nc" not in _CACHE:
        _CACHE["nc"] = build_program()
    return _CACHE["nc"]


def kernel(**inputs) -> np.ndarray:
    from concourse.bass_utils import run_bass_kernel_spmd
    nc = get_program()
    in_maps = _host_prep(inputs)
    res = run_bass_kernel_spmd(nc, in_maps, core_ids=list(range(NC)))
    out = np.empty((B, S, D), np.float32)
    for b in range(B):
        out[b, 0:SW] = res.results[2 * b]["outt"].T
        out[b, SW:S] = res.results[2 * b + 1]["outt"].T
    return out


# revision 38
# speedup vs baseline: 1.0385x; 1.0385x over previous
"""Trainium2 Bass kernel for nn_Decoder (dense transformer decoder layer).

Problem (hardcoded): B=4, S=T=1024, D=512, H=8 heads, fp32.
  h  = MHA_self(x, causal) ; x1 = LN(h + x)
  h  = MHA_cross(x1, encod_out) ; x2 = LN(h + x1)
  ff = relu(x2 @ fc1) @ fc2 ; out = LN(ff + x2)

Sharding (8 cores = 4 batch groups x 2-core pairs), same as the f32r
baseline: self-attention tensor-parallel over heads (4 heads/core, full
S); one pair ReduceScatter (bf16) combines partial head-sums and splits
the sequence; LN1, cross-attn (all 8 heads, redundant K/V), LN2, FFN,
LN3 run sequence-parallel on the core's 512-row half.

Speed: nearly all matmuls run as fp8(e4m3) DoubleRow (contraction 256
per instruction at 0.5 cycles/row = 4x the f32r rate in the cost
model).  Numerics (validated against the fp32 reference in numpy):
  - projections/scores/AV/W2 fp8 with power-of-2 scales folded into
    ACT epilogues (x*8, w*512, Q/K/V*32, e*8, w2*2048)
  - bk dropped (exact softmax invariance); bv/bo/bf folded into acc
  - causal tail fix: attention output for early tokens is dominated by
    a single V row, so V for keys 0-255 is recomputed with bf16
    operands and query-tile 0 runs its AV/W2 path in bf16
  - FFN entirely bf16 (fp8 FFN alone costs ~1.5e-2 rel err)
  - residuals/LN in f32; ReduceScatter in bf16
Attention is query-tile-outer / head-inner so the W2 output accumulates
across all heads in one pinned PSUM tile (single epilogue per tile).
"""
import math
import numpy as np

B, S, T, D, H = 4, 1024, 1024, 512, 8
P = 128
NC = 8
DC = D // P    # 4 feature chunks
TC = T // P    # 8 time chunks
SW = 512       # per-core sequence half
QW = 256       # query tile width (DoubleRow moving limit)
MC = 2048 // P  # 16 FFN hidden chunks
EPS = 1e-5
PAIRS = [[0, 1], [2, 3], [4, 5], [6, 7]]

# fp8 scales (powers of 2; folded into f32 epilogue constants)
SX = 8.0       # x / x1 / enc quant
SWQ = 512.0    # wq/wk/wv quant
SQ = 32.0      # Q requant
SK = 32.0      # K requant
SV = 32.0      # V requant
SE = 8.0       # exp output
SW2 = 2048.0   # folded w2 quant
RSQD = 1.0 / math.sqrt(D)

_CACHE = {}


def _host_prep(inputs):
    import ml_dtypes
    F8 = ml_dtypes.float8_e4m3
    BF = ml_dtypes.bfloat16
    x = np.asarray(inputs["x"], np.float32)
    enc = np.asarray(inputs["encod_out"], np.float32)

    def q8(a, s):
        return (np.asarray(a, np.float32) * s).astype(F8)

    per_phase = {}
    for p in ("sa", "ca"):
        wq = np.asarray(inputs[p + "_wq"], np.float32)
        bq = np.asarray(inputs[p + "_bq"], np.float32)
        wk = np.asarray(inputs[p + "_wk"], np.float32)
        wv = np.asarray(inputs[p + "_wv"], np.float32)
        bv = np.asarray(inputs[p + "_bv"], np.float32)
        wo = np.asarray(inputs[p + "_wo"], np.float32)
        bo = np.asarray(inputs[p + "_bo"], np.float32)
        wf = np.asarray(inputs[p + "_wf"], np.float32).reshape(H, D, D)
        bf = np.asarray(inputs[p + "_bf"], np.float32)
        w2 = np.einsum("hfg,hgk->hfk", wo.astype(np.float64),
                       wf.astype(np.float64)).astype(np.float32)
        acc = bf.astype(np.float64).copy()
        for h in range(H):
            acc += (bv[h].astype(np.float64) @ wo[h].astype(np.float64)
                    + bo[h].astype(np.float64)) @ wf[h].astype(np.float64)
        per_phase[p] = dict(
            wq8=q8(wq, SWQ), wk8=q8(wk, SWQ), wv8=q8(wv, SWQ),
            w28=q8(w2, SW2), wv_bf=wv.astype(BF),
            bq=bq * SQ, acc=acc.astype(np.float32))

    fc1_w = np.asarray(inputs["fc1_w"], np.float32)
    fc1_b = np.asarray(inputs["fc1_b"], np.float32)
    fc2_w = np.asarray(inputs["fc2_w"], np.float32)
    fc2_b = np.asarray(inputs["fc2_b"], np.float32)
    lns = {f"ln{i}_{k}": np.asarray(inputs[f"ln{i}_{k}"], np.float32)
           for i in (1, 2, 3) for k in ("g", "b")}

    # causal masks for the two diagonal key-blocks of each 256-query tile:
    # kb 2i: keep p <= c ; kb 2i+1: keep 128+p <= c   (c in 0..255)
    pp_ = np.arange(P)[:, None]
    cc = np.arange(QW)[None, :]
    mpair = np.stack([(pp_ <= cc), (P + pp_ <= cc)], axis=1)
    ones_pair = np.ones((P, 2, 1), np.float32)

    in_maps = []
    for c in range(NC):
        b, half = c // 2, c % 2
        hs = slice(4 * half, 4 * half + 4)
        ssl = slice(half * SW, (half + 1) * SW)
        xt = x[b].T
        m = {
            "xt8": np.ascontiguousarray((xt * SX)).astype(F8),
            "xbf": np.ascontiguousarray(xt[:, :QW]).astype(BF),
            "x_res": np.ascontiguousarray(xt[:, ssl]),
            "et8": np.ascontiguousarray(enc[b].T * SX).astype(F8),
            "masks8": mpair.astype(F8),
            "ones8": ones_pair.astype(F8),
            "onesbf": np.ones((P, 1), BF),
            "ones_row": np.ones((1, P), np.float32),
            "fc1bf": fc1_w.astype(BF), "fc1_b": fc1_b,
            "fc2bf": fc2_w.astype(BF), "ffn_bias": fc2_b,
        }
        pp = per_phase["sa"]
        for k in ("wq8", "wk8", "wv8", "w28", "wv_bf", "bq"):
            m["sa_" + k] = np.ascontiguousarray(pp[k][hs])
        m["sa_acc"] = pp["acc"] / 2.0
        pp = per_phase["ca"]
        for k in ("wq8", "wk8", "wv8", "w28", "bq"):
            m["ca_" + k] = pp[k]
        m["ca_acc"] = pp["acc"]
        for k, v in lns.items():
            m[k] = v
        in_maps.append(m)
    return in_maps


def build_program():
    import concourse.bacc as bacc
    import concourse.mybir as mybir
    import concourse.tile as tile

    F32 = mybir.dt.float32
    F32R = mybir.dt.float32r
    BF16 = mybir.dt.bfloat16
    F8 = mybir.dt.float8e4
    AF = mybir.ActivationFunctionType
    OP = mybir.AluOpType
    DR = mybir.MatmulPerfMode.DoubleRow

    nc = bacc.Bacc(None, target_bir_lowering=False, num_devices=NC)

    # ---- DRAM I/O ----
    xt8_d = nc.dram_tensor("xt8", [D, S], F8, kind="ExternalInput")
    xbf_d = nc.dram_tensor("xbf", [D, QW], BF16, kind="ExternalInput")
    xres_d = nc.dram_tensor("x_res", [D, SW], F32, kind="ExternalInput")
    et8_d = nc.dram_tensor("et8", [D, T], F8, kind="ExternalInput")
    masks8_d = nc.dram_tensor("masks8", [P, 2, QW], F8, kind="ExternalInput")
    ones8_d = nc.dram_tensor("ones8", [P, 2, 1], F8, kind="ExternalInput")
    onesbf_d = nc.dram_tensor("onesbf", [P, 1], BF16, kind="ExternalInput")
    onesrow_d = nc.dram_tensor("ones_row", [1, P], F32, kind="ExternalInput")
    sa_d = {k: nc.dram_tensor("sa_" + k, [4, D, D], F8, kind="ExternalInput")
            for k in ("wq8", "wk8", "wv8", "w28")}
    sa_d["wv_bf"] = nc.dram_tensor("sa_wv_bf", [4, D, D], BF16, kind="ExternalInput")
    sa_d["bq"] = nc.dram_tensor("sa_bq", [4, D], F32, kind="ExternalInput")
    sa_d["acc"] = nc.dram_tensor("sa_acc", [D], F32, kind="ExternalInput")
    ca_d = {k: nc.dram_tensor("ca_" + k, [H, D, D], F8, kind="ExternalInput")
            for k in ("wq8", "wk8", "wv8", "w28")}
    ca_d["bq"] = nc.dram_tensor("ca_bq", [H, D], F32, kind="ExternalInput")
    ca_d["acc"] = nc.dram_tensor("ca_acc", [D], F32, kind="ExternalInput")
    fc1_d = nc.dram_tensor("fc1bf", [D, 2048], BF16, kind="ExternalInput")
    fc1b_d = nc.dram_tensor("fc1_b", [2048], F32, kind="ExternalInput")
    fc2_d = nc.dram_tensor("fc2bf", [2048, D], BF16, kind="ExternalInput")
    ffnb_d = nc.dram_tensor("ffn_bias", [D], F32, kind="ExternalInput")
    ln_d = {f"ln{i}_{k}": nc.dram_tensor(f"ln{i}_{k}", [D], F32, kind="ExternalInput")
            for i in (1, 2, 3) for k in ("g", "b")}
    outt_d = nc.dram_tensor("outt", [D, SW], F32, kind="ExternalOutput")

    r32 = lambda ap: ap.bitcast(F32R)

    # epilogue constants
    C_Q = SQ / (SX * SWQ)
    C_K = SK / (SX * SWQ)
    C_V = SV / (SX * SWQ)
    C_EXP = RSQD / (SQ * SK)     # scale on score psum inside exp
    C_W2 = 1.0 / (SV * SW2)      # scale on fp8 W2 psum
    LN_SE = math.log(SE)

    with tile.TileContext(nc, pool_alloc_mode="queue") as tc:
        with tc.tile_pool(name="const", bufs=1) as constp, \
             tc.tile_pool(name="resid", bufs=2) as residp, \
             tc.tile_pool(name="smalls", bufs=3) as smallp, \
             tc.tile_pool(name="stats", bufs=4) as statp, \
             tc.tile_pool(name="pp", bufs=2, space="PSUM") as pp, \
             tc.tile_pool(name="sc", bufs=4, space="PSUM") as scp, \
             tc.tile_pool(name="pw", bufs=1, space="PSUM") as pwp, \
             tc.tile_pool(name="dram", bufs=1, space="DRAM") as dramp:

            # ---- constants ----
            eps_sb = constp.tile([1, 1], F32, name="eps_sb")
            nc.vector.memset(eps_sb[:], EPS)
            ln8_col = constp.tile([P, 1], F32, name="ln8_col")
            nc.vector.memset(ln8_col[:], LN_SE)
            zero_col = constp.tile([P, 1], F32, name="zero_col")
            nc.vector.memset(zero_col[:], 0.0)
            ones_col = constp.tile([P, 1], F32R, name="ones_col")
            nc.vector.memset(ones_col[:], 1.0)

            xt8_sb = residp.tile([P, DC, S], F8, name="xt8_sb", tag="resid")
            nc.sync.dma_start(out=xt8_sb[:],
                              in_=xt8_d.ap().rearrange("(c p) s -> p c s", p=P))
            xbf_sb = constp.tile([P, DC, QW], BF16, name="xbf_sb")
            nc.scalar.dma_start(out=xbf_sb[:],
                                in_=xbf_d.ap().rearrange("(c p) s -> p c s", p=P))
            ca_kv_scope = tc.tile_pool(name="ca_kvp", bufs=4)
            kvpp = ca_kv_scope.__enter__()
            ca_kvw_scope = tc.tile_pool(name="ca_kvw", bufs=2)
            kvwp = ca_kvw_scope.__enter__()
            early_scope = tc.tile_pool(name="early", bufs=1)
            earlyp = early_scope.__enter__()
            xres_sb = earlyp.tile([P, DC, SW], F32, name="xres_sb", tag="xres")
            nc.scalar.dma_start(out=xres_sb[:],
                                in_=xres_d.ap().rearrange("(c p) s -> p c s", p=P))
            et8_sb = kvwp.tile([P, DC, T], F8, name="et8_sb", tag="et",
                               bufs=1)
            nc.scalar.dma_start(out=et8_sb[:],
                                in_=et8_d.ap().rearrange("(c p) s -> p c s", p=P))

            masks8_sb = constp.tile([P, 2, QW], F8, name="masks8_sb")
            nc.scalar.dma_start(out=masks8_sb[:], in_=masks8_d.ap())
            ones8_sb = constp.tile([P, 2, 1], F8, name="ones8_sb")
            nc.scalar.dma_start(out=ones8_sb[:], in_=ones8_d.ap())
            onesbf_sb = constp.tile([P, 1], BF16, name="onesbf_sb")
            nc.scalar.dma_start(out=onesbf_sb[:], in_=onesbf_d.ap())
            ones_row = constp.tile([1, P], F32R, name="ones_row")
            nc.scalar.dma_start(out=ones_row[:], in_=r32(onesrow_d.ap()))

            def vec_to_pc(dram_ap, name, nch):
                t = constp.tile([P, nch], F32, name=name)
                nc.scalar.dma_start(out=t[:],
                                    in_=dram_ap.rearrange("(c p) -> p c", p=P))
                return t

            bias_sb = {}
            for pn, dd, nh in (("sa", sa_d, 4), ("ca", ca_d, H)):
                t = constp.tile([P, nh, DC], F32, name=f"{pn}_bq_sb")
                nc.scalar.dma_start(
                    out=t[:], in_=dd["bq"].ap().rearrange("h (c p) -> p h c", p=P))
                bias_sb[pn, "bq"] = t
                bias_sb[pn, "acc"] = vec_to_pc(dd["acc"].ap(), f"{pn}_acc_sb", DC)
            grow_sb = constp.tile([1, 3, DC, P], F32R, name="ln_grow")
            for _i in (1, 2, 3):
                nc.scalar.dma_start(
                    out=grow_sb[:, _i - 1, :, :],
                    in_=r32(ln_d[f"ln{_i}_g"].ap().rearrange(
                        "(a c p) -> a c p", a=1, p=P)))
            fc1b_sb = vec_to_pc(fc1b_d.ap(), "fc1b_sb", MC)
            ffnb_sb = vec_to_pc(ffnb_d.ap(), "ffnb_sb", DC)
            ln_sb = {k: vec_to_pc(v.ap(), k + "_sb", DC) for k, v in ln_d.items()}

            cc_in = dramp.tile([2, D, SW], BF16, name="cc_in")
            cc_half = dramp.tile([D, SW], BF16, name="cc_half")

            def layernorm_half(src, resid_sb, dst, b_sb, gri, csl):
                """dst[:, :, csl] = LN(src + resid) on a column slice."""
                W = csl.stop - csl.start
                for c in range(DC):
                    nc.vector.tensor_add(dst[:, c, csl], src[:, c, csl],
                                         resid_sb[:, c, csl])
                psum_sum = pp.tile([1, SW], F32, name="ln_sum", tag="pp")
                psum_ssq = pp.tile([1, SW], F32, name="ln_ssq", tag="pp")
                for c in range(DC):
                    sq = smallp.tile([P, SW], F32R, name=f"ln_sq_{c}", tag="sm")
                    nc.scalar.activation(sq[:, :W], dst[:, c, csl], AF.Square)
                    nc.tensor.matmul(psum_sum[:, :W], ones_col[:],
                                     dst[:, c, csl],
                                     start=(c == 0), stop=(c == DC - 1))
                    nc.tensor.matmul(psum_ssq[:, :W], ones_col[:], sq[:, :W],
                                     start=(c == 0), stop=(c == DC - 1))
                mean = statp.tile([1, SW], F32R, name="ln_mean", tag="st")
                nc.scalar.activation(mean[:, :W], psum_sum[:, :W], AF.Copy,
                                     scale=1.0 / D)
                msq = statp.tile([1, SW], F32, name="ln_msq", tag="st")
                nc.scalar.activation(msq[:, :W], psum_ssq[:, :W], AF.Copy,
                                     scale=1.0 / D)
                var = statp.tile([1, SW], F32, name="ln_var", tag="st")
                nc.vector.tensor_tensor(var[:, :W], mean[:, :W], mean[:, :W],
                                        OP.mult)
                nc.vector.tensor_sub(var[:, :W], msq[:, :W], var[:, :W])
                std = statp.tile([1, SW], F32, name="ln_std", tag="st")
                nc.scalar.activation(std[:, :W], var[:, :W], AF.Sqrt,
                                     bias=eps_sb[:])
                rstd = statp.tile([1, SW], F32R, name="ln_rstd", tag="st")
                with nc.allow_low_precision(reason="f32r feed for bcast matmul"):
                    nc.vector.reciprocal(rstd[:, :W], std[:, :W])
                mr = statp.tile([1, SW], F32R, name="ln_mr", tag="st")
                nc.vector.tensor_tensor(mr[:, :W], mean[:, :W], rstd[:, :W],
                                        OP.mult)
                for c in range(DC):
                    psum_rb = pp.tile([P, SW], F32, name=f"ln_rb_{c}", tag="pp")
                    nc.tensor.matmul(psum_rb[:, :W], grow_sb[:, gri, c, :],
                                     rstd[:, :W], start=True, stop=True)
                    nc.tensor.matmul(psum_rb[:, W:2 * W],
                                     grow_sb[:, gri, c, :],
                                     mr[:, :W], start=True, stop=True)
                    tmp = smallp.tile([P, SW], F32, name=f"ln_t_{c}", tag="sm")
                    nc.vector.tensor_tensor(tmp[:, :W], dst[:, c, csl],
                                            psum_rb[:, :W], OP.mult)
                    nc.vector.scalar_tensor_tensor(
                        dst[:, c, csl], tmp[:, :W], b_sb[:, c:c + 1],
                        psum_rb[:, W:2 * W], OP.add, OP.subtract)

            def layernorm(src, resid_sb, dst, b_sb, gri):
                for hlf in range(2):
                    layernorm_half(src, resid_sb, dst, b_sb, gri,
                                   slice(hlf * QW, (hlf + 1) * QW))

            def epi_rr(eng, out_ap, ps_ap, cscale, bias_ap):
                """projection epilogue out = ps*c (+bias) on a chosen engine"""
                if eng is nc.scalar:
                    return nc.scalar.activation(
                        out_ap, ps_ap, AF.Identity, scale=cscale,
                        bias=bias_ap if bias_ap is not None else zero_col[:])
                if bias_ap is None:
                    return eng.tensor_scalar(out_ap, ps_ap, cscale, None,
                                             OP.mult)
                return eng.tensor_scalar(out_ap, ps_ap, cscale, bias_ap,
                                         OP.mult, OP.add)

            def proj_head(src_sb, wk_sb, wv_sb, wq_ap, bq_ap, kt8, v8, qt8,
                          nq, rr):
                """fp8 K^T [f,t], V [t,f], and optional Q^T [f,q] for one
                head.  Epilogues round-robin across Pool/ACT/DVE so all
                three drain in parallel."""
                last = None
                nrr = len(rr)
                ei = [0]
                def nxt():
                    e = rr[ei[0] % nrr]; ei[0] += 1
                    return e
                for fc in range(DC):
                    for th in range(2):
                        ps = pp.tile([P, SW], F32, name=f"kp_{fc}_{th}",
                                     tag="pp")
                        for nt in range(2):
                            for cp in range(2):
                                nc.tensor.matmul(
                                    ps[:, nt * QW:(nt + 1) * QW],
                                    wk_sb[:, 2 * cp:2 * cp + 2,
                                          fc * P:(fc + 1) * P],
                                    xt8_slice(src_sb, cp,
                                              th * SW + nt * QW),
                                    start=(cp == 0), stop=(cp == 1),
                                    perf_mode=DR)
                        epi_rr(nxt(), kt8[:, fc, th * SW:(th + 1) * SW],
                               ps[:], C_K, None)
                    for tci in (2 * fc, 2 * fc + 1):
                        ps = scp.tile([P, 2, QW], F32, name=f"vp_{tci}",
                                      tag="sc")
                        for fh in range(2):
                            for cp in range(2):
                                nc.tensor.matmul(
                                    ps[:, fh, :],
                                    src_sb[:, 2 * cp:2 * cp + 2,
                                           tci * P:(tci + 1) * P],
                                    wv_sb[:, 2 * cp:2 * cp + 2,
                                          fh * QW:(fh + 1) * QW],
                                    start=(cp == 0), stop=(cp == 1),
                                    perf_mode=DR)
                        last = epi_rr(nxt(), v8[:, tci, :], ps[:], C_V, None)
                    if wq_ap is not None:
                        for qh in range(nq):
                            ps = pp.tile([P, SW], F32, name=f"qp_{fc}_{qh}",
                                         tag="pp")
                            for nt in range(2):
                                for cp in range(2):
                                    nc.tensor.matmul(
                                        ps[:, nt * QW:(nt + 1) * QW],
                                        wq_ap[:, 2 * cp:2 * cp + 2,
                                              fc * P:(fc + 1) * P],
                                        xt8_slice(src_sb, cp,
                                                  qh * SW + nt * QW),
                                        start=(cp == 0), stop=(cp == 1),
                                        perf_mode=DR)
                            epi_rr(nxt(), qt8[:, fc, qh * SW:(qh + 1) * SW],
                                   ps[:], C_Q, bq_ap[:, fc:fc + 1])
                return last

            def xt8_slice(src_sb, cp, q0):
                return src_sb[:, 2 * cp:2 * cp + 2, q0:q0 + QW]

            # ============ self-attention (head-split, full S) ============
            with tc.tile_pool(name="sa_w2", bufs=1) as w2p, \
                 tc.tile_pool(name="sa_qkv", bufs=1) as qkvp, \
                 tc.tile_pool(name="sa_e", bufs=3) as ep, \
                 tc.tile_pool(name="sa_av", bufs=2) as avp, \
                 tc.tile_pool(name="sa_f", bufs=1) as fp:
                wkv_scope = tc.tile_pool(name="sa_wkv", bufs=2)
                wkvp = wkv_scope.__enter__()
                # projections for all 4 heads; per-head JIT weight DMA
                wq_sb, wk_sb, wv_sb, wvb_sb, w28_sb, w2b_sb = {}, {}, {}, {}, {}, {}
                kt8, v8, vbf, qt8 = {}, {}, {}, {}
                for h in range(4):
                    wk_sb[h] = wkvp.tile([P, DC, D], F8, name=f"sa_wk_{h}",
                                         tag="wk")
                    nc.sync.dma_start(out=wk_sb[h][:], in_=sa_d["wk8"].ap()[h]
                                      .rearrange("(c p) f -> p c f", p=P))
                    wv_sb[h] = wkvp.tile([P, DC, D], F8, name=f"sa_wv_{h}",
                                         tag="wv")
                    nc.sync.dma_start(out=wv_sb[h][:], in_=sa_d["wv8"].ap()[h]
                                      .rearrange("(c p) f -> p c f", p=P))
                    wq_sb[h] = wkvp.tile([P, DC, D], F8, name=f"sa_wq_{h}",
                                         tag="wq")
                    nc.sync.dma_start(out=wq_sb[h][:], in_=sa_d["wq8"].ap()[h]
                                      .rearrange("(c p) f -> p c f", p=P))
                    wvb_sb[h] = wkvp.tile([P, DC, D], BF16, name=f"sa_wvb_{h}",
                                          tag="wvb")
                    nc.sync.dma_start(out=wvb_sb[h][:], in_=sa_d["wv_bf"].ap()[h]
                                      .rearrange("(c p) f -> p c f", p=P))
                    kt8[h] = qkvp.tile([P, DC, S], F8, name=f"sa_kt_{h}",
                                       tag=f"kt{h}")
                    v8[h] = qkvp.tile([P, TC, D], F8, name=f"sa_v_{h}",
                                      tag=f"v{h}")
                    qt8[h] = qkvp.tile([P, DC, S], F8, name=f"sa_qt_{h}",
                                       tag=f"qt{h}")
                    proj_head(xt8_sb, wk_sb[h], wv_sb[h], wq_sb[h],
                              bias_sb["sa", "bq"][:, h, :], kt8[h], v8[h],
                              qt8[h], 2, [nc.gpsimd, nc.scalar, nc.vector])
                    # bf16 V for keys 0..255 (early-token numerics)
                    vbf[h] = qkvp.tile([P, 2, D], BF16, name=f"sa_vb_{h}",
                                       tag=f"vb{h}")
                    for tci in range(2):
                        ps = pp.tile([P, SW], F32, name=f"vbp_{h}_{tci}",
                                     tag="pp")
                        for c in range(DC):
                            nc.tensor.matmul(
                                ps[:], xbf_sb[:, c, tci * P:(tci + 1) * P],
                                wvb_sb[h][:, c, :],
                                start=(c == 0), stop=(c == DC - 1))
                        nc.vector.tensor_copy(vbf[h][:, tci, :], ps[:])
                wkv_scope.__exit__(None, None, None)
                # W2 weights arrive after the projection weights
                for h in range(4):
                    w28_sb[h] = w2p.tile([P, DC, D], F8, name=f"sa_w28_{h}",
                                         tag=f"w28{h}")
                    nc.sync.dma_start(out=w28_sb[h][:], in_=sa_d["w28"].ap()[h]
                                      .rearrange("(c p) f -> p c f", p=P))

                fbf = fp.tile([P, DC, S], BF16, name="sa_fbf")

                # attention: query-tile outer, head inner
                for qt in range(4):
                    qsl = slice(qt * QW, (qt + 1) * QW)
                    npair = qt + 1     # key-block pairs for this tile
                    bf = (qt == 0)     # bf16 AV/W2 path for queries 0..255
                    pw = pwp.tile([P, DC, QW], F32, name=f"sa_pw_{qt}", tag="pw")
                    rbs = {}
                    for h in range(4):
                        e8 = ep.tile([P, 2, QW] if bf else [P, TC, QW],
                                     BF16 if bf else F8,
                                     name=f"sa_e_{qt}_{h}",
                                     tag="ebf" if bf else "e",
                                     bufs=2 if bf else None)
                        dn = pp.tile([1, 2, QW], F32, name=f"sa_dn_{qt}_{h}",
                                     tag="pp")
                        for jp in range(npair):
                            sps = scp.tile([P, 2, QW], F32,
                                           name=f"sa_s_{qt}_{h}_{jp}", tag="sc")
                            for j2 in range(2):
                                kb = 2 * jp + j2
                                for fcp in range(2):
                                    nc.tensor.matmul(
                                        sps[:, j2, :],
                                        kt8[h][:, 2 * fcp:2 * fcp + 2,
                                               kb * P:(kb + 1) * P],
                                        qt8[h][:, 2 * fcp:2 * fcp + 2, qsl],
                                        start=(fcp == 0), stop=(fcp == 1),
                                        perf_mode=DR)
                            nc.scalar.activation(
                                e8[:, 2 * jp:2 * jp + 2, :], sps[:], AF.Exp,
                                scale=C_EXP,
                                bias=zero_col[:] if bf else ln8_col[:])
                            if jp == npair - 1:   # diagonal pair: apply mask
                                nc.gpsimd.tensor_tensor(
                                    e8[:, 2 * jp:2 * jp + 2, :],
                                    e8[:, 2 * jp:2 * jp + 2, :],
                                    masks8_sb[:], OP.mult)
                        if bf:
                            for j2 in range(2):
                                nc.tensor.matmul(
                                    dn[:, 0, :], onesbf_sb[:],
                                    e8[:, j2, :],
                                    start=(j2 == 0), stop=(j2 == 1))
                        else:
                            for jp in range(npair):
                                nc.tensor.matmul(
                                    dn[:, 0, :], ones8_sb[:],
                                    e8[:, 2 * jp:2 * jp + 2, :],
                                    start=(jp == 0), stop=(jp == npair - 1),
                                    perf_mode=DR)
                        rc = statp.tile([1, QW], F32R, name=f"sa_rc_{qt}_{h}",
                                        tag="st")
                        with nc.allow_low_precision(reason="recip bcast"):
                            nc.vector.reciprocal(rc[:], dn[:, 0, :])
                        rb = pp.tile([P, QW], F32, name=f"sa_rb_{qt}_{h}",
                                     tag="pp")
                        nc.tensor.matmul(rb[:], ones_row[:], rc[:],
                                         start=True, stop=True)
                        rbs[h] = rb

                        avn = avp.tile([P, DC, QW], BF16 if bf else F8,
                                       name=f"sa_avn_{qt}_{h}",
                                       tag="avnbf" if bf else "avn",
                                       bufs=1 if bf else None)
                        for fp2 in range(2):
                            po = scp.tile([P, 2, QW], F32,
                                          name=f"sa_po_{qt}_{h}_{fp2}",
                                          tag="sc")
                            for f2_ in range(2):
                                fc = 2 * fp2 + f2_
                                if bf:
                                    for j in range(2):
                                        nc.tensor.matmul(
                                            po[:, f2_, :],
                                            vbf[h][:, j, fc * P:(fc + 1) * P],
                                            e8[:, j, :],
                                            start=(j == 0), stop=(j == 1))
                                else:
                                    for jp in range(npair):
                                        nc.tensor.matmul(
                                            po[:, f2_, :],
                                            v8[h][:, 2 * jp:2 * jp + 2,
                                                  fc * P:(fc + 1) * P],
                                            e8[:, 2 * jp:2 * jp + 2, :],
                                            start=(jp == 0),
                                            stop=(jp == npair - 1),
                                            perf_mode=DR)
                                (nc.vector if fp2 == 0 else nc.gpsimd).tensor_tensor(
                                    avn[:, fc, :], po[:, f2_, :], rbs[h][:],
                                    OP.mult)
                        w2s = w28_sb[h]
                        for gc in range(DC):
                            for fcp in range(2):
                                if bf:
                                    # bf16 avn x fp8 w2 (no DoubleRow)
                                    for c2 in range(2):
                                        nc.tensor.matmul(
                                            pw[:, gc, :],
                                            w2s[:, 2 * fcp + c2,
                                                gc * P:(gc + 1) * P],
                                            avn[:, 2 * fcp + c2, :],
                                            start=(h == 0 and fcp == 0
                                                   and c2 == 0),
                                            stop=(h == 3 and fcp == 1
                                                  and c2 == 1))
                                else:
                                    nc.tensor.matmul(
                                        pw[:, gc, :],
                                        w2s[:, 2 * fcp:2 * fcp + 2,
                                            gc * P:(gc + 1) * P],
                                        avn[:, 2 * fcp:2 * fcp + 2, :],
                                        start=(h == 0 and fcp == 0),
                                        stop=(h == 3 and fcp == 1),
                                        perf_mode=DR)
                    # epilogue: all 4 heads accumulated; + acc bias -> bf16
                    cw = (1.0 / SW2) if bf else C_W2
                    for gc in range(DC):
                        nc.vector.tensor_scalar(
                            fbf[:, gc, qsl], pw[:, gc, :], cw,
                            bias_sb["sa", "acc"][:, gc:gc + 1],
                            OP.mult, OP.add)

                for half in range(2):
                    nc.sync.dma_start(
                        out=cc_in[half].rearrange("(c p) s -> p c s", p=P),
                        in_=fbf[:, :, half * SW:(half + 1) * SW])

            # one pair collective: reduce partial head-sums + scatter seq halves
            nc.gpsimd.collective_compute(
                "ReduceScatter", mybir.AluOpType.add, replica_groups=PAIRS,
                ins=[cc_in.opt()], outs=[cc_half.opt()])

            # ---- cross-attention K/V projections.  All 8 heads emitted into
            # a 4-deep rotating pool: the first ~3 run during the collective,
            # the rest pipeline as the head-outer attention loop frees slots.
            ln1_anchor = [None]
            ca_kt8, ca_v8 = {}, {}

            def ca_kv_proj(h, rr=None):
                wk_s = kvwp.tile([P, DC, D], F8, name=f"ca_wk_{h}", tag="wk")
                nc.sync.dma_start(out=wk_s[:], in_=ca_d["wk8"].ap()[h]
                                  .rearrange("(c p) f -> p c f", p=P))
                wv_s = kvwp.tile([P, DC, D], F8, name=f"ca_wv_{h}", tag="wv")
                nc.sync.dma_start(out=wv_s[:], in_=ca_d["wv8"].ap()[h]
                                  .rearrange("(c p) f -> p c f", p=P))
                ca_kt8[h] = kvpp.tile([P, DC, T], F8, name=f"ca_ktp_{h}",
                                      tag="cktp")
                ca_v8[h] = kvpp.tile([P, TC, D], F8, name=f"ca_vp_{h}",
                                     tag="cvp")
                return proj_head(et8_sb, wk_s, wv_s, None, None,
                                 ca_kt8[h], ca_v8[h], None, 0,
                                 rr or [nc.gpsimd, nc.scalar, nc.vector])

            for h in range(4):
                last = ca_kv_proj(h)
                if h == 2:
                    ln1_anchor[0] = last

            # ---- LN1 on my sequence half ----
            cch_sb = earlyp.tile([P, DC, SW], BF16, name="cch_sb",
                                 tag="cch")
            nc.sync.dma_start(
                out=cch_sb[:],
                in_=cc_half.opt().rearrange("(c p) s -> p c s", p=P))
            x1_sb = residp.tile([P, DC, SW], F32R, name="x1_sb", tag="resid")
            from concourse.tile import add_dep_helper as _adh
            _bb = nc.main_func.blocks[-1]
            _n0 = len(_bb.instructions)
            layernorm(cch_sb, xres_sb, x1_sb, ln_sb["ln1_b"], 0)
            x18_sb = residp.tile([P, DC, SW], F8, name="x18_sb", tag="x18",
                                 bufs=1)
            for c in range(DC):
                nc.scalar.activation(x18_sb[:, c, :], x1_sb[:, c, :],
                                     AF.Copy, scale=SX)
            if ln1_anchor[0] is not None:
                for _ins in list(_bb.instructions)[_n0:]:
                    _adh(_ins, ln1_anchor[0].ins, sync=False,
                         reason="order LN1 after CA-KV precompute h2")
            early_scope.__exit__(None, None, None)

            # FFN weights prefetch (transfers overlap CA attention)
            ffn_w_scope = tc.tile_pool(name="ffn_w", bufs=1)
            fwp = ffn_w_scope.__enter__()
            fc1_sb = fwp.tile([P, DC, 2048], BF16, name="fc1_sb")
            fc2_sb = fwp.tile([P, MC, D], BF16, name="fc2_sb")

            # ============ cross-attention (seq-split, all heads) ============
            with tc.tile_pool(name="ca_w", bufs=1) as cwp, \
                 tc.tile_pool(name="ca_qt", bufs=1) as cqtp, \
                 tc.tile_pool(name="ca_e", bufs=2) as ep, \
                 tc.tile_pool(name="ca_av", bufs=2) as avp, \
                 tc.tile_pool(name="ca_f", bufs=1) as fp:
                wq_c, w2_c = {}, {}
                for h in range(H):
                    w2_c[h] = cwp.tile([P, DC, D], F8, name=f"ca_w2_{h}",
                                       tag=f"cw2{h}")
                    nc.sync.dma_start(out=w2_c[h][:], in_=ca_d["w28"].ap()[h]
                                      .rearrange("(c p) f -> p c f", p=P))
                # wq tiles 2-buffered: freed as Q projections complete
                cwqp_scope = tc.tile_pool(name="ca_wq", bufs=2)
                cwqp = cwqp_scope.__enter__()
                for h in range(H):
                    wq_c[h] = cwqp.tile([P, DC, D], F8, name=f"ca_wq_{h}",
                                        tag="cwq")
                    nc.sync.dma_start(out=wq_c[h][:], in_=ca_d["wq8"].ap()[h]
                                      .rearrange("(c p) f -> p c f", p=P))
                for mg in range(4):
                    nc.sync.dma_start(
                        out=fc1_sb[:, :, mg * SW:(mg + 1) * SW],
                        in_=fc1_d.ap().rearrange("(c p) m -> p c m", p=P)
                        [:, :, mg * SW:(mg + 1) * SW])
                nc.sync.dma_start(out=fc2_sb[:], in_=fc2_d.ap()
                                  .rearrange("(c p) g -> p c g", p=P))
                qt8_c = {}
                for h in range(H):
                    qt8_c[h] = cqtp.tile([P, DC, SW], F8, name=f"ca_qt_{h}",
                                         tag=f"cq{h}")
                    _ = wq_c[h]  # DMA emitted above; tiles rotate via pool
                    for fc in range(DC):
                        ps = pp.tile([P, SW], F32, name=f"cqp_{h}_{fc}",
                                     tag="pp")
                        for nt in range(2):
                            for cp in range(2):
                                nc.tensor.matmul(
                                    ps[:, nt * QW:(nt + 1) * QW],
                                    wq_c[h][:, 2 * cp:2 * cp + 2,
                                            fc * P:(fc + 1) * P],
                                    x18_sb[:, 2 * cp:2 * cp + 2,
                                           nt * QW:(nt + 1) * QW],
                                    start=(cp == 0), stop=(cp == 1),
                                    perf_mode=DR)
                        (nc.gpsimd if h % 2 else nc.vector).tensor_scalar(
                            qt8_c[h][:, fc, :], ps[:], C_Q,
                            bias_sb["ca", "bq"][:, h, fc:fc + 1],
                            OP.mult, OP.add)
                cwqp_scope.__exit__(None, None, None)

                f2_sb = fp.tile([P, DC, SW], F32, name="ca_f2")
                for h in range(H):
                    e8 = ep.tile([P, 2 * TC, QW], F8, name=f"ca_e_{h}",
                                 tag="e")
                    dn = pp.tile([1, 2, QW], F32, name=f"ca_dn_{h}", tag="pp")
                    rb = pp.tile([P, 2, QW], F32, name=f"ca_rb_{h}", tag="pp")
                    for qt in range(2):
                        qsl = slice(qt * QW, (qt + 1) * QW)
                        for jp in range(4):
                            sps = scp.tile([P, 2, QW], F32,
                                           name=f"ca_s_{qt}_{h}_{jp}", tag="sc")
                            for j2 in range(2):
                                kb = 2 * jp + j2
                                for fcp in range(2):
                                    nc.tensor.matmul(
                                        sps[:, j2, :],
                                        ca_kt8[h][:, 2 * fcp:2 * fcp + 2,
                                                  kb * P:(kb + 1) * P],
                                        qt8_c[h][:, 2 * fcp:2 * fcp + 2, qsl],
                                        start=(fcp == 0), stop=(fcp == 1),
                                        perf_mode=DR)
                            nc.scalar.activation(
                                e8[:, 8 * qt + 2 * jp:8 * qt + 2 * jp + 2, :],
                                sps[:], AF.Exp, scale=C_EXP, bias=ln8_col[:])
                    # denominators after the scores loop (avoids PE
                    # head-of-line wait on each exp)
                    for qt in range(2):
                        for jp in range(4):
                            nc.tensor.matmul(
                                dn[:, qt, :], ones8_sb[:],
                                e8[:, 8 * qt + 2 * jp:8 * qt + 2 * jp + 2, :],
                                start=(jp == 0), stop=(jp == 3),
                                perf_mode=DR)
                    rc = statp.tile([1, 2, QW], F32R, name=f"ca_rc_{h}",
                                    tag="st")
                    with nc.allow_low_precision(reason="recip bcast"):
                        nc.vector.reciprocal(rc[:], dn[:])
                    for qt in range(2):
                        nc.tensor.matmul(rb[:, qt, :], ones_row[:],
                                         rc[:, qt, :], start=True, stop=True)
                    for qt in range(2):
                        qsl = slice(qt * QW, (qt + 1) * QW)
                        avn = avp.tile([P, DC, QW], F8,
                                       name=f"ca_avn_{qt}_{h}", tag="avn")
                        for fp2 in range(2):
                            po = scp.tile([P, 2, QW], F32,
                                          name=f"ca_po_{qt}_{h}_{fp2}",
                                          tag="sc")
                            for f2_ in range(2):
                                fc = 2 * fp2 + f2_
                                for jp in range(4):
                                    nc.tensor.matmul(
                                        po[:, f2_, :],
                                        ca_v8[h][:, 2 * jp:2 * jp + 2,
                                                 fc * P:(fc + 1) * P],
                                        e8[:, 8 * qt + 2 * jp:8 * qt + 2 * jp + 2, :],
                                        start=(jp == 0), stop=(jp == 3),
                                        perf_mode=DR)
                                (nc.vector if fp2 == 0 else nc.gpsimd).tensor_tensor(
                                    avn[:, fc, :], po[:, f2_, :], rb[:, qt, :],
                                    OP.mult)
                        pw = pwp.tile([P, DC, QW], F32, name=f"ca_pw_{qt}_{h}",
                                      tag="pw")
                        for gc in range(DC):
                            for fcp in range(2):
                                nc.tensor.matmul(
                                    pw[:, gc, :],
                                    w2_c[h][:, 2 * fcp:2 * fcp + 2,
                                            gc * P:(gc + 1) * P],
                                    avn[:, 2 * fcp:2 * fcp + 2, :],
                                    start=(fcp == 0), stop=(fcp == 1),
                                    perf_mode=DR)
                        if h == 0:
                            for gc in range(DC):
                                nc.vector.tensor_scalar(
                                    f2_sb[:, gc, qsl], pw[:, gc, :], C_W2,
                                    bias_sb["ca", "acc"][:, gc:gc + 1],
                                    OP.mult, OP.add)
                        else:
                            nc.vector.scalar_tensor_tensor(
                                f2_sb[:, :, qsl], pw[:], C_W2,
                                f2_sb[:, :, qsl], OP.mult, OP.add)
                    if h + 4 < H:
                        ca_kv_proj(h + 4, rr=[nc.gpsimd, nc.scalar])

                # ---- LN2 ----
                x2_sb = residp.tile([P, DC, SW], F32R, name="x2_sb", tag="resid")
                layernorm(f2_sb, x1_sb, x2_sb, ln_sb["ln2_b"], 1)
                x2b_sb = residp.tile([P, DC, SW], BF16, name="x2b_sb",
                                     tag="x2b", bufs=1)
                for c in range(DC):
                    nc.gpsimd.tensor_copy(x2b_sb[:, c, :], x2_sb[:, c, :])
            # ============ FFN (bf16, seq-split) ============
            with tc.tile_pool(name="ffn_h", bufs=1) as fhp:
                h_sb = fhp.tile([P, MC, SW], BF16, name="h_sb")
                f3_sb = fhp.tile([P, DC, SW], F32, name="f3_sb")
                for mc in range(MC):
                    ps = pp.tile([P, SW], F32, name=f"f1_{mc}", tag="pp")
                    for c in range(DC):
                        nc.tensor.matmul(ps[:], fc1_sb[:, c, mc * P:(mc + 1) * P],
                                         x2b_sb[:, c, :],
                                         start=(c == 0), stop=(c == DC - 1))
                    nc.scalar.activation(h_sb[:, mc, :], ps[:], AF.Relu,
                                         bias=fc1b_sb[:, mc:mc + 1])
                for gc in range(DC):
                    ps = pp.tile([P, SW], F32, name=f"f2_{gc}", tag="pp")
                    for mc in range(MC):
                        nc.tensor.matmul(ps[:],
                                         fc2_sb[:, mc, gc * P:(gc + 1) * P],
                                         h_sb[:, mc, :],
                                         start=(mc == 0), stop=(mc == MC - 1))
                    nc.vector.tensor_scalar_add(f3_sb[:, gc, :], ps[:],
                                                ffnb_sb[:, gc:gc + 1])

                # ---- LN3 + output ----
                out_sb = residp.tile([P, DC, SW], F32R, name="out_sb",
                                     tag="resid")
                layernorm(f3_sb, x2_sb, out_sb, ln_sb["ln3_b"], 2)
                for c in range(DC):
                    nc.sync.dma_start(
                        out=outt_d.ap().rearrange("(c p) s -> p c s", p=P)[:, c, :],
                        in_=out_sb[:, c, :].bitcast(F32))

            ffn_w_scope.__exit__(None, None, None)
            ca_kvw_scope.__exit__(None, None, None)
            ca_kv_scope.__exit__(None, None, None)

    nc.compile()
    return nc


def get_program():
    if "nc" not in _CACHE:
        _CACHE["nc"] = build_program()
    return _CACHE["nc"]


def kernel(**inputs) -> np.ndarray:
    from concourse.bass_utils import run_bass_kernel_spmd
    nc = get_program()
    in_maps = _host_prep(inputs)
    res = run_bass_kernel_spmd(nc, in_maps, core_ids=list(range(NC)))
    out = np.empty((B, S, D), np.float32)
    for b in range(B):
        out[b, 0:SW] = res.results[2 * b]["outt"].T
        out[b, SW:S] = res.results[2 * b + 1]["outt"].T
    return out


# revision 39
# speedup vs baseline: 1.0851x; 1.0448x over previous
"""Trainium2 Bass kernel for nn_Decoder (dense transformer decoder layer).

Problem (hardcoded): B=4, S=T=1024, D=512, H=8 heads, fp32.
  h  = MHA_self(x, causal) ; x1 = LN(h + x)
  h  = MHA_cross(x1, encod_out) ; x2 = LN(h + x1)
  ff = relu(x2 @ fc1) @ fc2 ; out = LN(ff + x2)

Sharding (8 cores = 4 batch groups x 2-core pairs), same as the f32r
baseline: self-attention tensor-parallel over heads (4 heads/core, full
S); one pair ReduceScatter (bf16) combines partial head-sums and splits
the sequence; LN1, cross-attn (all 8 heads, redundant K/V), LN2, FFN,
LN3 run sequence-parallel on the core's 512-row half.

Speed: nearly all matmuls run as fp8(e4m3) DoubleRow (contraction 256
per instruction at 0.5 cycles/row = 4x the f32r rate in the cost
model).  Numerics (validated against the fp32 reference in numpy):
  - projections/scores/AV/W2 fp8 with power-of-2 scales folded into
    ACT epilogues (x*8, w*512, Q/K/V*32, e*8, w2*2048)
  - bk dropped (exact softmax invariance); bv/bo/bf folded into acc
  - causal tail fix: attention output for early tokens is dominated by
    a single V row, so V for keys 0-255 is recomputed with bf16
    operands and query-tile 0 runs its AV/W2 path in bf16
  - FFN entirely bf16 (fp8 FFN alone costs ~1.5e-2 rel err)
  - residuals/LN in f32; ReduceScatter in bf16
Attention is query-tile-outer / head-inner so the W2 output accumulates
across all heads in one pinned PSUM tile (single epilogue per tile).
"""
import math
import numpy as np

B, S, T, D, H = 4, 1024, 1024, 512, 8
P = 128
NC = 8
DC = D // P    # 4 feature chunks
TC = T // P    # 8 time chunks
SW = 512       # per-core sequence half
QW = 256       # query tile width (DoubleRow moving limit)
MC = 2048 // P  # 16 FFN hidden chunks
EPS = 1e-5
PAIRS = [[0, 1], [2, 3], [4, 5], [6, 7]]

# fp8 scales (powers of 2; folded into f32 epilogue constants)
SX = 8.0       # x / x1 / enc quant
SWQ = 512.0    # wq/wk/wv quant
SQ = 32.0      # Q requant
SK = 32.0      # K requant
SV = 32.0      # V requant
SE = 8.0       # exp output
SW2 = 2048.0   # folded w2 quant
RSQD = 1.0 / math.sqrt(D)

_CACHE = {}


def _host_prep(inputs):
    import ml_dtypes
    F8 = ml_dtypes.float8_e4m3
    BF = ml_dtypes.bfloat16
    x = np.asarray(inputs["x"], np.float32)
    enc = np.asarray(inputs["encod_out"], np.float32)

    def q8(a, s):
        return (np.asarray(a, np.float32) * s).astype(F8)

    per_phase = {}
    for p in ("sa", "ca"):
        wq = np.asarray(inputs[p + "_wq"], np.float32)
        bq = np.asarray(inputs[p + "_bq"], np.float32)
        wk = np.asarray(inputs[p + "_wk"], np.float32)
        wv = np.asarray(inputs[p + "_wv"], np.float32)
        bv = np.asarray(inputs[p + "_bv"], np.float32)
        wo = np.asarray(inputs[p + "_wo"], np.float32)
        bo = np.asarray(inputs[p + "_bo"], np.float32)
        wf = np.asarray(inputs[p + "_wf"], np.float32).reshape(H, D, D)
        bf = np.asarray(inputs[p + "_bf"], np.float32)
        w2 = np.einsum("hfg,hgk->hfk", wo.astype(np.float64),
                       wf.astype(np.float64)).astype(np.float32)
        acc = bf.astype(np.float64).copy()
        for h in range(H):
            acc += (bv[h].astype(np.float64) @ wo[h].astype(np.float64)
                    + bo[h].astype(np.float64)) @ wf[h].astype(np.float64)
        per_phase[p] = dict(
            wq8=q8(wq, SWQ), wk8=q8(wk, SWQ), wv8=q8(wv, SWQ),
            w28=q8(w2, SW2), wv_bf=wv.astype(BF),
            bq=bq * SQ, acc=acc.astype(np.float32))

    fc1_w = np.asarray(inputs["fc1_w"], np.float32)
    fc1_b = np.asarray(inputs["fc1_b"], np.float32)
    fc2_w = np.asarray(inputs["fc2_w"], np.float32)
    fc2_b = np.asarray(inputs["fc2_b"], np.float32)
    lns = {f"ln{i}_{k}": np.asarray(inputs[f"ln{i}_{k}"], np.float32)
           for i in (1, 2, 3) for k in ("g", "b")}

    # causal masks for the two diagonal key-blocks of each 256-query tile:
    # kb 2i: keep p <= c ; kb 2i+1: keep 128+p <= c   (c in 0..255)
    pp_ = np.arange(P)[:, None]
    cc = np.arange(QW)[None, :]
    mpair = np.stack([(pp_ <= cc), (P + pp_ <= cc)], axis=1)
    ones_pair = np.ones((P, 2, 1), np.float32)

    in_maps = []
    for c in range(NC):
        b, half = c // 2, c % 2
        hs = slice(4 * half, 4 * half + 4)
        ssl = slice(half * SW, (half + 1) * SW)
        xt = x[b].T
        m = {
            "xt8": np.ascontiguousarray((xt * SX)).astype(F8),
            "xbf": np.ascontiguousarray(xt[:, :QW]).astype(BF),
            "x_res": np.ascontiguousarray(xt[:, ssl]),
            "et8": np.ascontiguousarray(enc[b].T * SX).astype(F8),
            "masks8": mpair.astype(F8),
            "ones8": ones_pair.astype(F8),
            "onesbf": np.ones((P, 1), BF),
            "ones_row": np.ones((1, P), np.float32),
            "fc1bf": fc1_w.astype(BF), "fc1_b": fc1_b,
            "fc2bf": fc2_w.astype(BF), "ffn_bias": fc2_b,
        }
        pp = per_phase["sa"]
        for k in ("wq8", "wk8", "wv8", "w28", "wv_bf", "bq"):
            m["sa_" + k] = np.ascontiguousarray(pp[k][hs])
        m["sa_acc"] = pp["acc"] / 2.0
        pp = per_phase["ca"]
        for k in ("wq8", "wk8", "wv8", "w28", "bq"):
            m["ca_" + k] = pp[k]
        m["ca_acc"] = pp["acc"]
        for k, v in lns.items():
            m[k] = v
        in_maps.append(m)
    return in_maps


def build_program():
    import concourse.bacc as bacc
    import concourse.mybir as mybir
    import concourse.tile as tile

    F32 = mybir.dt.float32
    F32R = mybir.dt.float32r
    BF16 = mybir.dt.bfloat16
    F8 = mybir.dt.float8e4
    AF = mybir.ActivationFunctionType
    OP = mybir.AluOpType
    DR = mybir.MatmulPerfMode.DoubleRow

    nc = bacc.Bacc(None, target_bir_lowering=False, num_devices=NC)

    # ---- DRAM I/O ----
    xt8_d = nc.dram_tensor("xt8", [D, S], F8, kind="ExternalInput")
    xbf_d = nc.dram_tensor("xbf", [D, QW], BF16, kind="ExternalInput")
    xres_d = nc.dram_tensor("x_res", [D, SW], F32, kind="ExternalInput")
    et8_d = nc.dram_tensor("et8", [D, T], F8, kind="ExternalInput")
    masks8_d = nc.dram_tensor("masks8", [P, 2, QW], F8, kind="ExternalInput")
    ones8_d = nc.dram_tensor("ones8", [P, 2, 1], F8, kind="ExternalInput")
    onesbf_d = nc.dram_tensor("onesbf", [P, 1], BF16, kind="ExternalInput")
    onesrow_d = nc.dram_tensor("ones_row", [1, P], F32, kind="ExternalInput")
    sa_d = {k: nc.dram_tensor("sa_" + k, [4, D, D], F8, kind="ExternalInput")
            for k in ("wq8", "wk8", "wv8", "w28")}
    sa_d["wv_bf"] = nc.dram_tensor("sa_wv_bf", [4, D, D], BF16, kind="ExternalInput")
    sa_d["bq"] = nc.dram_tensor("sa_bq", [4, D], F32, kind="ExternalInput")
    sa_d["acc"] = nc.dram_tensor("sa_acc", [D], F32, kind="ExternalInput")
    ca_d = {k: nc.dram_tensor("ca_" + k, [H, D, D], F8, kind="ExternalInput")
            for k in ("wq8", "wk8", "wv8", "w28")}
    ca_d["bq"] = nc.dram_tensor("ca_bq", [H, D], F32, kind="ExternalInput")
    ca_d["acc"] = nc.dram_tensor("ca_acc", [D], F32, kind="ExternalInput")
    fc1_d = nc.dram_tensor("fc1bf", [D, 2048], BF16, kind="ExternalInput")
    fc1b_d = nc.dram_tensor("fc1_b", [2048], F32, kind="ExternalInput")
    fc2_d = nc.dram_tensor("fc2bf", [2048, D], BF16, kind="ExternalInput")
    ffnb_d = nc.dram_tensor("ffn_bias", [D], F32, kind="ExternalInput")
    ln_d = {f"ln{i}_{k}": nc.dram_tensor(f"ln{i}_{k}", [D], F32, kind="ExternalInput")
            for i in (1, 2, 3) for k in ("g", "b")}
    outt_d = nc.dram_tensor("outt", [D, SW], F32, kind="ExternalOutput")

    r32 = lambda ap: ap.bitcast(F32R)

    # epilogue constants
    C_Q = SQ / (SX * SWQ)
    C_K = SK / (SX * SWQ)
    C_V = SV / (SX * SWQ)
    C_EXP = RSQD / (SQ * SK)     # scale on score psum inside exp
    C_W2 = 1.0 / (SV * SW2)      # scale on fp8 W2 psum
    LN_SE = math.log(SE)

    with tile.TileContext(nc, pool_alloc_mode="queue") as tc:
        with tc.tile_pool(name="const", bufs=1) as constp, \
             tc.tile_pool(name="resid", bufs=2) as residp, \
             tc.tile_pool(name="smalls", bufs=3) as smallp, \
             tc.tile_pool(name="stats", bufs=4) as statp, \
             tc.tile_pool(name="pp", bufs=2, space="PSUM") as pp, \
             tc.tile_pool(name="sc", bufs=4, space="PSUM") as scp, \
             tc.tile_pool(name="pw", bufs=1, space="PSUM") as pwp, \
             tc.tile_pool(name="dram", bufs=1, space="DRAM") as dramp:

            # ---- constants ----
            eps_sb = constp.tile([1, 1], F32, name="eps_sb")
            nc.vector.memset(eps_sb[:], EPS)
            ln8_col = constp.tile([P, 1], F32, name="ln8_col")
            nc.vector.memset(ln8_col[:], LN_SE)
            zero_col = constp.tile([P, 1], F32, name="zero_col")
            nc.vector.memset(zero_col[:], 0.0)
            ones_col = constp.tile([P, 1], F32R, name="ones_col")
            nc.vector.memset(ones_col[:], 1.0)

            xt8_sb = residp.tile([P, DC, S], F8, name="xt8_sb", tag="resid")
            nc.sync.dma_start(out=xt8_sb[:],
                              in_=xt8_d.ap().rearrange("(c p) s -> p c s", p=P))
            xbf_sb = constp.tile([P, DC, QW], BF16, name="xbf_sb")
            nc.scalar.dma_start(out=xbf_sb[:],
                                in_=xbf_d.ap().rearrange("(c p) s -> p c s", p=P))
            ca_kv_scope = tc.tile_pool(name="ca_kvp", bufs=4)
            kvpp = ca_kv_scope.__enter__()
            ca_kvw_scope = tc.tile_pool(name="ca_kvw", bufs=2)
            kvwp = ca_kvw_scope.__enter__()
            early_scope = tc.tile_pool(name="early", bufs=1)
            earlyp = early_scope.__enter__()
            xres_sb = earlyp.tile([P, DC, SW], F32, name="xres_sb", tag="xres")
            nc.scalar.dma_start(out=xres_sb[:],
                                in_=xres_d.ap().rearrange("(c p) s -> p c s", p=P))
            et8_sb = kvwp.tile([P, DC, T], F8, name="et8_sb", tag="et",
                               bufs=1)
            nc.scalar.dma_start(out=et8_sb[:],
                                in_=et8_d.ap().rearrange("(c p) s -> p c s", p=P))

            masks8_sb = constp.tile([P, 2, QW], F8, name="masks8_sb")
            nc.scalar.dma_start(out=masks8_sb[:], in_=masks8_d.ap())
            ones8_sb = constp.tile([P, 2, 1], F8, name="ones8_sb")
            nc.scalar.dma_start(out=ones8_sb[:], in_=ones8_d.ap())
            onesbf_sb = constp.tile([P, 1], BF16, name="onesbf_sb")
            nc.scalar.dma_start(out=onesbf_sb[:], in_=onesbf_d.ap())
            ones_row = constp.tile([1, P], F32R, name="ones_row")
            nc.scalar.dma_start(out=ones_row[:], in_=r32(onesrow_d.ap()))

            def vec_to_pc(dram_ap, name, nch):
                t = constp.tile([P, nch], F32, name=name)
                nc.scalar.dma_start(out=t[:],
                                    in_=dram_ap.rearrange("(c p) -> p c", p=P))
                return t

            bias_sb = {}
            for pn, dd, nh in (("sa", sa_d, 4), ("ca", ca_d, H)):
                t = constp.tile([P, nh, DC], F32, name=f"{pn}_bq_sb")
                nc.scalar.dma_start(
                    out=t[:], in_=dd["bq"].ap().rearrange("h (c p) -> p h c", p=P))
                bias_sb[pn, "bq"] = t
                bias_sb[pn, "acc"] = vec_to_pc(dd["acc"].ap(), f"{pn}_acc_sb", DC)
            grow_sb = constp.tile([1, 3, DC, P], F32R, name="ln_grow")
            for _i in (1, 2, 3):
                nc.scalar.dma_start(
                    out=grow_sb[:, _i - 1, :, :],
                    in_=r32(ln_d[f"ln{_i}_g"].ap().rearrange(
                        "(a c p) -> a c p", a=1, p=P)))
            fc1b_sb = vec_to_pc(fc1b_d.ap(), "fc1b_sb", MC)
            ffnb_sb = vec_to_pc(ffnb_d.ap(), "ffnb_sb", DC)
            ln_sb = {k: vec_to_pc(v.ap(), k + "_sb", DC) for k, v in ln_d.items()}

            cc_in = dramp.tile([2, D, SW], BF16, name="cc_in")
            cc_half = dramp.tile([D, SW], BF16, name="cc_half")

            def layernorm_half(src, resid_sb, dst, b_sb, gri, csl):
                """dst[:, :, csl] = LN(src + resid) on a column slice."""
                W = csl.stop - csl.start
                for c in range(DC):
                    nc.vector.tensor_add(dst[:, c, csl], src[:, c, csl],
                                         resid_sb[:, c, csl])
                psum_sum = pp.tile([1, SW], F32, name="ln_sum", tag="pp")
                psum_ssq = pp.tile([1, SW], F32, name="ln_ssq", tag="pp")
                for c in range(DC):
                    sq = smallp.tile([P, SW], F32R, name=f"ln_sq_{c}", tag="sm")
                    nc.scalar.activation(sq[:, :W], dst[:, c, csl], AF.Square)
                    nc.tensor.matmul(psum_sum[:, :W], ones_col[:],
                                     dst[:, c, csl],
                                     start=(c == 0), stop=(c == DC - 1))
                    nc.tensor.matmul(psum_ssq[:, :W], ones_col[:], sq[:, :W],
                                     start=(c == 0), stop=(c == DC - 1))
                mean = statp.tile([1, SW], F32R, name="ln_mean", tag="st")
                nc.scalar.activation(mean[:, :W], psum_sum[:, :W], AF.Copy,
                                     scale=1.0 / D)
                msq = statp.tile([1, SW], F32, name="ln_msq", tag="st")
                nc.scalar.activation(msq[:, :W], psum_ssq[:, :W], AF.Copy,
                                     scale=1.0 / D)
                var = statp.tile([1, SW], F32, name="ln_var", tag="st")
                nc.vector.tensor_tensor(var[:, :W], mean[:, :W], mean[:, :W],
                                        OP.mult)
                nc.vector.tensor_sub(var[:, :W], msq[:, :W], var[:, :W])
                std = statp.tile([1, SW], F32, name="ln_std", tag="st")
                nc.scalar.activation(std[:, :W], var[:, :W], AF.Sqrt,
                                     bias=eps_sb[:])
                rstd = statp.tile([1, SW], F32R, name="ln_rstd", tag="st")
                with nc.allow_low_precision(reason="f32r feed for bcast matmul"):
                    nc.vector.reciprocal(rstd[:, :W], std[:, :W])
                mr = statp.tile([1, SW], F32R, name="ln_mr", tag="st")
                nc.vector.tensor_tensor(mr[:, :W], mean[:, :W], rstd[:, :W],
                                        OP.mult)
                for c in range(DC):
                    psum_rb = pp.tile([P, SW], F32, name=f"ln_rb_{c}", tag="pp")
                    nc.tensor.matmul(psum_rb[:, :W], grow_sb[:, gri, c, :],
                                     rstd[:, :W], start=True, stop=True)
                    nc.tensor.matmul(psum_rb[:, W:2 * W],
                                     grow_sb[:, gri, c, :],
                                     mr[:, :W], start=True, stop=True)
                    tmp = smallp.tile([P, SW], F32, name=f"ln_t_{c}", tag="sm")
                    nc.vector.tensor_tensor(tmp[:, :W], dst[:, c, csl],
                                            psum_rb[:, :W], OP.mult)
                    nc.vector.scalar_tensor_tensor(
                        dst[:, c, csl], tmp[:, :W], b_sb[:, c:c + 1],
                        psum_rb[:, W:2 * W], OP.add, OP.subtract)

            def layernorm(src, resid_sb, dst, b_sb, gri):
                for hlf in range(2):
                    layernorm_half(src, resid_sb, dst, b_sb, gri,
                                   slice(hlf * QW, (hlf + 1) * QW))

            def epi_rr(eng, out_ap, ps_ap, cscale, bias_ap):
                """projection epilogue out = ps*c (+bias) on a chosen engine"""
                if eng is nc.scalar:
                    return nc.scalar.activation(
                        out_ap, ps_ap, AF.Identity, scale=cscale,
                        bias=bias_ap if bias_ap is not None else zero_col[:])
                if bias_ap is None:
                    return eng.tensor_scalar(out_ap, ps_ap, cscale, None,
                                             OP.mult)
                return eng.tensor_scalar(out_ap, ps_ap, cscale, bias_ap,
                                         OP.mult, OP.add)

            def proj_head(src_sb, wk_sb, wv_sb, wq_ap, bq_ap, kt8, v8, qt8,
                          nq, rr):
                """fp8 K^T [f,t], V [t,f], and optional Q^T [f,q] for one
                head.  Epilogues round-robin across Pool/ACT/DVE so all
                three drain in parallel."""
                last = None
                nrr = len(rr)
                ei = [0]
                def nxt():
                    e = rr[ei[0] % nrr]; ei[0] += 1
                    return e
                for fc in range(DC):
                    for th in range(2):
                        ps = pp.tile([P, SW], F32, name=f"kp_{fc}_{th}",
                                     tag="pp")
                        for nt in range(2):
                            for cp in range(2):
                                nc.tensor.matmul(
                                    ps[:, nt * QW:(nt + 1) * QW],
                                    wk_sb[:, 2 * cp:2 * cp + 2,
                                          fc * P:(fc + 1) * P],
                                    xt8_slice(src_sb, cp,
                                              th * SW + nt * QW),
                                    start=(cp == 0), stop=(cp == 1),
                                    perf_mode=DR)
                        epi_rr(nxt(), kt8[:, fc, th * SW:(th + 1) * SW],
                               ps[:], C_K, None)
                    for tci in (2 * fc, 2 * fc + 1):
                        ps = scp.tile([P, 2, QW], F32, name=f"vp_{tci}",
                                      tag="sc")
                        for fh in range(2):
                            for cp in range(2):
                                nc.tensor.matmul(
                                    ps[:, fh, :],
                                    src_sb[:, 2 * cp:2 * cp + 2,
                                           tci * P:(tci + 1) * P],
                                    wv_sb[:, 2 * cp:2 * cp + 2,
                                          fh * QW:(fh + 1) * QW],
                                    start=(cp == 0), stop=(cp == 1),
                                    perf_mode=DR)
                        last = epi_rr(nxt(), v8[:, tci, :], ps[:], C_V, None)
                    if wq_ap is not None:
                        for qh in range(nq):
                            ps = pp.tile([P, SW], F32, name=f"qp_{fc}_{qh}",
                                         tag="pp")
                            for nt in range(2):
                                for cp in range(2):
                                    nc.tensor.matmul(
                                        ps[:, nt * QW:(nt + 1) * QW],
                                        wq_ap[:, 2 * cp:2 * cp + 2,
                                              fc * P:(fc + 1) * P],
                                        xt8_slice(src_sb, cp,
                                                  qh * SW + nt * QW),
                                        start=(cp == 0), stop=(cp == 1),
                                        perf_mode=DR)
                            epi_rr(nxt(), qt8[:, fc, qh * SW:(qh + 1) * SW],
                                   ps[:], C_Q, bq_ap[:, fc:fc + 1])
                return last

            def xt8_slice(src_sb, cp, q0):
                return src_sb[:, 2 * cp:2 * cp + 2, q0:q0 + QW]

            # ============ self-attention (head-split, full S) ============
            with tc.tile_pool(name="sa_w2", bufs=1) as w2p, \
                 tc.tile_pool(name="sa_qkv", bufs=1) as qkvp, \
                 tc.tile_pool(name="sa_e", bufs=3) as ep, \
                 tc.tile_pool(name="sa_av", bufs=2) as avp, \
                 tc.tile_pool(name="sa_f", bufs=1) as fp:
                wkv_scope = tc.tile_pool(name="sa_wkv", bufs=2)
                wkvp = wkv_scope.__enter__()
                # projections for all 4 heads; per-head JIT weight DMA
                wq_sb, wk_sb, wv_sb, wvb_sb, w28_sb, w2b_sb = {}, {}, {}, {}, {}, {}
                kt8, v8, vbf, qt8 = {}, {}, {}, {}
                for h in range(4):
                    wk_sb[h] = wkvp.tile([P, DC, D], F8, name=f"sa_wk_{h}",
                                         tag="wk")
                    nc.sync.dma_start(out=wk_sb[h][:], in_=sa_d["wk8"].ap()[h]
                                      .rearrange("(c p) f -> p c f", p=P))
                    wv_sb[h] = wkvp.tile([P, DC, D], F8, name=f"sa_wv_{h}",
                                         tag="wv")
                    nc.sync.dma_start(out=wv_sb[h][:], in_=sa_d["wv8"].ap()[h]
                                      .rearrange("(c p) f -> p c f", p=P))
                    wq_sb[h] = wkvp.tile([P, DC, D], F8, name=f"sa_wq_{h}",
                                         tag="wq")
                    nc.sync.dma_start(out=wq_sb[h][:], in_=sa_d["wq8"].ap()[h]
                                      .rearrange("(c p) f -> p c f", p=P))
                    wvb_sb[h] = wkvp.tile([P, DC, D], BF16, name=f"sa_wvb_{h}",
                                          tag="wvb")
                    nc.sync.dma_start(out=wvb_sb[h][:], in_=sa_d["wv_bf"].ap()[h]
                                      .rearrange("(c p) f -> p c f", p=P))
                    kt8[h] = qkvp.tile([P, DC, S], F8, name=f"sa_kt_{h}",
                                       tag=f"kt{h}")
                    v8[h] = qkvp.tile([P, TC, D], F8, name=f"sa_v_{h}",
                                      tag=f"v{h}")
                    qt8[h] = qkvp.tile([P, DC, S], F8, name=f"sa_qt_{h}",
                                       tag=f"qt{h}")
                    proj_head(xt8_sb, wk_sb[h], wv_sb[h], wq_sb[h],
                              bias_sb["sa", "bq"][:, h, :], kt8[h], v8[h],
                              qt8[h], 2, [nc.gpsimd, nc.scalar, nc.vector])
                    # bf16 V for keys 0..255 (early-token numerics)
                    vbf[h] = qkvp.tile([P, 2, D], BF16, name=f"sa_vb_{h}",
                                       tag=f"vb{h}")
                    for tci in range(2):
                        ps = pp.tile([P, SW], F32, name=f"vbp_{h}_{tci}",
                                     tag="pp")
                        for c in range(DC):
                            nc.tensor.matmul(
                                ps[:], xbf_sb[:, c, tci * P:(tci + 1) * P],
                                wvb_sb[h][:, c, :],
                                start=(c == 0), stop=(c == DC - 1))
                        nc.vector.tensor_copy(vbf[h][:, tci, :], ps[:])
                wkv_scope.__exit__(None, None, None)
                # W2 weights arrive after the projection weights
                for h in range(4):
                    w28_sb[h] = w2p.tile([P, DC, D], F8, name=f"sa_w28_{h}",
                                         tag=f"w28{h}")
                    nc.sync.dma_start(out=w28_sb[h][:], in_=sa_d["w28"].ap()[h]
                                      .rearrange("(c p) f -> p c f", p=P))

                fbf = fp.tile([P, DC, S], BF16, name="sa_fbf")

                # attention: query-tile outer, head inner
                for qt in range(4):
                    qsl = slice(qt * QW, (qt + 1) * QW)
                    npair = qt + 1     # key-block pairs for this tile
                    bf = (qt == 0)     # bf16 AV/W2 path for queries 0..255
                    pw = pwp.tile([P, DC, QW], F32, name=f"sa_pw_{qt}", tag="pw")
                    rbs = {}
                    for h in range(4):
                        e8 = ep.tile([P, 2, QW] if bf else [P, TC, QW],
                                     BF16 if bf else F8,
                                     name=f"sa_e_{qt}_{h}",
                                     tag="ebf" if bf else "e",
                                     bufs=2 if bf else None)
                        dn = pp.tile([1, 2, QW], F32, name=f"sa_dn_{qt}_{h}",
                                     tag="pp")
                        for jp in range(npair):
                            sps = scp.tile([P, 2, QW], F32,
                                           name=f"sa_s_{qt}_{h}_{jp}", tag="sc")
                            for j2 in range(2):
                                kb = 2 * jp + j2
                                for fcp in range(2):
                                    nc.tensor.matmul(
                                        sps[:, j2, :],
                                        kt8[h][:, 2 * fcp:2 * fcp + 2,
                                               kb * P:(kb + 1) * P],
                                        qt8[h][:, 2 * fcp:2 * fcp + 2, qsl],
                                        start=(fcp == 0), stop=(fcp == 1),
                                        perf_mode=DR)
                            nc.scalar.activation(
                                e8[:, 2 * jp:2 * jp + 2, :], sps[:], AF.Exp,
                                scale=C_EXP,
                                bias=zero_col[:] if bf else ln8_col[:])
                            if jp == npair - 1:   # diagonal pair: apply mask
                                nc.gpsimd.tensor_tensor(
                                    e8[:, 2 * jp:2 * jp + 2, :],
                                    e8[:, 2 * jp:2 * jp + 2, :],
                                    masks8_sb[:], OP.mult)
                        if bf:
                            for j2 in range(2):
                                nc.tensor.matmul(
                                    dn[:, 0, :], onesbf_sb[:],
                                    e8[:, j2, :],
                                    start=(j2 == 0), stop=(j2 == 1))
                        else:
                            for jp in range(npair):
                                nc.tensor.matmul(
                                    dn[:, 0, :], ones8_sb[:],
                                    e8[:, 2 * jp:2 * jp + 2, :],
                                    start=(jp == 0), stop=(jp == npair - 1),
                                    perf_mode=DR)
                        rc = statp.tile([1, QW], F32R, name=f"sa_rc_{qt}_{h}",
                                        tag="st")
                        with nc.allow_low_precision(reason="recip bcast"):
                            nc.vector.reciprocal(rc[:], dn[:, 0, :])
                        rbp = pp.tile([P, QW], F32, name=f"sa_rbp_{qt}_{h}",
                                      tag="pp")
                        nc.tensor.matmul(rbp[:], ones_row[:], rc[:],
                                         start=True, stop=True)
                        rb = smallp.tile([P, QW], F32, name=f"sa_rb_{qt}_{h}",
                                         tag="sm")
                        nc.gpsimd.tensor_copy(rb[:], rbp[:])
                        rbs[h] = rb

                        avn = avp.tile([P, DC, QW], BF16 if bf else F8,
                                       name=f"sa_avn_{qt}_{h}",
                                       tag="avnbf" if bf else "avn",
                                       bufs=1 if bf else None)
                        for fp2 in range(2):
                            po = scp.tile([P, 2, QW], F32,
                                          name=f"sa_po_{qt}_{h}_{fp2}",
                                          tag="sc")
                            for f2_ in range(2):
                                fc = 2 * fp2 + f2_
                                if bf:
                                    for j in range(2):
                                        nc.tensor.matmul(
                                            po[:, f2_, :],
                                            vbf[h][:, j, fc * P:(fc + 1) * P],
                                            e8[:, j, :],
                                            start=(j == 0), stop=(j == 1))
                                else:
                                    for jp in range(npair):
                                        nc.tensor.matmul(
                                            po[:, f2_, :],
                                            v8[h][:, 2 * jp:2 * jp + 2,
                                                  fc * P:(fc + 1) * P],
                                            e8[:, 2 * jp:2 * jp + 2, :],
                                            start=(jp == 0),
                                            stop=(jp == npair - 1),
                                            perf_mode=DR)
                                (nc.vector if fp2 == 0 else nc.gpsimd).tensor_tensor(
                                    avn[:, fc, :], po[:, f2_, :], rbs[h][:],
                                    OP.mult)
                        w2s = w28_sb[h]
                        for gc in range(DC):
                            for fcp in range(2):
                                if bf:
                                    # bf16 avn x fp8 w2 (no DoubleRow)
                                    for c2 in range(2):
                                        nc.tensor.matmul(
                                            pw[:, gc, :],
                                            w2s[:, 2 * fcp + c2,
                                                gc * P:(gc + 1) * P],
                                            avn[:, 2 * fcp + c2, :],
                                            start=(h == 0 and fcp == 0
                                                   and c2 == 0),
                                            stop=(h == 3 and fcp == 1
                                                  and c2 == 1))
                                else:
                                    nc.tensor.matmul(
                                        pw[:, gc, :],
                                        w2s[:, 2 * fcp:2 * fcp + 2,
                                            gc * P:(gc + 1) * P],
                                        avn[:, 2 * fcp:2 * fcp + 2, :],
                                        start=(h == 0 and fcp == 0),
                                        stop=(h == 3 and fcp == 1),
                                        perf_mode=DR)
                    # epilogue: all 4 heads accumulated; + acc bias -> bf16
                    cw = (1.0 / SW2) if bf else C_W2
                    for gc in range(DC):
                        nc.vector.tensor_scalar(
                            fbf[:, gc, qsl], pw[:, gc, :], cw,
                            bias_sb["sa", "acc"][:, gc:gc + 1],
                            OP.mult, OP.add)

                for half in range(2):
                    nc.sync.dma_start(
                        out=cc_in[half].rearrange("(c p) s -> p c s", p=P),
                        in_=fbf[:, :, half * SW:(half + 1) * SW])

            # one pair collective: reduce partial head-sums + scatter seq halves
            nc.gpsimd.collective_compute(
                "ReduceScatter", mybir.AluOpType.add, replica_groups=PAIRS,
                ins=[cc_in.opt()], outs=[cc_half.opt()])

            # ---- cross-attention K/V projections.  All 8 heads emitted into
            # a 4-deep rotating pool: the first ~3 run during the collective,
            # the rest pipeline as the head-outer attention loop frees slots.
            ln1_anchor = [None]
            ca_kt8, ca_v8 = {}, {}

            def ca_kv_proj(h, rr=None):
                wk_s = kvwp.tile([P, DC, D], F8, name=f"ca_wk_{h}", tag="wk")
                nc.sync.dma_start(out=wk_s[:], in_=ca_d["wk8"].ap()[h]
                                  .rearrange("(c p) f -> p c f", p=P))
                wv_s = kvwp.tile([P, DC, D], F8, name=f"ca_wv_{h}", tag="wv")
                nc.sync.dma_start(out=wv_s[:], in_=ca_d["wv8"].ap()[h]
                                  .rearrange("(c p) f -> p c f", p=P))
                ca_kt8[h] = kvpp.tile([P, DC, T], F8, name=f"ca_ktp_{h}",
                                      tag="cktp")
                ca_v8[h] = kvpp.tile([P, TC, D], F8, name=f"ca_vp_{h}",
                                     tag="cvp")
                return proj_head(et8_sb, wk_s, wv_s, None, None,
                                 ca_kt8[h], ca_v8[h], None, 0,
                                 rr or [nc.gpsimd, nc.scalar, nc.vector])

            for h in range(4):
                last = ca_kv_proj(h)
                if h == 2:
                    ln1_anchor[0] = last

            # ---- LN1 on my sequence half ----
            cch_sb = earlyp.tile([P, DC, SW], BF16, name="cch_sb",
                                 tag="cch")
            nc.sync.dma_start(
                out=cch_sb[:],
                in_=cc_half.opt().rearrange("(c p) s -> p c s", p=P))
            x1_sb = residp.tile([P, DC, SW], F32R, name="x1_sb", tag="resid")
            from concourse.tile import add_dep_helper as _adh
            _bb = nc.main_func.blocks[-1]
            _n0 = len(_bb.instructions)
            layernorm(cch_sb, xres_sb, x1_sb, ln_sb["ln1_b"], 0)
            x18_sb = residp.tile([P, DC, SW], F8, name="x18_sb", tag="x18",
                                 bufs=1)
            for c in range(DC):
                nc.scalar.activation(x18_sb[:, c, :], x1_sb[:, c, :],
                                     AF.Copy, scale=SX)
            if ln1_anchor[0] is not None:
                for _ins in list(_bb.instructions)[_n0:]:
                    _adh(_ins, ln1_anchor[0].ins, sync=False,
                         reason="order LN1 after CA-KV precompute h2")
            early_scope.__exit__(None, None, None)

            # FFN weights prefetch (transfers overlap CA attention)
            ffn_w_scope = tc.tile_pool(name="ffn_w", bufs=1)
            fwp = ffn_w_scope.__enter__()
            fc1_sb = fwp.tile([P, DC, 2048], BF16, name="fc1_sb")
            fc2_sb = fwp.tile([P, MC, D], BF16, name="fc2_sb")

            # ============ cross-attention (seq-split, all heads) ============
            with tc.tile_pool(name="ca_w", bufs=1) as cwp, \
                 tc.tile_pool(name="ca_qt", bufs=1) as cqtp, \
                 tc.tile_pool(name="ca_e", bufs=2) as ep, \
                 tc.tile_pool(name="ca_av", bufs=2) as avp, \
                 tc.tile_pool(name="ca_f", bufs=1) as fp:
                wq_c, w2_c = {}, {}
                for h in range(H):
                    w2_c[h] = cwp.tile([P, DC, D], F8, name=f"ca_w2_{h}",
                                       tag=f"cw2{h}")
                    nc.sync.dma_start(out=w2_c[h][:], in_=ca_d["w28"].ap()[h]
                                      .rearrange("(c p) f -> p c f", p=P))
                # wq tiles 2-buffered: freed as Q projections complete
                cwqp_scope = tc.tile_pool(name="ca_wq", bufs=2)
                cwqp = cwqp_scope.__enter__()
                for h in range(H):
                    wq_c[h] = cwqp.tile([P, DC, D], F8, name=f"ca_wq_{h}",
                                        tag="cwq")
                    nc.sync.dma_start(out=wq_c[h][:], in_=ca_d["wq8"].ap()[h]
                                      .rearrange("(c p) f -> p c f", p=P))
                for mg in range(4):
                    nc.sync.dma_start(
                        out=fc1_sb[:, :, mg * SW:(mg + 1) * SW],
                        in_=fc1_d.ap().rearrange("(c p) m -> p c m", p=P)
                        [:, :, mg * SW:(mg + 1) * SW])
                nc.sync.dma_start(out=fc2_sb[:], in_=fc2_d.ap()
                                  .rearrange("(c p) g -> p c g", p=P))
                qt8_c = {}
                for h in range(H):
                    qt8_c[h] = cqtp.tile([P, DC, SW], F8, name=f"ca_qt_{h}",
                                         tag=f"cq{h}")
                    _ = wq_c[h]  # DMA emitted above; tiles rotate via pool
                    for fc in range(DC):
                        ps = pp.tile([P, SW], F32, name=f"cqp_{h}_{fc}",
                                     tag="pp")
                        for nt in range(2):
                            for cp in range(2):
                                nc.tensor.matmul(
                                    ps[:, nt * QW:(nt + 1) * QW],
                                    wq_c[h][:, 2 * cp:2 * cp + 2,
                                            fc * P:(fc + 1) * P],
                                    x18_sb[:, 2 * cp:2 * cp + 2,
                                           nt * QW:(nt + 1) * QW],
                                    start=(cp == 0), stop=(cp == 1),
                                    perf_mode=DR)
                        (nc.gpsimd if h % 2 else nc.vector).tensor_scalar(
                            qt8_c[h][:, fc, :], ps[:], C_Q,
                            bias_sb["ca", "bq"][:, h, fc:fc + 1],
                            OP.mult, OP.add)
                cwqp_scope.__exit__(None, None, None)

                f2_sb = fp.tile([P, DC, SW], F32, name="ca_f2")
                for h in range(H):
                    e8 = ep.tile([P, 2 * TC, QW], F8, name=f"ca_e_{h}",
                                 tag="e")
                    dn = pp.tile([1, 2, QW], F32, name=f"ca_dn_{h}", tag="pp")
                    rb = pp.tile([P, 2, QW], F32, name=f"ca_rb_{h}", tag="pp")
                    for qt in range(2):
                        qsl = slice(qt * QW, (qt + 1) * QW)
                        for jp in range(4):
                            sps = scp.tile([P, 2, QW], F32,
                                           name=f"ca_s_{qt}_{h}_{jp}", tag="sc")
                            for j2 in range(2):
                                kb = 2 * jp + j2
                                for fcp in range(2):
                                    nc.tensor.matmul(
                                        sps[:, j2, :],
                                        ca_kt8[h][:, 2 * fcp:2 * fcp + 2,
                                                  kb * P:(kb + 1) * P],
                                        qt8_c[h][:, 2 * fcp:2 * fcp + 2, qsl],
                                        start=(fcp == 0), stop=(fcp == 1),
                                        perf_mode=DR)
                            nc.scalar.activation(
                                e8[:, 8 * qt + 2 * jp:8 * qt + 2 * jp + 2, :],
                                sps[:], AF.Exp, scale=C_EXP, bias=ln8_col[:])
                    # denominators after the scores loop (avoids PE
                    # head-of-line wait on each exp)
                    for qt in range(2):
                        for jp in range(4):
                            nc.tensor.matmul(
                                dn[:, qt, :], ones8_sb[:],
                                e8[:, 8 * qt + 2 * jp:8 * qt + 2 * jp + 2, :],
                                start=(jp == 0), stop=(jp == 3),
                                perf_mode=DR)
                    rc = statp.tile([1, 2, QW], F32R, name=f"ca_rc_{h}",
                                    tag="st")
                    with nc.allow_low_precision(reason="recip bcast"):
                        nc.vector.reciprocal(rc[:], dn[:])
                    for qt in range(2):
                        nc.tensor.matmul(rb[:, qt, :], ones_row[:],
                                         rc[:, qt, :], start=True, stop=True)
                    rbs_sb = smallp.tile([P, 2, QW], F32, name=f"ca_rbs_{h}",
                                         tag="sm")
                    nc.gpsimd.tensor_copy(rbs_sb[:], rb[:])
                    for qt in range(2):
                        qsl = slice(qt * QW, (qt + 1) * QW)
                        avn = avp.tile([P, DC, QW], F8,
                                       name=f"ca_avn_{qt}_{h}", tag="avn")
                        for fp2 in range(2):
                            po = scp.tile([P, 2, QW], F32,
                                          name=f"ca_po_{qt}_{h}_{fp2}",
                                          tag="sc")
                            for f2_ in range(2):
                                fc = 2 * fp2 + f2_
                                for jp in range(4):
                                    nc.tensor.matmul(
                                        po[:, f2_, :],
                                        ca_v8[h][:, 2 * jp:2 * jp + 2,
                                                 fc * P:(fc + 1) * P],
                                        e8[:, 8 * qt + 2 * jp:8 * qt + 2 * jp + 2, :],
                                        start=(jp == 0), stop=(jp == 3),
                                        perf_mode=DR)
                                (nc.vector if fp2 == 0 else nc.gpsimd).tensor_tensor(
                                    avn[:, fc, :], po[:, f2_, :],
                                    rbs_sb[:, qt, :], OP.mult)
                        pw = pwp.tile([P, DC, QW], F32, name=f"ca_pw_{qt}_{h}",
                                      tag="pw")
                        for gc in range(DC):
                            for fcp in range(2):
                                nc.tensor.matmul(
                                    pw[:, gc, :],
                                    w2_c[h][:, 2 * fcp:2 * fcp + 2,
                                            gc * P:(gc + 1) * P],
                                    avn[:, 2 * fcp:2 * fcp + 2, :],
                                    start=(fcp == 0), stop=(fcp == 1),
                                    perf_mode=DR)
                        if h == 0:
                            for gc in range(DC):
                                nc.vector.tensor_scalar(
                                    f2_sb[:, gc, qsl], pw[:, gc, :], C_W2,
                                    bias_sb["ca", "acc"][:, gc:gc + 1],
                                    OP.mult, OP.add)
                        else:
                            nc.vector.scalar_tensor_tensor(
                                f2_sb[:, :, qsl], pw[:], C_W2,
                                f2_sb[:, :, qsl], OP.mult, OP.add)
                    if h + 4 < H:
                        ca_kv_proj(h + 4, rr=[nc.gpsimd, nc.scalar])

                # ---- LN2 ----
                x2_sb = residp.tile([P, DC, SW], F32R, name="x2_sb", tag="resid")
                layernorm(f2_sb, x1_sb, x2_sb, ln_sb["ln2_b"], 1)
                x2b_sb = residp.tile([P, DC, SW], BF16, name="x2b_sb",
                                     tag="x2b", bufs=1)
                for c in range(DC):
                    nc.gpsimd.tensor_copy(x2b_sb[:, c, :], x2_sb[:, c, :])
            # ============ FFN (bf16, seq-split) ============
            with tc.tile_pool(name="ffn_h", bufs=1) as fhp:
                h_sb = fhp.tile([P, MC, SW], BF16, name="h_sb")
                f3_sb = fhp.tile([P, DC, SW], F32, name="f3_sb")
                for mc in range(MC):
                    ps = pp.tile([P, SW], F32, name=f"f1_{mc}", tag="pp")
                    for c in range(DC):
                        nc.tensor.matmul(ps[:], fc1_sb[:, c, mc * P:(mc + 1) * P],
                                         x2b_sb[:, c, :],
                                         start=(c == 0), stop=(c == DC - 1))
                    nc.scalar.activation(h_sb[:, mc, :], ps[:], AF.Relu,
                                         bias=fc1b_sb[:, mc:mc + 1])
                for gc in range(DC):
                    ps = pp.tile([P, SW], F32, name=f"f2_{gc}", tag="pp")
                    for mc in range(MC):
                        nc.tensor.matmul(ps[:],
                                         fc2_sb[:, mc, gc * P:(gc + 1) * P],
                                         h_sb[:, mc, :],
                                         start=(mc == 0), stop=(mc == MC - 1))
                    nc.vector.tensor_scalar_add(f3_sb[:, gc, :], ps[:],
                                                ffnb_sb[:, gc:gc + 1])

                # ---- LN3 + output ----
                out_sb = residp.tile([P, DC, SW], F32R, name="out_sb",
                                     tag="resid")
                layernorm(f3_sb, x2_sb, out_sb, ln_sb["ln3_b"], 2)
                for c in range(DC):
                    nc.sync.dma_start(
                        out=outt_d.ap().rearrange("(c p) s -> p c s", p=P)[:, c, :],
                        in_=out_sb[:, c, :].bitcast(F32))

            ffn_w_scope.__exit__(None, None, None)
            ca_kvw_scope.__exit__(None, None, None)
            ca_kv_scope.__exit__(None, None, None)

    nc.compile()
    return nc


def get_program():
    if "nc" not in _CACHE:
        _CACHE["nc"] = build_program()
    return _CACHE["nc"]


def kernel(**inputs) -> np.ndarray:
    from concourse.bass_utils import run_bass_kernel_spmd
    nc = get_program()
    in_maps = _host_prep(inputs)
    res = run_bass_kernel_spmd(nc, in_maps, core_ids=list(range(NC)))
    out = np.empty((B, S, D), np.float32)
    for b in range(B):
        out[b, 0:SW] = res.results[2 * b]["outt"].T
        out[b, SW:S] = res.results[2 * b + 1]["outt"].T
    return out


# revision 41
# speedup vs baseline: 1.0868x; 1.0016x over previous
"""Trainium2 Bass kernel for nn_Decoder (dense transformer decoder layer).

Problem (hardcoded): B=4, S=T=1024, D=512, H=8 heads, fp32.
  h  = MHA_self(x, causal) ; x1 = LN(h + x)
  h  = MHA_cross(x1, encod_out) ; x2 = LN(h + x1)
  ff = relu(x2 @ fc1) @ fc2 ; out = LN(ff + x2)

Sharding (8 cores = 4 batch groups x 2-core pairs), same as the f32r
baseline: self-attention tensor-parallel over heads (4 heads/core, full
S); one pair ReduceScatter (bf16) combines partial head-sums and splits
the sequence; LN1, cross-attn (all 8 heads, redundant K/V), LN2, FFN,
LN3 run sequence-parallel on the core's 512-row half.

Speed: nearly all matmuls run as fp8(e4m3) DoubleRow (contraction 256
per instruction at 0.5 cycles/row = 4x the f32r rate in the cost
model).  Numerics (validated against the fp32 reference in numpy):
  - projections/scores/AV/W2 fp8 with power-of-2 scales folded into
    ACT epilogues (x*8, w*512, Q/K/V*32, e*8, w2*2048)
  - bk dropped (exact softmax invariance); bv/bo/bf folded into acc
  - causal tail fix: attention output for early tokens is dominated by
    a single V row, so V for keys 0-255 is recomputed with bf16
    operands and query-tile 0 runs its AV/W2 path in bf16
  - FFN entirely bf16 (fp8 FFN alone costs ~1.5e-2 rel err)
  - residuals/LN in f32; ReduceScatter in bf16
Attention is query-tile-outer / head-inner so the W2 output accumulates
across all heads in one pinned PSUM tile (single epilogue per tile).
"""
import math
import numpy as np

B, S, T, D, H = 4, 1024, 1024, 512, 8
P = 128
NC = 8
DC = D // P    # 4 feature chunks
TC = T // P    # 8 time chunks
SW = 512       # per-core sequence half
QW = 256       # query tile width (DoubleRow moving limit)
MC = 2048 // P  # 16 FFN hidden chunks
EPS = 1e-5
PAIRS = [[0, 1], [2, 3], [4, 5], [6, 7]]

# fp8 scales (powers of 2; folded into f32 epilogue constants)
SX = 8.0       # x / x1 / enc quant
SWQ = 512.0    # wq/wk/wv quant
SQ = 32.0      # Q requant
SK = 32.0      # K requant
SV = 32.0      # V requant
SE = 8.0       # exp output
SW2 = 2048.0   # folded w2 quant
RSQD = 1.0 / math.sqrt(D)

_CACHE = {}


def _host_prep(inputs):
    import ml_dtypes
    F8 = ml_dtypes.float8_e4m3
    BF = ml_dtypes.bfloat16
    x = np.asarray(inputs["x"], np.float32)
    enc = np.asarray(inputs["encod_out"], np.float32)

    def q8(a, s):
        return (np.asarray(a, np.float32) * s).astype(F8)

    per_phase = {}
    for p in ("sa", "ca"):
        wq = np.asarray(inputs[p + "_wq"], np.float32)
        bq = np.asarray(inputs[p + "_bq"], np.float32)
        wk = np.asarray(inputs[p + "_wk"], np.float32)
        wv = np.asarray(inputs[p + "_wv"], np.float32)
        bv = np.asarray(inputs[p + "_bv"], np.float32)
        wo = np.asarray(inputs[p + "_wo"], np.float32)
        bo = np.asarray(inputs[p + "_bo"], np.float32)
        wf = np.asarray(inputs[p + "_wf"], np.float32).reshape(H, D, D)
        bf = np.asarray(inputs[p + "_bf"], np.float32)
        w2 = np.einsum("hfg,hgk->hfk", wo.astype(np.float64),
                       wf.astype(np.float64)).astype(np.float32)
        acc = bf.astype(np.float64).copy()
        for h in range(H):
            acc += (bv[h].astype(np.float64) @ wo[h].astype(np.float64)
                    + bo[h].astype(np.float64)) @ wf[h].astype(np.float64)
        per_phase[p] = dict(
            wq8=q8(wq, SWQ), wk8=q8(wk, SWQ), wv8=q8(wv, SWQ),
            w28=q8(w2, SW2), wv_bf=wv.astype(BF),
            bq=bq * SQ, acc=acc.astype(np.float32))

    fc1_w = np.asarray(inputs["fc1_w"], np.float32)
    fc1_b = np.asarray(inputs["fc1_b"], np.float32)
    fc2_w = np.asarray(inputs["fc2_w"], np.float32)
    fc2_b = np.asarray(inputs["fc2_b"], np.float32)
    lns = {f"ln{i}_{k}": np.asarray(inputs[f"ln{i}_{k}"], np.float32)
           for i in (1, 2, 3) for k in ("g", "b")}

    # causal masks for the two diagonal key-blocks of each 256-query tile:
    # kb 2i: keep p <= c ; kb 2i+1: keep 128+p <= c   (c in 0..255)
    pp_ = np.arange(P)[:, None]
    cc = np.arange(QW)[None, :]
    mpair = np.stack([(pp_ <= cc), (P + pp_ <= cc)], axis=1)
    ones_pair = np.ones((P, 2, 1), np.float32)

    in_maps = []
    for c in range(NC):
        b, half = c // 2, c % 2
        hs = slice(4 * half, 4 * half + 4)
        ssl = slice(half * SW, (half + 1) * SW)
        xt = x[b].T
        m = {
            "xt8": np.ascontiguousarray((xt * SX)).astype(F8),
            "xbf": np.ascontiguousarray(xt[:, :QW]).astype(BF),
            "x_res": np.ascontiguousarray(xt[:, ssl]),
            "et8": np.ascontiguousarray(enc[b].T * SX).astype(F8),
            "masks8": mpair.astype(F8),
            "ones8": ones_pair.astype(F8),
            "onesbf": np.ones((P, 1), BF),
            "ones_row": np.ones((1, P), np.float32),
            "fc1bf": fc1_w.astype(BF), "fc1_b": fc1_b,
            "fc2bf": fc2_w.astype(BF), "ffn_bias": fc2_b,
        }
        pp = per_phase["sa"]
        for k in ("wq8", "wk8", "wv8", "w28", "wv_bf", "bq"):
            m["sa_" + k] = np.ascontiguousarray(pp[k][hs])
        m["sa_acc"] = pp["acc"] / 2.0
        pp = per_phase["ca"]
        for k in ("wq8", "wk8", "wv8", "w28", "bq"):
            m["ca_" + k] = pp[k]
        m["ca_acc"] = pp["acc"]
        for k, v in lns.items():
            m[k] = v
        in_maps.append(m)
    return in_maps


def build_program():
    import concourse.bacc as bacc
    import concourse.mybir as mybir
    import concourse.tile as tile

    F32 = mybir.dt.float32
    F32R = mybir.dt.float32r
    BF16 = mybir.dt.bfloat16
    F8 = mybir.dt.float8e4
    AF = mybir.ActivationFunctionType
    OP = mybir.AluOpType
    DR = mybir.MatmulPerfMode.DoubleRow

    nc = bacc.Bacc(None, target_bir_lowering=False, num_devices=NC)

    # ---- DRAM I/O ----
    xt8_d = nc.dram_tensor("xt8", [D, S], F8, kind="ExternalInput")
    xbf_d = nc.dram_tensor("xbf", [D, QW], BF16, kind="ExternalInput")
    xres_d = nc.dram_tensor("x_res", [D, SW], F32, kind="ExternalInput")
    et8_d = nc.dram_tensor("et8", [D, T], F8, kind="ExternalInput")
    masks8_d = nc.dram_tensor("masks8", [P, 2, QW], F8, kind="ExternalInput")
    ones8_d = nc.dram_tensor("ones8", [P, 2, 1], F8, kind="ExternalInput")
    onesbf_d = nc.dram_tensor("onesbf", [P, 1], BF16, kind="ExternalInput")
    onesrow_d = nc.dram_tensor("ones_row", [1, P], F32, kind="ExternalInput")
    sa_d = {k: nc.dram_tensor("sa_" + k, [4, D, D], F8, kind="ExternalInput")
            for k in ("wq8", "wk8", "wv8", "w28")}
    sa_d["wv_bf"] = nc.dram_tensor("sa_wv_bf", [4, D, D], BF16, kind="ExternalInput")
    sa_d["bq"] = nc.dram_tensor("sa_bq", [4, D], F32, kind="ExternalInput")
    sa_d["acc"] = nc.dram_tensor("sa_acc", [D], F32, kind="ExternalInput")
    ca_d = {k: nc.dram_tensor("ca_" + k, [H, D, D], F8, kind="ExternalInput")
            for k in ("wq8", "wk8", "wv8", "w28")}
    ca_d["bq"] = nc.dram_tensor("ca_bq", [H, D], F32, kind="ExternalInput")
    ca_d["acc"] = nc.dram_tensor("ca_acc", [D], F32, kind="ExternalInput")
    fc1_d = nc.dram_tensor("fc1bf", [D, 2048], BF16, kind="ExternalInput")
    fc1b_d = nc.dram_tensor("fc1_b", [2048], F32, kind="ExternalInput")
    fc2_d = nc.dram_tensor("fc2bf", [2048, D], BF16, kind="ExternalInput")
    ffnb_d = nc.dram_tensor("ffn_bias", [D], F32, kind="ExternalInput")
    ln_d = {f"ln{i}_{k}": nc.dram_tensor(f"ln{i}_{k}", [D], F32, kind="ExternalInput")
            for i in (1, 2, 3) for k in ("g", "b")}
    outt_d = nc.dram_tensor("outt", [D, SW], F32, kind="ExternalOutput")

    r32 = lambda ap: ap.bitcast(F32R)

    # epilogue constants
    C_Q = SQ / (SX * SWQ)
    C_K = SK / (SX * SWQ)
    C_V = SV / (SX * SWQ)
    C_EXP = RSQD / (SQ * SK)     # scale on score psum inside exp
    C_W2 = 1.0 / (SV * SW2)      # scale on fp8 W2 psum
    LN_SE = math.log(SE)

    with tile.TileContext(nc, pool_alloc_mode="queue") as tc:
        with tc.tile_pool(name="const", bufs=1) as constp, \
             tc.tile_pool(name="resid", bufs=2) as residp, \
             tc.tile_pool(name="smalls", bufs=3) as smallp, \
             tc.tile_pool(name="stats", bufs=4) as statp, \
             tc.tile_pool(name="pp", bufs=2, space="PSUM") as pp, \
             tc.tile_pool(name="sc", bufs=4, space="PSUM") as scp, \
             tc.tile_pool(name="pw", bufs=1, space="PSUM") as pwp, \
             tc.tile_pool(name="dram", bufs=1, space="DRAM") as dramp:

            # ---- constants ----
            eps_sb = constp.tile([1, 1], F32, name="eps_sb")
            nc.vector.memset(eps_sb[:], EPS)
            ln8_col = constp.tile([P, 1], F32, name="ln8_col")
            nc.vector.memset(ln8_col[:], LN_SE)
            zero_col = constp.tile([P, 1], F32, name="zero_col")
            nc.vector.memset(zero_col[:], 0.0)
            ones_col = constp.tile([P, 1], F32R, name="ones_col")
            nc.vector.memset(ones_col[:], 1.0)

            xt8_sb = residp.tile([P, DC, S], F8, name="xt8_sb", tag="resid")
            nc.sync.dma_start(out=xt8_sb[:],
                              in_=xt8_d.ap().rearrange("(c p) s -> p c s", p=P))
            xbf_sb = constp.tile([P, DC, QW], BF16, name="xbf_sb")
            nc.scalar.dma_start(out=xbf_sb[:],
                                in_=xbf_d.ap().rearrange("(c p) s -> p c s", p=P))
            ca_kv_scope = tc.tile_pool(name="ca_kvp", bufs=4)
            kvpp = ca_kv_scope.__enter__()
            ca_kvw_scope = tc.tile_pool(name="ca_kvw", bufs=2)
            kvwp = ca_kvw_scope.__enter__()
            early_scope = tc.tile_pool(name="early", bufs=1)
            earlyp = early_scope.__enter__()
            xres_sb = earlyp.tile([P, DC, SW], F32, name="xres_sb", tag="xres")
            nc.scalar.dma_start(out=xres_sb[:],
                                in_=xres_d.ap().rearrange("(c p) s -> p c s", p=P))
            et8_sb = kvwp.tile([P, DC, T], F8, name="et8_sb", tag="et",
                               bufs=1)
            nc.scalar.dma_start(out=et8_sb[:],
                                in_=et8_d.ap().rearrange("(c p) s -> p c s", p=P))

            masks8_sb = constp.tile([P, 2, QW], F8, name="masks8_sb")
            nc.scalar.dma_start(out=masks8_sb[:], in_=masks8_d.ap())
            ones8_sb = constp.tile([P, 2, 1], F8, name="ones8_sb")
            nc.scalar.dma_start(out=ones8_sb[:], in_=ones8_d.ap())
            onesbf_sb = constp.tile([P, 1], BF16, name="onesbf_sb")
            nc.scalar.dma_start(out=onesbf_sb[:], in_=onesbf_d.ap())
            ones_row = constp.tile([1, P], F32R, name="ones_row")
            nc.scalar.dma_start(out=ones_row[:], in_=r32(onesrow_d.ap()))

            def vec_to_pc(dram_ap, name, nch):
                t = constp.tile([P, nch], F32, name=name)
                nc.scalar.dma_start(out=t[:],
                                    in_=dram_ap.rearrange("(c p) -> p c", p=P))
                return t

            bias_sb = {}
            for pn, dd, nh in (("sa", sa_d, 4), ("ca", ca_d, H)):
                t = constp.tile([P, nh, DC], F32, name=f"{pn}_bq_sb")
                nc.scalar.dma_start(
                    out=t[:], in_=dd["bq"].ap().rearrange("h (c p) -> p h c", p=P))
                bias_sb[pn, "bq"] = t
                bias_sb[pn, "acc"] = vec_to_pc(dd["acc"].ap(), f"{pn}_acc_sb", DC)
            grow_sb = constp.tile([1, 3, DC, P], F32R, name="ln_grow")
            for _i in (1, 2, 3):
                nc.scalar.dma_start(
                    out=grow_sb[:, _i - 1, :, :],
                    in_=r32(ln_d[f"ln{_i}_g"].ap().rearrange(
                        "(a c p) -> a c p", a=1, p=P)))
            fc1b_sb = vec_to_pc(fc1b_d.ap(), "fc1b_sb", MC)
            ffnb_sb = vec_to_pc(ffnb_d.ap(), "ffnb_sb", DC)
            ln_sb = {k: vec_to_pc(v.ap(), k + "_sb", DC) for k, v in ln_d.items()}

            cc_in = dramp.tile([2, D, SW], BF16, name="cc_in")
            cc_half = dramp.tile([D, SW], BF16, name="cc_half")

            def layernorm_half(src, resid_sb, dst, b_sb, gri, csl):
                """dst[:, :, csl] = LN(src + resid) on a column slice."""
                W = csl.stop - csl.start
                for c in range(DC):
                    nc.gpsimd.tensor_add(dst[:, c, csl], src[:, c, csl],
                                         resid_sb[:, c, csl])
                psum_sum = pp.tile([1, SW], F32, name="ln_sum", tag="pp")
                psum_ssq = pp.tile([1, SW], F32, name="ln_ssq", tag="pp")
                for c in range(DC):
                    sq = smallp.tile([P, SW], F32R, name=f"ln_sq_{c}", tag="sm")
                    nc.scalar.activation(sq[:, :W], dst[:, c, csl], AF.Square)
                    nc.tensor.matmul(psum_sum[:, :W], ones_col[:],
                                     dst[:, c, csl],
                                     start=(c == 0), stop=(c == DC - 1))
                    nc.tensor.matmul(psum_ssq[:, :W], ones_col[:], sq[:, :W],
                                     start=(c == 0), stop=(c == DC - 1))
                mean = statp.tile([1, SW], F32R, name="ln_mean", tag="st")
                nc.scalar.activation(mean[:, :W], psum_sum[:, :W], AF.Copy,
                                     scale=1.0 / D)
                msq = statp.tile([1, SW], F32, name="ln_msq", tag="st")
                nc.scalar.activation(msq[:, :W], psum_ssq[:, :W], AF.Copy,
                                     scale=1.0 / D)
                var = statp.tile([1, SW], F32, name="ln_var", tag="st")
                nc.vector.tensor_tensor(var[:, :W], mean[:, :W], mean[:, :W],
                                        OP.mult)
                nc.vector.tensor_sub(var[:, :W], msq[:, :W], var[:, :W])
                std = statp.tile([1, SW], F32, name="ln_std", tag="st")
                nc.scalar.activation(std[:, :W], var[:, :W], AF.Sqrt,
                                     bias=eps_sb[:])
                rstd = statp.tile([1, SW], F32R, name="ln_rstd", tag="st")
                with nc.allow_low_precision(reason="f32r feed for bcast matmul"):
                    nc.vector.reciprocal(rstd[:, :W], std[:, :W])
                mr = statp.tile([1, SW], F32R, name="ln_mr", tag="st")
                nc.vector.tensor_tensor(mr[:, :W], mean[:, :W], rstd[:, :W],
                                        OP.mult)
                for c in range(DC):
                    psum_rb = pp.tile([P, SW], F32, name=f"ln_rb_{c}", tag="pp")
                    nc.tensor.matmul(psum_rb[:, :W], grow_sb[:, gri, c, :],
                                     rstd[:, :W], start=True, stop=True)
                    nc.tensor.matmul(psum_rb[:, W:2 * W],
                                     grow_sb[:, gri, c, :],
                                     mr[:, :W], start=True, stop=True)
                    tmp = smallp.tile([P, SW], F32, name=f"ln_t_{c}", tag="sm")
                    nc.vector.tensor_tensor(tmp[:, :W], dst[:, c, csl],
                                            psum_rb[:, :W], OP.mult)
                    nc.vector.scalar_tensor_tensor(
                        dst[:, c, csl], tmp[:, :W], b_sb[:, c:c + 1],
                        psum_rb[:, W:2 * W], OP.add, OP.subtract)

            def layernorm(src, resid_sb, dst, b_sb, gri):
                for hlf in range(2):
                    layernorm_half(src, resid_sb, dst, b_sb, gri,
                                   slice(hlf * QW, (hlf + 1) * QW))

            def epi_rr(eng, out_ap, ps_ap, cscale, bias_ap):
                """projection epilogue out = ps*c (+bias) on a chosen engine"""
                if eng is nc.scalar:
                    return nc.scalar.activation(
                        out_ap, ps_ap, AF.Identity, scale=cscale,
                        bias=bias_ap if bias_ap is not None else zero_col[:])
                if bias_ap is None:
                    return eng.tensor_scalar(out_ap, ps_ap, cscale, None,
                                             OP.mult)
                return eng.tensor_scalar(out_ap, ps_ap, cscale, bias_ap,
                                         OP.mult, OP.add)

            def proj_head(src_sb, wk_sb, wv_sb, wq_ap, bq_ap, kt8, v8, qt8,
                          nq, rr):
                """fp8 K^T [f,t], V [t,f], and optional Q^T [f,q] for one
                head.  Epilogues round-robin across Pool/ACT/DVE so all
                three drain in parallel."""
                last = None
                nrr = len(rr)
                ei = [0]
                def nxt():
                    e = rr[ei[0] % nrr]; ei[0] += 1
                    return e
                for fc in range(DC):
                    for th in range(2):
                        ps = pp.tile([P, SW], F32, name=f"kp_{fc}_{th}",
                                     tag="pp")
                        for nt in range(2):
                            for cp in range(2):
                                nc.tensor.matmul(
                                    ps[:, nt * QW:(nt + 1) * QW],
                                    wk_sb[:, 2 * cp:2 * cp + 2,
                                          fc * P:(fc + 1) * P],
                                    xt8_slice(src_sb, cp,
                                              th * SW + nt * QW),
                                    start=(cp == 0), stop=(cp == 1),
                                    perf_mode=DR)
                        epi_rr(nxt(), kt8[:, fc, th * SW:(th + 1) * SW],
                               ps[:], C_K, None)
                    for tci in (2 * fc, 2 * fc + 1):
                        ps = scp.tile([P, 2, QW], F32, name=f"vp_{tci}",
                                      tag="sc")
                        for fh in range(2):
                            for cp in range(2):
                                nc.tensor.matmul(
                                    ps[:, fh, :],
                                    src_sb[:, 2 * cp:2 * cp + 2,
                                           tci * P:(tci + 1) * P],
                                    wv_sb[:, 2 * cp:2 * cp + 2,
                                          fh * QW:(fh + 1) * QW],
                                    start=(cp == 0), stop=(cp == 1),
                                    perf_mode=DR)
                        last = epi_rr(nxt(), v8[:, tci, :], ps[:], C_V, None)
                    if wq_ap is not None:
                        for qh in range(nq):
                            ps = pp.tile([P, SW], F32, name=f"qp_{fc}_{qh}",
                                         tag="pp")
                            for nt in range(2):
                                for cp in range(2):
                                    nc.tensor.matmul(
                                        ps[:, nt * QW:(nt + 1) * QW],
                                        wq_ap[:, 2 * cp:2 * cp + 2,
                                              fc * P:(fc + 1) * P],
                                        xt8_slice(src_sb, cp,
                                                  qh * SW + nt * QW),
                                        start=(cp == 0), stop=(cp == 1),
                                        perf_mode=DR)
                            epi_rr(nxt(), qt8[:, fc, qh * SW:(qh + 1) * SW],
                                   ps[:], C_Q, bq_ap[:, fc:fc + 1])
                return last

            def xt8_slice(src_sb, cp, q0):
                return src_sb[:, 2 * cp:2 * cp + 2, q0:q0 + QW]

            # ============ self-attention (head-split, full S) ============
            with tc.tile_pool(name="sa_w2", bufs=1) as w2p, \
                 tc.tile_pool(name="sa_qkv", bufs=1) as qkvp, \
                 tc.tile_pool(name="sa_e", bufs=3) as ep, \
                 tc.tile_pool(name="sa_av", bufs=2) as avp, \
                 tc.tile_pool(name="sa_f", bufs=1) as fp:
                wkv_scope = tc.tile_pool(name="sa_wkv", bufs=2)
                wkvp = wkv_scope.__enter__()
                # projections for all 4 heads; per-head JIT weight DMA
                wq_sb, wk_sb, wv_sb, wvb_sb, w28_sb, w2b_sb = {}, {}, {}, {}, {}, {}
                kt8, v8, vbf, qt8 = {}, {}, {}, {}
                for h in range(4):
                    wk_sb[h] = wkvp.tile([P, DC, D], F8, name=f"sa_wk_{h}",
                                         tag="wk")
                    nc.sync.dma_start(out=wk_sb[h][:], in_=sa_d["wk8"].ap()[h]
                                      .rearrange("(c p) f -> p c f", p=P))
                    wv_sb[h] = wkvp.tile([P, DC, D], F8, name=f"sa_wv_{h}",
                                         tag="wv")
                    nc.sync.dma_start(out=wv_sb[h][:], in_=sa_d["wv8"].ap()[h]
                                      .rearrange("(c p) f -> p c f", p=P))
                    wq_sb[h] = wkvp.tile([P, DC, D], F8, name=f"sa_wq_{h}",
                                         tag="wq")
                    nc.sync.dma_start(out=wq_sb[h][:], in_=sa_d["wq8"].ap()[h]
                                      .rearrange("(c p) f -> p c f", p=P))
                    wvb_sb[h] = wkvp.tile([P, DC, D], BF16, name=f"sa_wvb_{h}",
                                          tag="wvb")
                    nc.sync.dma_start(out=wvb_sb[h][:], in_=sa_d["wv_bf"].ap()[h]
                                      .rearrange("(c p) f -> p c f", p=P))
                    kt8[h] = qkvp.tile([P, DC, S], F8, name=f"sa_kt_{h}",
                                       tag=f"kt{h}")
                    v8[h] = qkvp.tile([P, TC, D], F8, name=f"sa_v_{h}",
                                      tag=f"v{h}")
                    qt8[h] = qkvp.tile([P, DC, S], F8, name=f"sa_qt_{h}",
                                       tag=f"qt{h}")
                    proj_head(xt8_sb, wk_sb[h], wv_sb[h], wq_sb[h],
                              bias_sb["sa", "bq"][:, h, :], kt8[h], v8[h],
                              qt8[h], 2, [nc.vector, nc.scalar])
                    # bf16 V for keys 0..255 (early-token numerics)
                    vbf[h] = qkvp.tile([P, 2, D], BF16, name=f"sa_vb_{h}",
                                       tag=f"vb{h}")
                    for tci in range(2):
                        ps = pp.tile([P, SW], F32, name=f"vbp_{h}_{tci}",
                                     tag="pp")
                        for c in range(DC):
                            nc.tensor.matmul(
                                ps[:], xbf_sb[:, c, tci * P:(tci + 1) * P],
                                wvb_sb[h][:, c, :],
                                start=(c == 0), stop=(c == DC - 1))
                        nc.vector.tensor_copy(vbf[h][:, tci, :], ps[:])
                wkv_scope.__exit__(None, None, None)
                # W2 weights arrive after the projection weights
                for h in range(4):
                    w28_sb[h] = w2p.tile([P, DC, D], F8, name=f"sa_w28_{h}",
                                         tag=f"w28{h}")
                    nc.sync.dma_start(out=w28_sb[h][:], in_=sa_d["w28"].ap()[h]
                                      .rearrange("(c p) f -> p c f", p=P))

                fbf = fp.tile([P, DC, S], BF16, name="sa_fbf")

                # attention: query-tile outer, head inner
                for qt in range(4):
                    qsl = slice(qt * QW, (qt + 1) * QW)
                    npair = qt + 1     # key-block pairs for this tile
                    bf = (qt == 0)     # bf16 AV/W2 path for queries 0..255
                    pw = pwp.tile([P, DC, QW], F32, name=f"sa_pw_{qt}", tag="pw")
                    rbs = {}
                    for h in range(4):
                        e8 = ep.tile([P, 2, QW] if bf else [P, TC, QW],
                                     BF16 if bf else F8,
                                     name=f"sa_e_{qt}_{h}",
                                     tag="ebf" if bf else "e",
                                     bufs=2 if bf else None)
                        dn = pp.tile([1, 2, QW], F32, name=f"sa_dn_{qt}_{h}",
                                     tag="pp")
                        for jp in range(npair):
                            sps = scp.tile([P, 2, QW], F32,
                                           name=f"sa_s_{qt}_{h}_{jp}", tag="sc")
                            for j2 in range(2):
                                kb = 2 * jp + j2
                                for fcp in range(2):
                                    nc.tensor.matmul(
                                        sps[:, j2, :],
                                        kt8[h][:, 2 * fcp:2 * fcp + 2,
                                               kb * P:(kb + 1) * P],
                                        qt8[h][:, 2 * fcp:2 * fcp + 2, qsl],
                                        start=(fcp == 0), stop=(fcp == 1),
                                        perf_mode=DR)
                            nc.scalar.activation(
                                e8[:, 2 * jp:2 * jp + 2, :], sps[:], AF.Exp,
                                scale=C_EXP,
                                bias=zero_col[:] if bf else ln8_col[:])
                            if jp == npair - 1:   # diagonal pair: apply mask
                                nc.gpsimd.tensor_tensor(
                                    e8[:, 2 * jp:2 * jp + 2, :],
                                    e8[:, 2 * jp:2 * jp + 2, :],
                                    masks8_sb[:], OP.mult)
                        if bf:
                            for j2 in range(2):
                                nc.tensor.matmul(
                                    dn[:, 0, :], onesbf_sb[:],
                                    e8[:, j2, :],
                                    start=(j2 == 0), stop=(j2 == 1))
                        else:
                            for jp in range(npair):
                                nc.tensor.matmul(
                                    dn[:, 0, :], ones8_sb[:],
                                    e8[:, 2 * jp:2 * jp + 2, :],
                                    start=(jp == 0), stop=(jp == npair - 1),
                                    perf_mode=DR)
                        rc = statp.tile([1, QW], F32R, name=f"sa_rc_{qt}_{h}",
                                        tag="st")
                        with nc.allow_low_precision(reason="recip bcast"):
                            nc.vector.reciprocal(rc[:], dn[:, 0, :])
                        rb = pp.tile([P, QW], F32, name=f"sa_rb_{qt}_{h}",
                                     tag="pp")
                        nc.tensor.matmul(rb[:], ones_row[:], rc[:],
                                         start=True, stop=True)
                        rbs[h] = rb

                        avn = avp.tile([P, DC, QW], BF16 if bf else F8,
                                       name=f"sa_avn_{qt}_{h}",
                                       tag="avnbf" if bf else "avn",
                                       bufs=1 if bf else None)
                        for fp2 in range(2):
                            po = scp.tile([P, 2, QW], F32,
                                          name=f"sa_po_{qt}_{h}_{fp2}",
                                          tag="sc")
                            for f2_ in range(2):
                                fc = 2 * fp2 + f2_
                                if bf:
                                    for j in range(2):
                                        nc.tensor.matmul(
                                            po[:, f2_, :],
                                            vbf[h][:, j, fc * P:(fc + 1) * P],
                                            e8[:, j, :],
                                            start=(j == 0), stop=(j == 1))
                                else:
                                    for jp in range(npair):
                                        nc.tensor.matmul(
                                            po[:, f2_, :],
                                            v8[h][:, 2 * jp:2 * jp + 2,
                                                  fc * P:(fc + 1) * P],
                                            e8[:, 2 * jp:2 * jp + 2, :],
                                            start=(jp == 0),
                                            stop=(jp == npair - 1),
                                            perf_mode=DR)
                                nc.vector.tensor_tensor(
                                    avn[:, fc, :], po[:, f2_, :], rbs[h][:],
                                    OP.mult)
                        w2s = w28_sb[h]
                        for gc in range(DC):
                            for fcp in range(2):
                                if bf:
                                    # bf16 avn x fp8 w2 (no DoubleRow)
                                    for c2 in range(2):
                                        nc.tensor.matmul(
                                            pw[:, gc, :],
                                            w2s[:, 2 * fcp + c2,
                                                gc * P:(gc + 1) * P],
                                            avn[:, 2 * fcp + c2, :],
                                            start=(h == 0 and fcp == 0
                                                   and c2 == 0),
                                            stop=(h == 3 and fcp == 1
                                                  and c2 == 1))
                                else:
                                    nc.tensor.matmul(
                                        pw[:, gc, :],
                                        w2s[:, 2 * fcp:2 * fcp + 2,
                                            gc * P:(gc + 1) * P],
                                        avn[:, 2 * fcp:2 * fcp + 2, :],
                                        start=(h == 0 and fcp == 0),
                                        stop=(h == 3 and fcp == 1),
                                        perf_mode=DR)
                    # epilogue: all 4 heads accumulated; + acc bias -> bf16
                    cw = (1.0 / SW2) if bf else C_W2
                    for gc in range(DC):
                        nc.scalar.activation(
                            fbf[:, gc, qsl], pw[:, gc, :], AF.Identity,
                            scale=cw, bias=bias_sb["sa", "acc"][:, gc:gc + 1])

                for half in range(2):
                    nc.sync.dma_start(
                        out=cc_in[half].rearrange("(c p) s -> p c s", p=P),
                        in_=fbf[:, :, half * SW:(half + 1) * SW])

            # one pair collective: reduce partial head-sums + scatter seq halves
            nc.gpsimd.collective_compute(
                "ReduceScatter", mybir.AluOpType.add, replica_groups=PAIRS,
                ins=[cc_in.opt()], outs=[cc_half.opt()])

            # ---- cross-attention K/V projections.  All 8 heads emitted into
            # a 4-deep rotating pool: the first ~3 run during the collective,
            # the rest pipeline as the head-outer attention loop frees slots.
            ln1_anchor = [None]
            ca_kt8, ca_v8 = {}, {}

            def ca_kv_proj(h, rr=None):
                wk_s = kvwp.tile([P, DC, D], F8, name=f"ca_wk_{h}", tag="wk")
                nc.sync.dma_start(out=wk_s[:], in_=ca_d["wk8"].ap()[h]
                                  .rearrange("(c p) f -> p c f", p=P))
                wv_s = kvwp.tile([P, DC, D], F8, name=f"ca_wv_{h}", tag="wv")
                nc.sync.dma_start(out=wv_s[:], in_=ca_d["wv8"].ap()[h]
                                  .rearrange("(c p) f -> p c f", p=P))
                ca_kt8[h] = kvpp.tile([P, DC, T], F8, name=f"ca_ktp_{h}",
                                      tag="cktp")
                ca_v8[h] = kvpp.tile([P, TC, D], F8, name=f"ca_vp_{h}",
                                     tag="cvp")
                return proj_head(et8_sb, wk_s, wv_s, None, None,
                                 ca_kt8[h], ca_v8[h], None, 0,
                                 rr or [nc.vector, nc.scalar])

            for h in range(4):
                last = ca_kv_proj(h)
                if h == 2:
                    ln1_anchor[0] = last

            # ---- LN1 on my sequence half ----
            cch_sb = earlyp.tile([P, DC, SW], BF16, name="cch_sb",
                                 tag="cch")
            nc.sync.dma_start(
                out=cch_sb[:],
                in_=cc_half.opt().rearrange("(c p) s -> p c s", p=P))
            x1_sb = residp.tile([P, DC, SW], F32R, name="x1_sb", tag="resid")
            from concourse.tile import add_dep_helper as _adh
            _bb = nc.main_func.blocks[-1]
            _n0 = len(_bb.instructions)
            layernorm(cch_sb, xres_sb, x1_sb, ln_sb["ln1_b"], 0)
            x18_sb = residp.tile([P, DC, SW], F8, name="x18_sb", tag="x18",
                                 bufs=1)
            for c in range(DC):
                nc.scalar.activation(x18_sb[:, c, :], x1_sb[:, c, :],
                                     AF.Copy, scale=SX)
            if ln1_anchor[0] is not None:
                for _ins in list(_bb.instructions)[_n0:]:
                    _adh(_ins, ln1_anchor[0].ins, sync=False,
                         reason="order LN1 after CA-KV precompute h2")
            early_scope.__exit__(None, None, None)

            # FFN weights prefetch (transfers overlap CA attention)
            ffn_w_scope = tc.tile_pool(name="ffn_w", bufs=1)
            fwp = ffn_w_scope.__enter__()
            fc1_sb = fwp.tile([P, DC, 2048], BF16, name="fc1_sb")
            fc2_sb = fwp.tile([P, MC, D], BF16, name="fc2_sb")

            # ============ cross-attention (seq-split, all heads) ============
            with tc.tile_pool(name="ca_w", bufs=1) as cwp, \
                 tc.tile_pool(name="ca_qt", bufs=1) as cqtp, \
                 tc.tile_pool(name="ca_e", bufs=2) as ep, \
                 tc.tile_pool(name="ca_av", bufs=2) as avp, \
                 tc.tile_pool(name="ca_f", bufs=1) as fp:
                wq_c, w2_c = {}, {}
                for h in range(H):
                    w2_c[h] = cwp.tile([P, DC, D], F8, name=f"ca_w2_{h}",
                                       tag=f"cw2{h}")
                    nc.sync.dma_start(out=w2_c[h][:], in_=ca_d["w28"].ap()[h]
                                      .rearrange("(c p) f -> p c f", p=P))
                # wq tiles 2-buffered: freed as Q projections complete
                cwqp_scope = tc.tile_pool(name="ca_wq", bufs=2)
                cwqp = cwqp_scope.__enter__()
                for h in range(H):
                    wq_c[h] = cwqp.tile([P, DC, D], F8, name=f"ca_wq_{h}",
                                        tag="cwq")
                    nc.sync.dma_start(out=wq_c[h][:], in_=ca_d["wq8"].ap()[h]
                                      .rearrange("(c p) f -> p c f", p=P))
                for mg in range(4):
                    nc.sync.dma_start(
                        out=fc1_sb[:, :, mg * SW:(mg + 1) * SW],
                        in_=fc1_d.ap().rearrange("(c p) m -> p c m", p=P)
                        [:, :, mg * SW:(mg + 1) * SW])
                nc.sync.dma_start(out=fc2_sb[:], in_=fc2_d.ap()
                                  .rearrange("(c p) g -> p c g", p=P))
                qt8_c = {}
                for h in range(H):
                    qt8_c[h] = cqtp.tile([P, DC, SW], F8, name=f"ca_qt_{h}",
                                         tag=f"cq{h}")
                    _ = wq_c[h]  # DMA emitted above; tiles rotate via pool
                    for fc in range(DC):
                        ps = pp.tile([P, SW], F32, name=f"cqp_{h}_{fc}",
                                     tag="pp")
                        for nt in range(2):
                            for cp in range(2):
                                nc.tensor.matmul(
                                    ps[:, nt * QW:(nt + 1) * QW],
                                    wq_c[h][:, 2 * cp:2 * cp + 2,
                                            fc * P:(fc + 1) * P],
                                    x18_sb[:, 2 * cp:2 * cp + 2,
                                           nt * QW:(nt + 1) * QW],
                                    start=(cp == 0), stop=(cp == 1),
                                    perf_mode=DR)
                        if h % 2:
                            nc.scalar.activation(
                                qt8_c[h][:, fc, :], ps[:], AF.Identity,
                                scale=C_Q,
                                bias=bias_sb["ca", "bq"][:, h, fc:fc + 1])
                        else:
                            nc.vector.tensor_scalar(
                                qt8_c[h][:, fc, :], ps[:], C_Q,
                                bias_sb["ca", "bq"][:, h, fc:fc + 1],
                                OP.mult, OP.add)
                cwqp_scope.__exit__(None, None, None)

                f2_sb = fp.tile([P, DC, SW], F32, name="ca_f2")
                for h in range(H):
                    e8 = ep.tile([P, 2 * TC, QW], F8, name=f"ca_e_{h}",
                                 tag="e")
                    dn = pp.tile([1, 2, QW], F32, name=f"ca_dn_{h}", tag="pp")
                    rb = pp.tile([P, 2, QW], F32, name=f"ca_rb_{h}", tag="pp")
                    for qt in range(2):
                        qsl = slice(qt * QW, (qt + 1) * QW)
                        for jp in range(4):
                            sps = scp.tile([P, 2, QW], F32,
                                           name=f"ca_s_{qt}_{h}_{jp}", tag="sc")
                            for j2 in range(2):
                                kb = 2 * jp + j2
                                for fcp in range(2):
                                    nc.tensor.matmul(
                                        sps[:, j2, :],
                                        ca_kt8[h][:, 2 * fcp:2 * fcp + 2,
                                                  kb * P:(kb + 1) * P],
                                        qt8_c[h][:, 2 * fcp:2 * fcp + 2, qsl],
                                        start=(fcp == 0), stop=(fcp == 1),
                                        perf_mode=DR)
                            nc.scalar.activation(
                                e8[:, 8 * qt + 2 * jp:8 * qt + 2 * jp + 2, :],
                                sps[:], AF.Exp, scale=C_EXP, bias=ln8_col[:])
                    # denominators after the scores loop (avoids PE
                    # head-of-line wait on each exp)
                    for qt in range(2):
                        for jp in range(4):
                            nc.tensor.matmul(
                                dn[:, qt, :], ones8_sb[:],
                                e8[:, 8 * qt + 2 * jp:8 * qt + 2 * jp + 2, :],
                                start=(jp == 0), stop=(jp == 3),
                                perf_mode=DR)
                    rc = statp.tile([1, 2, QW], F32R, name=f"ca_rc_{h}",
                                    tag="st")
                    with nc.allow_low_precision(reason="recip bcast"):
                        nc.vector.reciprocal(rc[:], dn[:])
                    for qt in range(2):
                        nc.tensor.matmul(rb[:, qt, :], ones_row[:],
                                         rc[:, qt, :], start=True, stop=True)

                    for qt in range(2):
                        qsl = slice(qt * QW, (qt + 1) * QW)
                        avn = avp.tile([P, DC, QW], F8,
                                       name=f"ca_avn_{qt}_{h}", tag="avn")
                        for fp2 in range(2):
                            po = scp.tile([P, 2, QW], F32,
                                          name=f"ca_po_{qt}_{h}_{fp2}",
                                          tag="sc")
                            for f2_ in range(2):
                                fc = 2 * fp2 + f2_
                                for jp in range(4):
                                    nc.tensor.matmul(
                                        po[:, f2_, :],
                                        ca_v8[h][:, 2 * jp:2 * jp + 2,
                                                 fc * P:(fc + 1) * P],
                                        e8[:, 8 * qt + 2 * jp:8 * qt + 2 * jp + 2, :],
                                        start=(jp == 0), stop=(jp == 3),
                                        perf_mode=DR)
                                nc.vector.tensor_tensor(
                                    avn[:, fc, :], po[:, f2_, :],
                                    rb[:, qt, :], OP.mult)
                        pw = pwp.tile([P, DC, QW], F32, name=f"ca_pw_{qt}_{h}",
                                      tag="pw")
                        for gc in range(DC):
                            for fcp in range(2):
                                nc.tensor.matmul(
                                    pw[:, gc, :],
                                    w2_c[h][:, 2 * fcp:2 * fcp + 2,
                                            gc * P:(gc + 1) * P],
                                    avn[:, 2 * fcp:2 * fcp + 2, :],
                                    start=(fcp == 0), stop=(fcp == 1),
                                    perf_mode=DR)
                        if h == 0:
                            for gc in range(DC):
                                nc.vector.tensor_scalar(
                                    f2_sb[:, gc, qsl], pw[:, gc, :], C_W2,
                                    bias_sb["ca", "acc"][:, gc:gc + 1],
                                    OP.mult, OP.add)
                        else:
                            nc.vector.scalar_tensor_tensor(
                                f2_sb[:, :, qsl], pw[:], C_W2,
                                f2_sb[:, :, qsl], OP.mult, OP.add)
                    if h + 4 < H:
                        ca_kv_proj(h + 4, rr=[nc.vector, nc.scalar])

                # ---- LN2 ----
                x2_sb = residp.tile([P, DC, SW], F32R, name="x2_sb", tag="resid")
                layernorm(f2_sb, x1_sb, x2_sb, ln_sb["ln2_b"], 1)
                x2b_sb = residp.tile([P, DC, SW], BF16, name="x2b_sb",
                                     tag="x2b", bufs=1)
                for c in range(DC):
                    nc.gpsimd.tensor_copy(x2b_sb[:, c, :], x2_sb[:, c, :])
            # ============ FFN (bf16, seq-split) ============
            with tc.tile_pool(name="ffn_h", bufs=1) as fhp:
                h_sb = fhp.tile([P, MC, SW], BF16, name="h_sb")
                f3_sb = fhp.tile([P, DC, SW], F32, name="f3_sb")
                for mc in range(MC):
                    ps = pp.tile([P, SW], F32, name=f"f1_{mc}", tag="pp")
                    for c in range(DC):
                        nc.tensor.matmul(ps[:], fc1_sb[:, c, mc * P:(mc + 1) * P],
                                         x2b_sb[:, c, :],
                                         start=(c == 0), stop=(c == DC - 1))
                    nc.scalar.activation(h_sb[:, mc, :], ps[:], AF.Relu,
                                         bias=fc1b_sb[:, mc:mc + 1])
                for gc in range(DC):
                    ps = pp.tile([P, SW], F32, name=f"f2_{gc}", tag="pp")
                    for mc in range(MC):
                        nc.tensor.matmul(ps[:],
                                         fc2_sb[:, mc, gc * P:(gc + 1) * P],
                                         h_sb[:, mc, :],
                                         start=(mc == 0), stop=(mc == MC - 1))
                    nc.vector.tensor_scalar_add(f3_sb[:, gc, :], ps[:],
                                                ffnb_sb[:, gc:gc + 1])

                # ---- LN3 + output ----
                out_sb = residp.tile([P, DC, SW], F32R, name="out_sb",
                                     tag="resid")
                layernorm(f3_sb, x2_sb, out_sb, ln_sb["ln3_b"], 2)
                for c in range(DC):
                    nc.sync.dma_start(
                        out=outt_d.ap().rearrange("(c p) s -> p c s", p=P)[:, c, :],
                        in_=out_sb[:, c, :].bitcast(F32))

            ffn_w_scope.__exit__(None, None, None)
            ca_kvw_scope.__exit__(None, None, None)
            ca_kv_scope.__exit__(None, None, None)

    nc.compile()
    return nc


def get_program():
    if "nc" not in _CACHE:
        _CACHE["nc"] = build_program()
    return _CACHE["nc"]


def kernel(**inputs) -> np.ndarray:
    from concourse.bass_utils import run_bass_kernel_spmd
    nc = get_program()
    in_maps = _host_prep(inputs)
    res = run_bass_kernel_spmd(nc, in_maps, core_ids=list(range(NC)))
    out = np.empty((B, S, D), np.float32)
    for b in range(B):
        out[b, 0:SW] = res.results[2 * b]["outt"].T
        out[b, SW:S] = res.results[2 * b + 1]["outt"].T
    return out


# revision 42
# speedup vs baseline: 1.0974x; 1.0097x over previous
"""Trainium2 Bass kernel for nn_Decoder (dense transformer decoder layer).

Problem (hardcoded): B=4, S=T=1024, D=512, H=8 heads, fp32.
  h  = MHA_self(x, causal) ; x1 = LN(h + x)
  h  = MHA_cross(x1, encod_out) ; x2 = LN(h + x1)
  ff = relu(x2 @ fc1) @ fc2 ; out = LN(ff + x2)

Sharding (8 cores = 4 batch groups x 2-core pairs), same as the f32r
baseline: self-attention tensor-parallel over heads (4 heads/core, full
S); one pair ReduceScatter (bf16) combines partial head-sums and splits
the sequence; LN1, cross-attn (all 8 heads, redundant K/V), LN2, FFN,
LN3 run sequence-parallel on the core's 512-row half.

Speed: nearly all matmuls run as fp8(e4m3) DoubleRow (contraction 256
per instruction at 0.5 cycles/row = 4x the f32r rate in the cost
model).  Numerics (validated against the fp32 reference in numpy):
  - projections/scores/AV/W2 fp8 with power-of-2 scales folded into
    ACT epilogues (x*8, w*512, Q/K/V*32, e*8, w2*2048)
  - bk dropped (exact softmax invariance); bv/bo/bf folded into acc
  - causal tail fix: attention output for early tokens is dominated by
    a single V row, so V for keys 0-255 is recomputed with bf16
    operands and query-tile 0 runs its AV/W2 path in bf16
  - FFN entirely bf16 (fp8 FFN alone costs ~1.5e-2 rel err)
  - residuals/LN in f32; ReduceScatter in bf16
Attention is query-tile-outer / head-inner so the W2 output accumulates
across all heads in one pinned PSUM tile (single epilogue per tile).
"""
import math
import numpy as np

B, S, T, D, H = 4, 1024, 1024, 512, 8
P = 128
NC = 8
DC = D // P    # 4 feature chunks
TC = T // P    # 8 time chunks
SW = 512       # per-core sequence half
QW = 256       # query tile width (DoubleRow moving limit)
MC = 2048 // P  # 16 FFN hidden chunks
EPS = 1e-5
PAIRS = [[0, 1], [2, 3], [4, 5], [6, 7]]

# fp8 scales (powers of 2; folded into f32 epilogue constants)
SX = 8.0       # x / x1 / enc quant
SWQ = 512.0    # wq/wk/wv quant
SQ = 32.0      # Q requant
SK = 32.0      # K requant
SV = 32.0      # V requant
SE = 8.0       # exp output
SW2 = 2048.0   # folded w2 quant
RSQD = 1.0 / math.sqrt(D)

_CACHE = {}


def _host_prep(inputs):
    import ml_dtypes
    F8 = ml_dtypes.float8_e4m3
    BF = ml_dtypes.bfloat16
    x = np.asarray(inputs["x"], np.float32)
    enc = np.asarray(inputs["encod_out"], np.float32)

    def q8(a, s):
        return (np.asarray(a, np.float32) * s).astype(F8)

    per_phase = {}
    for p in ("sa", "ca"):
        wq = np.asarray(inputs[p + "_wq"], np.float32)
        bq = np.asarray(inputs[p + "_bq"], np.float32)
        wk = np.asarray(inputs[p + "_wk"], np.float32)
        wv = np.asarray(inputs[p + "_wv"], np.float32)
        bv = np.asarray(inputs[p + "_bv"], np.float32)
        wo = np.asarray(inputs[p + "_wo"], np.float32)
        bo = np.asarray(inputs[p + "_bo"], np.float32)
        wf = np.asarray(inputs[p + "_wf"], np.float32).reshape(H, D, D)
        bf = np.asarray(inputs[p + "_bf"], np.float32)
        w2 = np.einsum("hfg,hgk->hfk", wo.astype(np.float64),
                       wf.astype(np.float64)).astype(np.float32)
        acc = bf.astype(np.float64).copy()
        for h in range(H):
            acc += (bv[h].astype(np.float64) @ wo[h].astype(np.float64)
                    + bo[h].astype(np.float64)) @ wf[h].astype(np.float64)
        per_phase[p] = dict(
            wq8=q8(wq, SWQ), wk8=q8(wk, SWQ), wv8=q8(wv, SWQ),
            w28=q8(w2, SW2), wv_bf=wv.astype(BF),
            bq=bq * SQ, acc=acc.astype(np.float32))

    fc1_w = np.asarray(inputs["fc1_w"], np.float32)
    fc1_b = np.asarray(inputs["fc1_b"], np.float32)
    fc2_w = np.asarray(inputs["fc2_w"], np.float32)
    fc2_b = np.asarray(inputs["fc2_b"], np.float32)
    lns = {f"ln{i}_{k}": np.asarray(inputs[f"ln{i}_{k}"], np.float32)
           for i in (1, 2, 3) for k in ("g", "b")}

    # causal masks for the two diagonal key-blocks of each 256-query tile:
    # kb 2i: keep p <= c ; kb 2i+1: keep 128+p <= c   (c in 0..255)
    pp_ = np.arange(P)[:, None]
    cc = np.arange(QW)[None, :]
    mpair = np.stack([(pp_ <= cc), (P + pp_ <= cc)], axis=1)
    ones_pair = np.ones((P, 2, 1), np.float32)

    in_maps = []
    for c in range(NC):
        b, half = c // 2, c % 2
        hs = slice(4 * half, 4 * half + 4)
        ssl = slice(half * SW, (half + 1) * SW)
        xt = x[b].T
        m = {
            "xt8": np.ascontiguousarray((xt * SX)).astype(F8),
            "xbf": np.ascontiguousarray(xt[:, :QW]).astype(BF),
            "x_res": np.ascontiguousarray(xt[:, ssl]),
            "et8": np.ascontiguousarray(enc[b].T * SX).astype(F8),
            "masks8": mpair.astype(F8),
            "ones8": ones_pair.astype(F8),
            "onesbf": np.ones((P, 1), BF),
            "ones_row": np.ones((1, P), np.float32),
            "fc1bf": fc1_w.astype(BF), "fc1_b": fc1_b,
            "fc2bf": fc2_w.astype(BF), "ffn_bias": fc2_b,
        }
        pp = per_phase["sa"]
        for k in ("wq8", "wk8", "wv8", "w28", "wv_bf", "bq"):
            m["sa_" + k] = np.ascontiguousarray(pp[k][hs])
        m["sa_acc"] = pp["acc"] / 2.0
        pp = per_phase["ca"]
        for k in ("wq8", "wk8", "wv8", "w28", "bq"):
            m["ca_" + k] = pp[k]
        m["ca_acc"] = pp["acc"]
        for k, v in lns.items():
            m[k] = v
        in_maps.append(m)
    return in_maps


def build_program():
    import concourse.bacc as bacc
    import concourse.mybir as mybir
    import concourse.tile as tile

    F32 = mybir.dt.float32
    F32R = mybir.dt.float32r
    BF16 = mybir.dt.bfloat16
    F8 = mybir.dt.float8e4
    AF = mybir.ActivationFunctionType
    OP = mybir.AluOpType
    DR = mybir.MatmulPerfMode.DoubleRow

    nc = bacc.Bacc(None, target_bir_lowering=False, num_devices=NC)

    # ---- DRAM I/O ----
    xt8_d = nc.dram_tensor("xt8", [D, S], F8, kind="ExternalInput")
    xbf_d = nc.dram_tensor("xbf", [D, QW], BF16, kind="ExternalInput")
    xres_d = nc.dram_tensor("x_res", [D, SW], F32, kind="ExternalInput")
    et8_d = nc.dram_tensor("et8", [D, T], F8, kind="ExternalInput")
    masks8_d = nc.dram_tensor("masks8", [P, 2, QW], F8, kind="ExternalInput")
    ones8_d = nc.dram_tensor("ones8", [P, 2, 1], F8, kind="ExternalInput")
    onesbf_d = nc.dram_tensor("onesbf", [P, 1], BF16, kind="ExternalInput")
    onesrow_d = nc.dram_tensor("ones_row", [1, P], F32, kind="ExternalInput")
    sa_d = {k: nc.dram_tensor("sa_" + k, [4, D, D], F8, kind="ExternalInput")
            for k in ("wq8", "wk8", "wv8", "w28")}
    sa_d["wv_bf"] = nc.dram_tensor("sa_wv_bf", [4, D, D], BF16, kind="ExternalInput")
    sa_d["bq"] = nc.dram_tensor("sa_bq", [4, D], F32, kind="ExternalInput")
    sa_d["acc"] = nc.dram_tensor("sa_acc", [D], F32, kind="ExternalInput")
    ca_d = {k: nc.dram_tensor("ca_" + k, [H, D, D], F8, kind="ExternalInput")
            for k in ("wq8", "wk8", "wv8", "w28")}
    ca_d["bq"] = nc.dram_tensor("ca_bq", [H, D], F32, kind="ExternalInput")
    ca_d["acc"] = nc.dram_tensor("ca_acc", [D], F32, kind="ExternalInput")
    fc1_d = nc.dram_tensor("fc1bf", [D, 2048], BF16, kind="ExternalInput")
    fc1b_d = nc.dram_tensor("fc1_b", [2048], F32, kind="ExternalInput")
    fc2_d = nc.dram_tensor("fc2bf", [2048, D], BF16, kind="ExternalInput")
    ffnb_d = nc.dram_tensor("ffn_bias", [D], F32, kind="ExternalInput")
    ln_d = {f"ln{i}_{k}": nc.dram_tensor(f"ln{i}_{k}", [D], F32, kind="ExternalInput")
            for i in (1, 2, 3) for k in ("g", "b")}
    outt_d = nc.dram_tensor("outt", [D, SW], F32, kind="ExternalOutput")

    r32 = lambda ap: ap.bitcast(F32R)

    # epilogue constants
    C_Q = SQ / (SX * SWQ)
    C_K = SK / (SX * SWQ)
    C_V = SV / (SX * SWQ)
    C_EXP = RSQD / (SQ * SK)     # scale on score psum inside exp
    C_W2 = 1.0 / (SV * SW2)      # scale on fp8 W2 psum
    LN_SE = math.log(SE)

    with tile.TileContext(nc, pool_alloc_mode="queue") as tc:
        with tc.tile_pool(name="const", bufs=1) as constp, \
             tc.tile_pool(name="resid", bufs=2) as residp, \
             tc.tile_pool(name="smalls", bufs=3) as smallp, \
             tc.tile_pool(name="stats", bufs=4) as statp, \
             tc.tile_pool(name="pp", bufs=2, space="PSUM") as pp, \
             tc.tile_pool(name="sc", bufs=4, space="PSUM") as scp, \
             tc.tile_pool(name="pw", bufs=1, space="PSUM") as pwp, \
             tc.tile_pool(name="dram", bufs=1, space="DRAM") as dramp:

            # ---- constants ----
            eps_sb = constp.tile([1, 1], F32, name="eps_sb")
            nc.vector.memset(eps_sb[:], EPS)
            ln8_col = constp.tile([P, 1], F32, name="ln8_col")
            nc.vector.memset(ln8_col[:], LN_SE)
            zero_col = constp.tile([P, 1], F32, name="zero_col")
            nc.vector.memset(zero_col[:], 0.0)
            ones_col = constp.tile([P, 1], F32R, name="ones_col")
            nc.vector.memset(ones_col[:], 1.0)

            xt8_sb = residp.tile([P, DC, S], F8, name="xt8_sb", tag="resid")
            nc.sync.dma_start(out=xt8_sb[:],
                              in_=xt8_d.ap().rearrange("(c p) s -> p c s", p=P))
            xbf_sb = constp.tile([P, DC, QW], BF16, name="xbf_sb")
            nc.scalar.dma_start(out=xbf_sb[:],
                                in_=xbf_d.ap().rearrange("(c p) s -> p c s", p=P))
            ca_kv_scope = tc.tile_pool(name="ca_kvp", bufs=4)
            kvpp = ca_kv_scope.__enter__()
            ca_kvw_scope = tc.tile_pool(name="ca_kvw", bufs=2)
            kvwp = ca_kvw_scope.__enter__()
            early_scope = tc.tile_pool(name="early", bufs=1)
            earlyp = early_scope.__enter__()
            xres_sb = earlyp.tile([P, DC, SW], F32, name="xres_sb", tag="xres")
            nc.scalar.dma_start(out=xres_sb[:],
                                in_=xres_d.ap().rearrange("(c p) s -> p c s", p=P))
            et8_sb = kvwp.tile([P, DC, T], F8, name="et8_sb", tag="et",
                               bufs=1)
            nc.scalar.dma_start(out=et8_sb[:],
                                in_=et8_d.ap().rearrange("(c p) s -> p c s", p=P))

            masks8_sb = constp.tile([P, 2, QW], F8, name="masks8_sb")
            nc.scalar.dma_start(out=masks8_sb[:], in_=masks8_d.ap())
            ones8_sb = constp.tile([P, 2, 1], F8, name="ones8_sb")
            nc.scalar.dma_start(out=ones8_sb[:], in_=ones8_d.ap())
            onesbf_sb = constp.tile([P, 1], BF16, name="onesbf_sb")
            nc.scalar.dma_start(out=onesbf_sb[:], in_=onesbf_d.ap())
            ones_row = constp.tile([1, P], F32R, name="ones_row")
            nc.scalar.dma_start(out=ones_row[:], in_=r32(onesrow_d.ap()))

            def vec_to_pc(dram_ap, name, nch):
                t = constp.tile([P, nch], F32, name=name)
                nc.scalar.dma_start(out=t[:],
                                    in_=dram_ap.rearrange("(c p) -> p c", p=P))
                return t

            bias_sb = {}
            for pn, dd, nh in (("sa", sa_d, 4), ("ca", ca_d, H)):
                t = constp.tile([P, nh, DC], F32, name=f"{pn}_bq_sb")
                nc.scalar.dma_start(
                    out=t[:], in_=dd["bq"].ap().rearrange("h (c p) -> p h c", p=P))
                bias_sb[pn, "bq"] = t
                bias_sb[pn, "acc"] = vec_to_pc(dd["acc"].ap(), f"{pn}_acc_sb", DC)
            grow_sb = constp.tile([1, 3, DC, P], F32R, name="ln_grow")
            for _i in (1, 2, 3):
                nc.scalar.dma_start(
                    out=grow_sb[:, _i - 1, :, :],
                    in_=r32(ln_d[f"ln{_i}_g"].ap().rearrange(
                        "(a c p) -> a c p", a=1, p=P)))
            fc1b_sb = vec_to_pc(fc1b_d.ap(), "fc1b_sb", MC)
            ffnb_sb = vec_to_pc(ffnb_d.ap(), "ffnb_sb", DC)
            ln_sb = {k: vec_to_pc(v.ap(), k + "_sb", DC) for k, v in ln_d.items()}

            cc_in = dramp.tile([2, D, SW], BF16, name="cc_in")
            cc_half = dramp.tile([D, SW], BF16, name="cc_half")

            def layernorm_half(src, resid_sb, dst, b_sb, gri, csl):
                """dst[:, :, csl] = LN(src + resid) on a column slice."""
                W = csl.stop - csl.start
                for c in range(DC):
                    nc.gpsimd.tensor_add(dst[:, c, csl], src[:, c, csl],
                                         resid_sb[:, c, csl])
                psum_sum = pp.tile([1, SW], F32, name="ln_sum", tag="pp")
                psum_ssq = pp.tile([1, SW], F32, name="ln_ssq", tag="pp")
                for c in range(DC):
                    sq = smallp.tile([P, SW], F32R, name=f"ln_sq_{c}", tag="sm")
                    nc.scalar.activation(sq[:, :W], dst[:, c, csl], AF.Square)
                    nc.tensor.matmul(psum_sum[:, :W], ones_col[:],
                                     dst[:, c, csl],
                                     start=(c == 0), stop=(c == DC - 1))
                    nc.tensor.matmul(psum_ssq[:, :W], ones_col[:], sq[:, :W],
                                     start=(c == 0), stop=(c == DC - 1))
                mean = statp.tile([1, SW], F32R, name="ln_mean", tag="st")
                nc.scalar.activation(mean[:, :W], psum_sum[:, :W], AF.Copy,
                                     scale=1.0 / D)
                msq = statp.tile([1, SW], F32, name="ln_msq", tag="st")
                nc.scalar.activation(msq[:, :W], psum_ssq[:, :W], AF.Copy,
                                     scale=1.0 / D)
                var = statp.tile([1, SW], F32, name="ln_var", tag="st")
                nc.vector.tensor_tensor(var[:, :W], mean[:, :W], mean[:, :W],
                                        OP.mult)
                nc.vector.tensor_sub(var[:, :W], msq[:, :W], var[:, :W])
                std = statp.tile([1, SW], F32, name="ln_std", tag="st")
                nc.scalar.activation(std[:, :W], var[:, :W], AF.Sqrt,
                                     bias=eps_sb[:])
                rstd = statp.tile([1, SW], F32R, name="ln_rstd", tag="st")
                with nc.allow_low_precision(reason="f32r feed for bcast matmul"):
                    nc.vector.reciprocal(rstd[:, :W], std[:, :W])
                mr = statp.tile([1, SW], F32R, name="ln_mr", tag="st")
                nc.vector.tensor_tensor(mr[:, :W], mean[:, :W], rstd[:, :W],
                                        OP.mult)
                for c in range(DC):
                    psum_rb = pp.tile([P, SW], F32, name=f"ln_rb_{c}", tag="pp")
                    nc.tensor.matmul(psum_rb[:, :W], grow_sb[:, gri, c, :],
                                     rstd[:, :W], start=True, stop=True)
                    nc.tensor.matmul(psum_rb[:, W:2 * W],
                                     grow_sb[:, gri, c, :],
                                     mr[:, :W], start=True, stop=True)
                    tmp = smallp.tile([P, SW], F32, name=f"ln_t_{c}", tag="sm")
                    nc.vector.tensor_tensor(tmp[:, :W], dst[:, c, csl],
                                            psum_rb[:, :W], OP.mult)
                    nc.vector.scalar_tensor_tensor(
                        dst[:, c, csl], tmp[:, :W], b_sb[:, c:c + 1],
                        psum_rb[:, W:2 * W], OP.add, OP.subtract)

            def layernorm(src, resid_sb, dst, b_sb, gri):
                for hlf in range(2):
                    layernorm_half(src, resid_sb, dst, b_sb, gri,
                                   slice(hlf * QW, (hlf + 1) * QW))

            def epi_rr(eng, out_ap, ps_ap, cscale, bias_ap):
                """projection epilogue out = ps*c (+bias) on a chosen engine"""
                if eng is nc.scalar:
                    return nc.scalar.activation(
                        out_ap, ps_ap, AF.Identity, scale=cscale,
                        bias=bias_ap if bias_ap is not None else zero_col[:])
                if bias_ap is None:
                    return eng.tensor_scalar(out_ap, ps_ap, cscale, None,
                                             OP.mult)
                return eng.tensor_scalar(out_ap, ps_ap, cscale, bias_ap,
                                         OP.mult, OP.add)

            def proj_head(src_sb, wk_sb, wv_sb, wq_ap, bq_ap, kt8, v8, qt8,
                          nq, rr):
                """fp8 K^T [f,t], V [t,f], and optional Q^T [f,q] for one
                head.  Epilogues round-robin across Pool/ACT/DVE so all
                three drain in parallel."""
                last = None
                nrr = len(rr)
                ei = [0]
                def nxt():
                    e = rr[ei[0] % nrr]; ei[0] += 1
                    return e
                for fc in range(DC):
                    for th in range(2):
                        ps = pp.tile([P, SW], F32, name=f"kp_{fc}_{th}",
                                     tag="pp")
                        for nt in range(2):
                            for cp in range(2):
                                nc.tensor.matmul(
                                    ps[:, nt * QW:(nt + 1) * QW],
                                    wk_sb[:, 2 * cp:2 * cp + 2,
                                          fc * P:(fc + 1) * P],
                                    xt8_slice(src_sb, cp,
                                              th * SW + nt * QW),
                                    start=(cp == 0), stop=(cp == 1),
                                    perf_mode=DR)
                        epi_rr(nxt(), kt8[:, fc, th * SW:(th + 1) * SW],
                               ps[:], C_K, None)
                    for tci in (2 * fc, 2 * fc + 1):
                        ps = scp.tile([P, 2, QW], F32, name=f"vp_{tci}",
                                      tag="sc")
                        for fh in range(2):
                            for cp in range(2):
                                nc.tensor.matmul(
                                    ps[:, fh, :],
                                    src_sb[:, 2 * cp:2 * cp + 2,
                                           tci * P:(tci + 1) * P],
                                    wv_sb[:, 2 * cp:2 * cp + 2,
                                          fh * QW:(fh + 1) * QW],
                                    start=(cp == 0), stop=(cp == 1),
                                    perf_mode=DR)
                        last = epi_rr(nxt(), v8[:, tci, :], ps[:], C_V, None)
                    if wq_ap is not None:
                        for qh in range(nq):
                            ps = pp.tile([P, SW], F32, name=f"qp_{fc}_{qh}",
                                         tag="pp")
                            for nt in range(2):
                                for cp in range(2):
                                    nc.tensor.matmul(
                                        ps[:, nt * QW:(nt + 1) * QW],
                                        wq_ap[:, 2 * cp:2 * cp + 2,
                                              fc * P:(fc + 1) * P],
                                        xt8_slice(src_sb, cp,
                                                  qh * SW + nt * QW),
                                        start=(cp == 0), stop=(cp == 1),
                                        perf_mode=DR)
                            epi_rr(nxt(), qt8[:, fc, qh * SW:(qh + 1) * SW],
                                   ps[:], C_Q, bq_ap[:, fc:fc + 1])
                return last

            def xt8_slice(src_sb, cp, q0):
                return src_sb[:, 2 * cp:2 * cp + 2, q0:q0 + QW]

            # ============ self-attention (head-split, full S) ============
            with tc.tile_pool(name="sa_w2", bufs=1) as w2p, \
                 tc.tile_pool(name="sa_qkv", bufs=1) as qkvp, \
                 tc.tile_pool(name="sa_e", bufs=3) as ep, \
                 tc.tile_pool(name="sa_av", bufs=2) as avp, \
                 tc.tile_pool(name="sa_f", bufs=1) as fp:
                wkv_scope = tc.tile_pool(name="sa_wkv", bufs=2)
                wkvp = wkv_scope.__enter__()
                # projections for all 4 heads; per-head JIT weight DMA
                wq_sb, wk_sb, wv_sb, wvb_sb, w28_sb, w2b_sb = {}, {}, {}, {}, {}, {}
                kt8, v8, vbf, qt8 = {}, {}, {}, {}
                for h in range(4):
                    wk_sb[h] = wkvp.tile([P, DC, D], F8, name=f"sa_wk_{h}",
                                         tag="wk")
                    nc.sync.dma_start(out=wk_sb[h][:], in_=sa_d["wk8"].ap()[h]
                                      .rearrange("(c p) f -> p c f", p=P))
                    wv_sb[h] = wkvp.tile([P, DC, D], F8, name=f"sa_wv_{h}",
                                         tag="wv")
                    nc.sync.dma_start(out=wv_sb[h][:], in_=sa_d["wv8"].ap()[h]
                                      .rearrange("(c p) f -> p c f", p=P))
                    wq_sb[h] = wkvp.tile([P, DC, D], F8, name=f"sa_wq_{h}",
                                         tag="wq")
                    nc.sync.dma_start(out=wq_sb[h][:], in_=sa_d["wq8"].ap()[h]
                                      .rearrange("(c p) f -> p c f", p=P))
                    wvb_sb[h] = wkvp.tile([P, DC, D], BF16, name=f"sa_wvb_{h}",
                                          tag="wvb")
                    nc.sync.dma_start(out=wvb_sb[h][:], in_=sa_d["wv_bf"].ap()[h]
                                      .rearrange("(c p) f -> p c f", p=P))
                    kt8[h] = qkvp.tile([P, DC, S], F8, name=f"sa_kt_{h}",
                                       tag=f"kt{h}")
                    v8[h] = qkvp.tile([P, TC, D], F8, name=f"sa_v_{h}",
                                      tag=f"v{h}")
                    qt8[h] = qkvp.tile([P, DC, S], F8, name=f"sa_qt_{h}",
                                       tag=f"qt{h}")
                    proj_head(xt8_sb, wk_sb[h], wv_sb[h], wq_sb[h],
                              bias_sb["sa", "bq"][:, h, :], kt8[h], v8[h],
                              qt8[h], 2, [nc.vector, nc.scalar])
                    # bf16 V for keys 0..255 (early-token numerics)
                    vbf[h] = qkvp.tile([P, 2, D], BF16, name=f"sa_vb_{h}",
                                       tag=f"vb{h}")
                    for tci in range(2):
                        ps = pp.tile([P, SW], F32, name=f"vbp_{h}_{tci}",
                                     tag="pp")
                        for c in range(DC):
                            nc.tensor.matmul(
                                ps[:], xbf_sb[:, c, tci * P:(tci + 1) * P],
                                wvb_sb[h][:, c, :],
                                start=(c == 0), stop=(c == DC - 1))
                        nc.vector.tensor_copy(vbf[h][:, tci, :], ps[:])
                wkv_scope.__exit__(None, None, None)
                # W2 weights arrive after the projection weights
                for h in range(4):
                    w28_sb[h] = w2p.tile([P, DC, D], F8, name=f"sa_w28_{h}",
                                         tag=f"w28{h}")
                    nc.sync.dma_start(out=w28_sb[h][:], in_=sa_d["w28"].ap()[h]
                                      .rearrange("(c p) f -> p c f", p=P))

                fbf = fp.tile([P, DC, S], BF16, name="sa_fbf")

                # attention: query-tile outer, head inner
                for qt in range(4):
                    qsl = slice(qt * QW, (qt + 1) * QW)
                    npair = qt + 1     # key-block pairs for this tile
                    bf = (qt == 0)     # bf16 AV/W2 path for queries 0..255
                    pw = pwp.tile([P, DC, QW], F32, name=f"sa_pw_{qt}", tag="pw")
                    rbs = {}
                    for h in range(4):
                        e8 = ep.tile([P, 2, QW] if bf else [P, TC, QW],
                                     BF16 if bf else F8,
                                     name=f"sa_e_{qt}_{h}",
                                     tag="ebf" if bf else "e",
                                     bufs=2 if bf else None)
                        dn = pp.tile([1, 2, QW], F32, name=f"sa_dn_{qt}_{h}",
                                     tag="pp")
                        for jp in range(npair):
                            sps = scp.tile([P, 2, QW], F32,
                                           name=f"sa_s_{qt}_{h}_{jp}", tag="sc")
                            for j2 in range(2):
                                kb = 2 * jp + j2
                                for fcp in range(2):
                                    nc.tensor.matmul(
                                        sps[:, j2, :],
                                        kt8[h][:, 2 * fcp:2 * fcp + 2,
                                               kb * P:(kb + 1) * P],
                                        qt8[h][:, 2 * fcp:2 * fcp + 2, qsl],
                                        start=(fcp == 0), stop=(fcp == 1),
                                        perf_mode=DR)
                            nc.scalar.activation(
                                e8[:, 2 * jp:2 * jp + 2, :], sps[:], AF.Exp,
                                scale=C_EXP,
                                bias=zero_col[:] if bf else ln8_col[:])
                            if jp == npair - 1:   # diagonal pair: apply mask
                                nc.gpsimd.tensor_tensor(
                                    e8[:, 2 * jp:2 * jp + 2, :],
                                    e8[:, 2 * jp:2 * jp + 2, :],
                                    masks8_sb[:], OP.mult)
                        if bf:
                            for j2 in range(2):
                                nc.tensor.matmul(
                                    dn[:, 0, :], onesbf_sb[:],
                                    e8[:, j2, :],
                                    start=(j2 == 0), stop=(j2 == 1))
                        else:
                            for jp in range(npair):
                                nc.tensor.matmul(
                                    dn[:, 0, :], ones8_sb[:],
                                    e8[:, 2 * jp:2 * jp + 2, :],
                                    start=(jp == 0), stop=(jp == npair - 1),
                                    perf_mode=DR)
                        rc = statp.tile([1, QW], F32R, name=f"sa_rc_{qt}_{h}",
                                        tag="st")
                        with nc.allow_low_precision(reason="recip bcast"):
                            nc.vector.reciprocal(rc[:], dn[:, 0, :])
                        rbp = pp.tile([P, QW], F32, name=f"sa_rbp_{qt}_{h}",
                                      tag="pp")
                        nc.tensor.matmul(rbp[:], ones_row[:], rc[:],
                                         start=True, stop=True)
                        rb = smallp.tile([P, QW], F32, name=f"sa_rb_{qt}_{h}",
                                         tag="sm")
                        nc.scalar.activation(rb[:], rbp[:], AF.Copy)
                        rbs[h] = rb

                        avn = avp.tile([P, DC, QW], BF16 if bf else F8,
                                       name=f"sa_avn_{qt}_{h}",
                                       tag="avnbf" if bf else "avn",
                                       bufs=1 if bf else None)
                        for fp2 in range(2):
                            po = scp.tile([P, 2, QW], F32,
                                          name=f"sa_po_{qt}_{h}_{fp2}",
                                          tag="sc")
                            for f2_ in range(2):
                                fc = 2 * fp2 + f2_
                                if bf:
                                    for j in range(2):
                                        nc.tensor.matmul(
                                            po[:, f2_, :],
                                            vbf[h][:, j, fc * P:(fc + 1) * P],
                                            e8[:, j, :],
                                            start=(j == 0), stop=(j == 1))
                                else:
                                    for jp in range(npair):
                                        nc.tensor.matmul(
                                            po[:, f2_, :],
                                            v8[h][:, 2 * jp:2 * jp + 2,
                                                  fc * P:(fc + 1) * P],
                                            e8[:, 2 * jp:2 * jp + 2, :],
                                            start=(jp == 0),
                                            stop=(jp == npair - 1),
                                            perf_mode=DR)
                                nc.vector.tensor_tensor(
                                    avn[:, fc, :], po[:, f2_, :], rbs[h][:],
                                    OP.mult)
                        w2s = w28_sb[h]
                        for gc in range(DC):
                            for fcp in range(2):
                                if bf:
                                    # bf16 avn x fp8 w2 (no DoubleRow)
                                    for c2 in range(2):
                                        nc.tensor.matmul(
                                            pw[:, gc, :],
                                            w2s[:, 2 * fcp + c2,
                                                gc * P:(gc + 1) * P],
                                            avn[:, 2 * fcp + c2, :],
                                            start=(h == 0 and fcp == 0
                                                   and c2 == 0),
                                            stop=(h == 3 and fcp == 1
                                                  and c2 == 1))
                                else:
                                    nc.tensor.matmul(
                                        pw[:, gc, :],
                                        w2s[:, 2 * fcp:2 * fcp + 2,
                                            gc * P:(gc + 1) * P],
                                        avn[:, 2 * fcp:2 * fcp + 2, :],
                                        start=(h == 0 and fcp == 0),
                                        stop=(h == 3 and fcp == 1),
                                        perf_mode=DR)
                    # epilogue: all 4 heads accumulated; + acc bias -> bf16
                    cw = (1.0 / SW2) if bf else C_W2
                    for gc in range(DC):
                        nc.scalar.activation(
                            fbf[:, gc, qsl], pw[:, gc, :], AF.Identity,
                            scale=cw, bias=bias_sb["sa", "acc"][:, gc:gc + 1])

                for half in range(2):
                    nc.sync.dma_start(
                        out=cc_in[half].rearrange("(c p) s -> p c s", p=P),
                        in_=fbf[:, :, half * SW:(half + 1) * SW])

            # one pair collective: reduce partial head-sums + scatter seq halves
            nc.gpsimd.collective_compute(
                "ReduceScatter", mybir.AluOpType.add, replica_groups=PAIRS,
                ins=[cc_in.opt()], outs=[cc_half.opt()])

            # ---- cross-attention K/V projections.  All 8 heads emitted into
            # a 4-deep rotating pool: the first ~3 run during the collective,
            # the rest pipeline as the head-outer attention loop frees slots.
            ln1_anchor = [None]
            ca_kt8, ca_v8 = {}, {}

            def ca_kv_proj(h, rr=None):
                wk_s = kvwp.tile([P, DC, D], F8, name=f"ca_wk_{h}", tag="wk")
                nc.sync.dma_start(out=wk_s[:], in_=ca_d["wk8"].ap()[h]
                                  .rearrange("(c p) f -> p c f", p=P))
                wv_s = kvwp.tile([P, DC, D], F8, name=f"ca_wv_{h}", tag="wv")
                nc.sync.dma_start(out=wv_s[:], in_=ca_d["wv8"].ap()[h]
                                  .rearrange("(c p) f -> p c f", p=P))
                ca_kt8[h] = kvpp.tile([P, DC, T], F8, name=f"ca_ktp_{h}",
                                      tag="cktp")
                ca_v8[h] = kvpp.tile([P, TC, D], F8, name=f"ca_vp_{h}",
                                     tag="cvp")
                return proj_head(et8_sb, wk_s, wv_s, None, None,
                                 ca_kt8[h], ca_v8[h], None, 0,
                                 rr or [nc.vector, nc.scalar])

            for h in range(4):
                last = ca_kv_proj(h)
                if h == 2:
                    ln1_anchor[0] = last

            # ---- LN1 on my sequence half ----
            cch_sb = earlyp.tile([P, DC, SW], BF16, name="cch_sb",
                                 tag="cch")
            nc.sync.dma_start(
                out=cch_sb[:],
                in_=cc_half.opt().rearrange("(c p) s -> p c s", p=P))
            x1_sb = residp.tile([P, DC, SW], F32R, name="x1_sb", tag="resid")
            from concourse.tile import add_dep_helper as _adh
            _bb = nc.main_func.blocks[-1]
            _n0 = len(_bb.instructions)
            layernorm(cch_sb, xres_sb, x1_sb, ln_sb["ln1_b"], 0)
            x18_sb = residp.tile([P, DC, SW], F8, name="x18_sb", tag="x18",
                                 bufs=1)
            for c in range(DC):
                nc.scalar.activation(x18_sb[:, c, :], x1_sb[:, c, :],
                                     AF.Copy, scale=SX)
            if ln1_anchor[0] is not None:
                for _ins in list(_bb.instructions)[_n0:]:
                    _adh(_ins, ln1_anchor[0].ins, sync=False,
                         reason="order LN1 after CA-KV precompute h2")
            early_scope.__exit__(None, None, None)

            # FFN weights prefetch (transfers overlap CA attention)
            ffn_w_scope = tc.tile_pool(name="ffn_w", bufs=1)
            fwp = ffn_w_scope.__enter__()
            fc1_sb = fwp.tile([P, DC, 2048], BF16, name="fc1_sb")
            fc2_sb = fwp.tile([P, MC, D], BF16, name="fc2_sb")

            # ============ cross-attention (seq-split, all heads) ============
            with tc.tile_pool(name="ca_w", bufs=1) as cwp, \
                 tc.tile_pool(name="ca_qt", bufs=1) as cqtp, \
                 tc.tile_pool(name="ca_e", bufs=2) as ep, \
                 tc.tile_pool(name="ca_av", bufs=2) as avp, \
                 tc.tile_pool(name="ca_f", bufs=1) as fp:
                wq_c, w2_c = {}, {}
                for h in range(H):
                    w2_c[h] = cwp.tile([P, DC, D], F8, name=f"ca_w2_{h}",
                                       tag=f"cw2{h}")
                    nc.sync.dma_start(out=w2_c[h][:], in_=ca_d["w28"].ap()[h]
                                      .rearrange("(c p) f -> p c f", p=P))
                # wq tiles 2-buffered: freed as Q projections complete
                cwqp_scope = tc.tile_pool(name="ca_wq", bufs=2)
                cwqp = cwqp_scope.__enter__()
                for h in range(H):
                    wq_c[h] = cwqp.tile([P, DC, D], F8, name=f"ca_wq_{h}",
                                        tag="cwq")
                    nc.sync.dma_start(out=wq_c[h][:], in_=ca_d["wq8"].ap()[h]
                                      .rearrange("(c p) f -> p c f", p=P))
                for mg in range(4):
                    nc.sync.dma_start(
                        out=fc1_sb[:, :, mg * SW:(mg + 1) * SW],
                        in_=fc1_d.ap().rearrange("(c p) m -> p c m", p=P)
                        [:, :, mg * SW:(mg + 1) * SW])
                nc.sync.dma_start(out=fc2_sb[:], in_=fc2_d.ap()
                                  .rearrange("(c p) g -> p c g", p=P))
                qt8_c = {}
                for h in range(H):
                    qt8_c[h] = cqtp.tile([P, DC, SW], F8, name=f"ca_qt_{h}",
                                         tag=f"cq{h}")
                    _ = wq_c[h]  # DMA emitted above; tiles rotate via pool
                    for fc in range(DC):
                        ps = pp.tile([P, SW], F32, name=f"cqp_{h}_{fc}",
                                     tag="pp")
                        for nt in range(2):
                            for cp in range(2):
                                nc.tensor.matmul(
                                    ps[:, nt * QW:(nt + 1) * QW],
                                    wq_c[h][:, 2 * cp:2 * cp + 2,
                                            fc * P:(fc + 1) * P],
                                    x18_sb[:, 2 * cp:2 * cp + 2,
                                           nt * QW:(nt + 1) * QW],
                                    start=(cp == 0), stop=(cp == 1),
                                    perf_mode=DR)
                        if h % 2:
                            nc.scalar.activation(
                                qt8_c[h][:, fc, :], ps[:], AF.Identity,
                                scale=C_Q,
                                bias=bias_sb["ca", "bq"][:, h, fc:fc + 1])
                        else:
                            nc.vector.tensor_scalar(
                                qt8_c[h][:, fc, :], ps[:], C_Q,
                                bias_sb["ca", "bq"][:, h, fc:fc + 1],
                                OP.mult, OP.add)
                cwqp_scope.__exit__(None, None, None)

                f2_sb = fp.tile([P, DC, SW], F32, name="ca_f2")
                for h in range(H):
                    e8 = ep.tile([P, 2 * TC, QW], F8, name=f"ca_e_{h}",
                                 tag="e")
                    dn = pp.tile([1, 2, QW], F32, name=f"ca_dn_{h}", tag="pp")
                    rb = pp.tile([P, 2, QW], F32, name=f"ca_rb_{h}", tag="pp")
                    for qt in range(2):
                        qsl = slice(qt * QW, (qt + 1) * QW)
                        for jp in range(4):
                            sps = scp.tile([P, 2, QW], F32,
                                           name=f"ca_s_{qt}_{h}_{jp}", tag="sc")
                            for j2 in range(2):
                                kb = 2 * jp + j2
                                for fcp in range(2):
                                    nc.tensor.matmul(
                                        sps[:, j2, :],
                                        ca_kt8[h][:, 2 * fcp:2 * fcp + 2,
                                                  kb * P:(kb + 1) * P],
                                        qt8_c[h][:, 2 * fcp:2 * fcp + 2, qsl],
                                        start=(fcp == 0), stop=(fcp == 1),
                                        perf_mode=DR)
                            nc.scalar.activation(
                                e8[:, 8 * qt + 2 * jp:8 * qt + 2 * jp + 2, :],
                                sps[:], AF.Exp, scale=C_EXP, bias=ln8_col[:])
                    # denominators after the scores loop (avoids PE
                    # head-of-line wait on each exp)
                    for qt in range(2):
                        for jp in range(4):
                            nc.tensor.matmul(
                                dn[:, qt, :], ones8_sb[:],
                                e8[:, 8 * qt + 2 * jp:8 * qt + 2 * jp + 2, :],
                                start=(jp == 0), stop=(jp == 3),
                                perf_mode=DR)
                    rc = statp.tile([1, 2, QW], F32R, name=f"ca_rc_{h}",
                                    tag="st")
                    with nc.allow_low_precision(reason="recip bcast"):
                        nc.vector.reciprocal(rc[:], dn[:])
                    for qt in range(2):
                        nc.tensor.matmul(rb[:, qt, :], ones_row[:],
                                         rc[:, qt, :], start=True, stop=True)
                    rb_sb = smallp.tile([P, 2, QW], F32, name=f"ca_rbs_{h}",
                                        tag="sm")
                    nc.scalar.activation(rb_sb[:], rb[:], AF.Copy)

                    for qt in range(2):
                        qsl = slice(qt * QW, (qt + 1) * QW)
                        avn = avp.tile([P, DC, QW], F8,
                                       name=f"ca_avn_{qt}_{h}", tag="avn")
                        for fp2 in range(2):
                            po = scp.tile([P, 2, QW], F32,
                                          name=f"ca_po_{qt}_{h}_{fp2}",
                                          tag="sc")
                            for f2_ in range(2):
                                fc = 2 * fp2 + f2_
                                for jp in range(4):
                                    nc.tensor.matmul(
                                        po[:, f2_, :],
                                        ca_v8[h][:, 2 * jp:2 * jp + 2,
                                                 fc * P:(fc + 1) * P],
                                        e8[:, 8 * qt + 2 * jp:8 * qt + 2 * jp + 2, :],
                                        start=(jp == 0), stop=(jp == 3),
                                        perf_mode=DR)
                                nc.vector.tensor_tensor(
                                    avn[:, fc, :], po[:, f2_, :],
                                    rb_sb[:, qt, :], OP.mult)
                        pw = pwp.tile([P, DC, QW], F32, name=f"ca_pw_{qt}_{h}",
                                      tag="pw")
                        for gc in range(DC):
                            for fcp in range(2):
                                nc.tensor.matmul(
                                    pw[:, gc, :],
                                    w2_c[h][:, 2 * fcp:2 * fcp + 2,
                                            gc * P:(gc + 1) * P],
                                    avn[:, 2 * fcp:2 * fcp + 2, :],
                                    start=(fcp == 0), stop=(fcp == 1),
                                    perf_mode=DR)
                        if h == 0:
                            for gc in range(DC):
                                nc.vector.tensor_scalar(
                                    f2_sb[:, gc, qsl], pw[:, gc, :], C_W2,
                                    bias_sb["ca", "acc"][:, gc:gc + 1],
                                    OP.mult, OP.add)
                        else:
                            nc.vector.scalar_tensor_tensor(
                                f2_sb[:, :, qsl], pw[:], C_W2,
                                f2_sb[:, :, qsl], OP.mult, OP.add)
                    if h + 4 < H:
                        ca_kv_proj(h + 4, rr=[nc.vector, nc.scalar])

                # ---- LN2 ----
                x2_sb = residp.tile([P, DC, SW], F32R, name="x2_sb", tag="resid")
                layernorm(f2_sb, x1_sb, x2_sb, ln_sb["ln2_b"], 1)
                x2b_sb = residp.tile([P, DC, SW], BF16, name="x2b_sb",
                                     tag="x2b", bufs=1)
                for c in range(DC):
                    nc.gpsimd.tensor_copy(x2b_sb[:, c, :], x2_sb[:, c, :])
            # ============ FFN (bf16, seq-split) ============
            with tc.tile_pool(name="ffn_h", bufs=1) as fhp:
                h_sb = fhp.tile([P, MC, SW], BF16, name="h_sb")
                f3_sb = fhp.tile([P, DC, SW], F32, name="f3_sb")
                for mc in range(MC):
                    ps = pp.tile([P, SW], F32, name=f"f1_{mc}", tag="pp")
                    for c in range(DC):
                        nc.tensor.matmul(ps[:], fc1_sb[:, c, mc * P:(mc + 1) * P],
                                         x2b_sb[:, c, :],
                                         start=(c == 0), stop=(c == DC - 1))
                    nc.scalar.activation(h_sb[:, mc, :], ps[:], AF.Relu,
                                         bias=fc1b_sb[:, mc:mc + 1])
                for gc in range(DC):
                    ps = pp.tile([P, SW], F32, name=f"f2_{gc}", tag="pp")
                    for mc in range(MC):
                        nc.tensor.matmul(ps[:],
                                         fc2_sb[:, mc, gc * P:(gc + 1) * P],
                                         h_sb[:, mc, :],
                                         start=(mc == 0), stop=(mc == MC - 1))
                    nc.vector.tensor_scalar_add(f3_sb[:, gc, :], ps[:],
                                                ffnb_sb[:, gc:gc + 1])

                # ---- LN3 + output ----
                out_sb = residp.tile([P, DC, SW], F32R, name="out_sb",
                                     tag="resid")
                layernorm(f3_sb, x2_sb, out_sb, ln_sb["ln3_b"], 2)
                for c in range(DC):
                    nc.sync.dma_start(
                        out=outt_d.ap().rearrange("(c p) s -> p c s", p=P)[:, c, :],
                        in_=out_sb[:, c, :].bitcast(F32))

            ffn_w_scope.__exit__(None, None, None)
            ca_kvw_scope.__exit__(None, None, None)
            ca_kv_scope.__exit__(None, None, None)

    nc.compile()
    return nc


def get_program():
    if "nc" not in _CACHE:
        _CACHE["nc"] = build_program()
    return _CACHE["nc"]


def kernel(**inputs) -> np.ndarray:
    from concourse.bass_utils import run_bass_kernel_spmd
    nc = get_program()
    in_maps = _host_prep(inputs)
    res = run_bass_kernel_spmd(nc, in_maps, core_ids=list(range(NC)))
    out = np.empty((B, S, D), np.float32)
    for b in range(B):
        out[b, 0:SW] = res.results[2 * b]["outt"].T
        out[b, SW:S] = res.results[2 * b + 1]["outt"].T
    return out


# revision 46
# speedup vs baseline: 1.0984x; 1.0009x over previous
"""Trainium2 Bass kernel for nn_Decoder (dense transformer decoder layer).

Problem (hardcoded): B=4, S=T=1024, D=512, H=8 heads, fp32.
  h  = MHA_self(x, causal) ; x1 = LN(h + x)
  h  = MHA_cross(x1, encod_out) ; x2 = LN(h + x1)
  ff = relu(x2 @ fc1) @ fc2 ; out = LN(ff + x2)

Sharding (8 cores = 4 batch groups x 2-core pairs), same as the f32r
baseline: self-attention tensor-parallel over heads (4 heads/core, full
S); one pair ReduceScatter (bf16) combines partial head-sums and splits
the sequence; LN1, cross-attn (all 8 heads, redundant K/V), LN2, FFN,
LN3 run sequence-parallel on the core's 512-row half.

Speed: nearly all matmuls run as fp8(e4m3) DoubleRow (contraction 256
per instruction at 0.5 cycles/row = 4x the f32r rate in the cost
model).  Numerics (validated against the fp32 reference in numpy):
  - projections/scores/AV/W2 fp8 with power-of-2 scales folded into
    ACT epilogues (x*8, w*512, Q/K/V*32, e*8, w2*2048)
  - bk dropped (exact softmax invariance); bv/bo/bf folded into acc
  - causal tail fix: attention output for early tokens is dominated by
    a single V row, so V for keys 0-255 is recomputed with bf16
    operands and query-tile 0 runs its AV/W2 path in bf16
  - FFN entirely bf16 (fp8 FFN alone costs ~1.5e-2 rel err)
  - residuals/LN in f32; ReduceScatter in bf16
Attention is query-tile-outer / head-inner so the W2 output accumulates
across all heads in one pinned PSUM tile (single epilogue per tile).
"""
import math
import numpy as np

B, S, T, D, H = 4, 1024, 1024, 512, 8
P = 128
NC = 8
DC = D // P    # 4 feature chunks
TC = T // P    # 8 time chunks
SW = 512       # per-core sequence half
QW = 256       # query tile width (DoubleRow moving limit)
MC = 2048 // P  # 16 FFN hidden chunks
EPS = 1e-5
PAIRS = [[0, 1], [2, 3], [4, 5], [6, 7]]

# fp8 scales (powers of 2; folded into f32 epilogue constants)
SX = 8.0       # x / x1 / enc quant
SWQ = 512.0    # wq/wk/wv quant
SQ = 32.0      # Q requant
SK = 32.0      # K requant
SV = 32.0      # V requant
SE = 8.0       # exp output
SW2 = 2048.0   # folded w2 quant
RSQD = 1.0 / math.sqrt(D)

_CACHE = {}


def _host_prep(inputs):
    import ml_dtypes
    F8 = ml_dtypes.float8_e4m3
    BF = ml_dtypes.bfloat16
    x = np.asarray(inputs["x"], np.float32)
    enc = np.asarray(inputs["encod_out"], np.float32)

    def q8(a, s):
        return (np.asarray(a, np.float32) * s).astype(F8)

    per_phase = {}
    for p in ("sa", "ca"):
        wq = np.asarray(inputs[p + "_wq"], np.float32)
        bq = np.asarray(inputs[p + "_bq"], np.float32)
        wk = np.asarray(inputs[p + "_wk"], np.float32)
        wv = np.asarray(inputs[p + "_wv"], np.float32)
        bv = np.asarray(inputs[p + "_bv"], np.float32)
        wo = np.asarray(inputs[p + "_wo"], np.float32)
        bo = np.asarray(inputs[p + "_bo"], np.float32)
        wf = np.asarray(inputs[p + "_wf"], np.float32).reshape(H, D, D)
        bf = np.asarray(inputs[p + "_bf"], np.float32)
        w2 = np.einsum("hfg,hgk->hfk", wo.astype(np.float64),
                       wf.astype(np.float64)).astype(np.float32)
        acc = bf.astype(np.float64).copy()
        for h in range(H):
            acc += (bv[h].astype(np.float64) @ wo[h].astype(np.float64)
                    + bo[h].astype(np.float64)) @ wf[h].astype(np.float64)
        per_phase[p] = dict(
            wq8=q8(wq, SWQ), wk8=q8(wk, SWQ), wv8=q8(wv, SWQ),
            w28=q8(w2, SW2), wv_bf=wv.astype(BF),
            bq=bq * SQ, acc=acc.astype(np.float32))

    fc1_w = np.asarray(inputs["fc1_w"], np.float32)
    fc1_b = np.asarray(inputs["fc1_b"], np.float32)
    fc2_w = np.asarray(inputs["fc2_w"], np.float32)
    fc2_b = np.asarray(inputs["fc2_b"], np.float32)
    lns = {f"ln{i}_{k}": np.asarray(inputs[f"ln{i}_{k}"], np.float32)
           for i in (1, 2, 3) for k in ("g", "b")}

    # causal masks for the two diagonal key-blocks of each 256-query tile:
    # kb 2i: keep p <= c ; kb 2i+1: keep 128+p <= c   (c in 0..255)
    pp_ = np.arange(P)[:, None]
    cc = np.arange(QW)[None, :]
    mpair = np.stack([(pp_ <= cc), (P + pp_ <= cc)], axis=1)
    ones_pair = np.ones((P, 2, 1), np.float32)

    in_maps = []
    for c in range(NC):
        b, half = c // 2, c % 2
        hs = slice(4 * half, 4 * half + 4)
        ssl = slice(half * SW, (half + 1) * SW)
        xt = x[b].T
        m = {
            "xt8": np.ascontiguousarray((xt * SX)).astype(F8),
            "xbf": np.ascontiguousarray(xt[:, :QW]).astype(BF),
            "x_res": np.ascontiguousarray(xt[:, ssl]),
            "et8": np.ascontiguousarray(enc[b].T * SX).astype(F8),
            "masks8": mpair.astype(F8),
            "masksbf": mpair.astype(BF),
            "ones8": ones_pair.astype(F8),
            "onesbf": np.ones((P, 1), BF),
            "ones_row": np.ones((1, P), np.float32),
            "ones_col": np.ones((P, 1), np.float32),
            "fc1bf": fc1_w.astype(BF), "fc1_b": fc1_b,
            "fc2bf": fc2_w.astype(BF), "ffn_bias": fc2_b,
        }
        pp = per_phase["sa"]
        for k in ("wq8", "wk8", "wv8", "w28", "wv_bf", "bq"):
            m["sa_" + k] = np.ascontiguousarray(pp[k][hs])
        m["sa_acc"] = pp["acc"] / 2.0
        pp = per_phase["ca"]
        for k in ("wq8", "wk8", "wv8", "w28", "bq"):
            m["ca_" + k] = pp[k]
        m["ca_acc"] = pp["acc"]
        for k, v in lns.items():
            m[k] = v
        in_maps.append(m)
    return in_maps


def build_program():
    import concourse.bacc as bacc
    import concourse.mybir as mybir
    import concourse.tile as tile

    F32 = mybir.dt.float32
    F32R = mybir.dt.float32r
    BF16 = mybir.dt.bfloat16
    F8 = mybir.dt.float8e4
    AF = mybir.ActivationFunctionType
    OP = mybir.AluOpType
    DR = mybir.MatmulPerfMode.DoubleRow

    nc = bacc.Bacc(None, target_bir_lowering=False, num_devices=NC)

    # ---- DRAM I/O ----
    xt8_d = nc.dram_tensor("xt8", [D, S], F8, kind="ExternalInput")
    xbf_d = nc.dram_tensor("xbf", [D, QW], BF16, kind="ExternalInput")
    xres_d = nc.dram_tensor("x_res", [D, SW], F32, kind="ExternalInput")
    et8_d = nc.dram_tensor("et8", [D, T], F8, kind="ExternalInput")
    masks8_d = nc.dram_tensor("masks8", [P, 2, QW], F8, kind="ExternalInput")
    masksbf_d = nc.dram_tensor("masksbf", [P, 2, QW], BF16, kind="ExternalInput")
    ones8_d = nc.dram_tensor("ones8", [P, 2, 1], F8, kind="ExternalInput")
    onesbf_d = nc.dram_tensor("onesbf", [P, 1], BF16, kind="ExternalInput")
    onesrow_d = nc.dram_tensor("ones_row", [1, P], F32, kind="ExternalInput")
    onescol_d = nc.dram_tensor("ones_col", [P, 1], F32, kind="ExternalInput")
    sa_d = {k: nc.dram_tensor("sa_" + k, [4, D, D], F8, kind="ExternalInput")
            for k in ("wq8", "wk8", "wv8", "w28")}
    sa_d["wv_bf"] = nc.dram_tensor("sa_wv_bf", [4, D, D], BF16, kind="ExternalInput")
    sa_d["bq"] = nc.dram_tensor("sa_bq", [4, D], F32, kind="ExternalInput")
    sa_d["acc"] = nc.dram_tensor("sa_acc", [D], F32, kind="ExternalInput")
    ca_d = {k: nc.dram_tensor("ca_" + k, [H, D, D], F8, kind="ExternalInput")
            for k in ("wq8", "wk8", "wv8", "w28")}
    ca_d["bq"] = nc.dram_tensor("ca_bq", [H, D], F32, kind="ExternalInput")
    ca_d["acc"] = nc.dram_tensor("ca_acc", [D], F32, kind="ExternalInput")
    fc1_d = nc.dram_tensor("fc1bf", [D, 2048], BF16, kind="ExternalInput")
    fc1b_d = nc.dram_tensor("fc1_b", [2048], F32, kind="ExternalInput")
    fc2_d = nc.dram_tensor("fc2bf", [2048, D], BF16, kind="ExternalInput")
    ffnb_d = nc.dram_tensor("ffn_bias", [D], F32, kind="ExternalInput")
    ln_d = {f"ln{i}_{k}": nc.dram_tensor(f"ln{i}_{k}", [D], F32, kind="ExternalInput")
            for i in (1, 2, 3) for k in ("g", "b")}
    outt_d = nc.dram_tensor("outt", [D, SW], F32, kind="ExternalOutput")

    r32 = lambda ap: ap.bitcast(F32R)

    # epilogue constants
    C_Q = SQ / (SX * SWQ)
    C_K = SK / (SX * SWQ)
    C_V = SV / (SX * SWQ)
    C_EXP = RSQD / (SQ * SK)     # scale on score psum inside exp
    C_W2 = 1.0 / (SV * SW2)      # scale on fp8 W2 psum
    LN_SE = math.log(SE)

    with tile.TileContext(nc, pool_alloc_mode="queue") as tc:
        with tc.tile_pool(name="const", bufs=1) as constp, \
             tc.tile_pool(name="resid", bufs=2) as residp, \
             tc.tile_pool(name="smalls", bufs=3) as smallp, \
             tc.tile_pool(name="stats", bufs=4) as statp, \
             tc.tile_pool(name="pp", bufs=2, space="PSUM") as pp, \
             tc.tile_pool(name="sc", bufs=4, space="PSUM") as scp, \
             tc.tile_pool(name="pw", bufs=1, space="PSUM") as pwp, \
             tc.tile_pool(name="dram", bufs=1, space="DRAM") as dramp:

            # ---- constants ----
            eps_sb = constp.tile([1, 1], F32, name="eps_sb")
            nc.vector.memset(eps_sb[:], EPS)
            ln8_col = constp.tile([P, 1], F32, name="ln8_col")
            nc.vector.memset(ln8_col[:], LN_SE)
            zero_col = constp.tile([P, 1], F32, name="zero_col")
            nc.vector.memset(zero_col[:], 0.0)
            ones_col = constp.tile([P, 1], F32R, name="ones_col")
            nc.scalar.dma_start(out=ones_col[:],
                                in_=r32(onescol_d.ap()))

            xt8_sb = residp.tile([P, DC, S], F8, name="xt8_sb", tag="resid")
            nc.sync.dma_start(out=xt8_sb[:],
                              in_=xt8_d.ap().rearrange("(c p) s -> p c s", p=P))
            xbf_sb = constp.tile([P, DC, QW], BF16, name="xbf_sb")
            nc.scalar.dma_start(out=xbf_sb[:],
                                in_=xbf_d.ap().rearrange("(c p) s -> p c s", p=P))
            ca_kv_scope = tc.tile_pool(name="ca_kvp", bufs=4)
            kvpp = ca_kv_scope.__enter__()
            ca_kvw_scope = tc.tile_pool(name="ca_kvw", bufs=2)
            kvwp = ca_kvw_scope.__enter__()
            early_scope = tc.tile_pool(name="early", bufs=1)
            earlyp = early_scope.__enter__()
            xres_sb = earlyp.tile([P, DC, SW], F32, name="xres_sb", tag="xres")
            nc.scalar.dma_start(out=xres_sb[:],
                                in_=xres_d.ap().rearrange("(c p) s -> p c s", p=P))
            et8_sb = kvwp.tile([P, DC, T], F8, name="et8_sb", tag="et",
                               bufs=1)
            nc.scalar.dma_start(out=et8_sb[:],
                                in_=et8_d.ap().rearrange("(c p) s -> p c s", p=P))

            masks8_sb = constp.tile([P, 2, QW], F8, name="masks8_sb")
            nc.scalar.dma_start(out=masks8_sb[:], in_=masks8_d.ap())
            masksbf_sb = constp.tile([P, 2, QW], BF16, name="masksbf_sb")
            nc.scalar.dma_start(out=masksbf_sb[:], in_=masksbf_d.ap())
            ones8_sb = constp.tile([P, 2, 1], F8, name="ones8_sb")
            nc.scalar.dma_start(out=ones8_sb[:], in_=ones8_d.ap())
            onesbf_sb = constp.tile([P, 1], BF16, name="onesbf_sb")
            nc.scalar.dma_start(out=onesbf_sb[:], in_=onesbf_d.ap())
            ones_row = constp.tile([1, P], F32R, name="ones_row")
            nc.scalar.dma_start(out=ones_row[:], in_=r32(onesrow_d.ap()))

            def vec_to_pc(dram_ap, name, nch):
                t = constp.tile([P, nch], F32, name=name)
                nc.scalar.dma_start(out=t[:],
                                    in_=dram_ap.rearrange("(c p) -> p c", p=P))
                return t

            bias_sb = {}
            for pn, dd, nh in (("sa", sa_d, 4), ("ca", ca_d, H)):
                t = constp.tile([P, nh, DC], F32, name=f"{pn}_bq_sb")
                nc.scalar.dma_start(
                    out=t[:], in_=dd["bq"].ap().rearrange("h (c p) -> p h c", p=P))
                bias_sb[pn, "bq"] = t
                bias_sb[pn, "acc"] = vec_to_pc(dd["acc"].ap(), f"{pn}_acc_sb", DC)
            grow_sb = constp.tile([1, 3, DC, P], F32R, name="ln_grow")
            for _i in (1, 2, 3):
                nc.scalar.dma_start(
                    out=grow_sb[:, _i - 1, :, :],
                    in_=r32(ln_d[f"ln{_i}_g"].ap().rearrange(
                        "(a c p) -> a c p", a=1, p=P)))
            fc1b_sb = vec_to_pc(fc1b_d.ap(), "fc1b_sb", MC)
            ffnb_sb = vec_to_pc(ffnb_d.ap(), "ffnb_sb", DC)
            ln_sb = {k: vec_to_pc(v.ap(), k + "_sb", DC) for k, v in ln_d.items()}

            cc_in = dramp.tile([2, D, SW], BF16, name="cc_in")
            cc_half = dramp.tile([D, SW], BF16, name="cc_half")

            def layernorm_half(src, resid_sb, dst, b_sb, gri, csl):
                """dst[:, :, csl] = LN(src + resid) on a column slice."""
                W = csl.stop - csl.start
                for c in range(DC):
                    nc.gpsimd.tensor_add(dst[:, c, csl], src[:, c, csl],
                                         resid_sb[:, c, csl])
                psum_sum = pp.tile([1, SW], F32, name="ln_sum", tag="pp")
                psum_ssq = pp.tile([1, SW], F32, name="ln_ssq", tag="pp")
                for c in range(DC):
                    sq = smallp.tile([P, SW], F32R, name=f"ln_sq_{c}", tag="sm")
                    nc.scalar.activation(sq[:, :W], dst[:, c, csl], AF.Square)
                    nc.tensor.matmul(psum_sum[:, :W], ones_col[:],
                                     dst[:, c, csl],
                                     start=(c == 0), stop=(c == DC - 1))
                    nc.tensor.matmul(psum_ssq[:, :W], ones_col[:], sq[:, :W],
                                     start=(c == 0), stop=(c == DC - 1))
                mean = statp.tile([1, SW], F32R, name="ln_mean", tag="st")
                nc.scalar.activation(mean[:, :W], psum_sum[:, :W], AF.Copy,
                                     scale=1.0 / D)
                msq = statp.tile([1, SW], F32, name="ln_msq", tag="st")
                nc.scalar.activation(msq[:, :W], psum_ssq[:, :W], AF.Copy,
                                     scale=1.0 / D)
                var = statp.tile([1, SW], F32, name="ln_var", tag="st")
                nc.vector.tensor_tensor(var[:, :W], mean[:, :W], mean[:, :W],
                                        OP.mult)
                nc.vector.tensor_sub(var[:, :W], msq[:, :W], var[:, :W])
                std = statp.tile([1, SW], F32, name="ln_std", tag="st")
                nc.scalar.activation(std[:, :W], var[:, :W], AF.Sqrt,
                                     bias=eps_sb[:])
                rstd = statp.tile([1, SW], F32R, name="ln_rstd", tag="st")
                with nc.allow_low_precision(reason="f32r feed for bcast matmul"):
                    nc.vector.reciprocal(rstd[:, :W], std[:, :W])
                mr = statp.tile([1, SW], F32R, name="ln_mr", tag="st")
                nc.vector.tensor_tensor(mr[:, :W], mean[:, :W], rstd[:, :W],
                                        OP.mult)
                for c in range(DC):
                    psum_rb = pp.tile([P, SW], F32, name=f"ln_rb_{c}", tag="pp")
                    nc.tensor.matmul(psum_rb[:, :W], grow_sb[:, gri, c, :],
                                     rstd[:, :W], start=True, stop=True)
                    nc.tensor.matmul(psum_rb[:, W:2 * W],
                                     grow_sb[:, gri, c, :],
                                     mr[:, :W], start=True, stop=True)
                    tmp = smallp.tile([P, SW], F32, name=f"ln_t_{c}", tag="sm")
                    nc.vector.tensor_tensor(tmp[:, :W], dst[:, c, csl],
                                            psum_rb[:, :W], OP.mult)
                    nc.vector.scalar_tensor_tensor(
                        dst[:, c, csl], tmp[:, :W], b_sb[:, c:c + 1],
                        psum_rb[:, W:2 * W], OP.add, OP.subtract)

            def layernorm(src, resid_sb, dst, b_sb, gri):
                for hlf in range(2):
                    layernorm_half(src, resid_sb, dst, b_sb, gri,
                                   slice(hlf * QW, (hlf + 1) * QW))

            def epi_rr(eng, out_ap, ps_ap, cscale, bias_ap):
                """projection epilogue out = ps*c (+bias) on a chosen engine"""
                if eng is nc.scalar:
                    return nc.scalar.activation(
                        out_ap, ps_ap, AF.Identity, scale=cscale,
                        bias=bias_ap if bias_ap is not None else zero_col[:])
                if bias_ap is None:
                    return eng.tensor_scalar(out_ap, ps_ap, cscale, None,
                                             OP.mult)
                return eng.tensor_scalar(out_ap, ps_ap, cscale, bias_ap,
                                         OP.mult, OP.add)

            def proj_head(src_sb, wk_sb, wv_sb, wq_ap, bq_ap, kt8, v8, qt8,
                          nq, rr):
                """fp8 K^T [f,t], V [t,f], and optional Q^T [f,q] for one
                head.  Epilogues round-robin across Pool/ACT/DVE so all
                three drain in parallel."""
                last = None
                nrr = len(rr)
                ei = [0]
                def nxt():
                    e = rr[ei[0] % nrr]; ei[0] += 1
                    return e
                for fc in range(DC):
                    for th in range(2):
                        ps = pp.tile([P, SW], F32, name=f"kp_{fc}_{th}",
                                     tag="pp")
                        for nt in range(2):
                            for cp in range(2):
                                nc.tensor.matmul(
                                    ps[:, nt * QW:(nt + 1) * QW],
                                    wk_sb[:, 2 * cp:2 * cp + 2,
                                          fc * P:(fc + 1) * P],
                                    xt8_slice(src_sb, cp,
                                              th * SW + nt * QW),
                                    start=(cp == 0), stop=(cp == 1),
                                    perf_mode=DR)
                        epi_rr(nxt(), kt8[:, fc, th * SW:(th + 1) * SW],
                               ps[:], C_K, None)
                    for tci in (2 * fc, 2 * fc + 1):
                        ps = scp.tile([P, 2, QW], F32, name=f"vp_{tci}",
                                      tag="sc")
                        for fh in range(2):
                            for cp in range(2):
                                nc.tensor.matmul(
                                    ps[:, fh, :],
                                    src_sb[:, 2 * cp:2 * cp + 2,
                                           tci * P:(tci + 1) * P],
                                    wv_sb[:, 2 * cp:2 * cp + 2,
                                          fh * QW:(fh + 1) * QW],
                                    start=(cp == 0), stop=(cp == 1),
                                    perf_mode=DR)
                        last = epi_rr(nxt(), v8[:, tci, :], ps[:], C_V, None)
                    if wq_ap is not None:
                        for qh in range(nq):
                            ps = pp.tile([P, SW], F32, name=f"qp_{fc}_{qh}",
                                         tag="pp")
                            for nt in range(2):
                                for cp in range(2):
                                    nc.tensor.matmul(
                                        ps[:, nt * QW:(nt + 1) * QW],
                                        wq_ap[:, 2 * cp:2 * cp + 2,
                                              fc * P:(fc + 1) * P],
                                        xt8_slice(src_sb, cp,
                                                  qh * SW + nt * QW),
                                        start=(cp == 0), stop=(cp == 1),
                                        perf_mode=DR)
                            epi_rr(nxt(), qt8[:, fc, qh * SW:(qh + 1) * SW],
                                   ps[:], C_Q, bq_ap[:, fc:fc + 1])
                return last

            def xt8_slice(src_sb, cp, q0):
                return src_sb[:, 2 * cp:2 * cp + 2, q0:q0 + QW]

            # ============ self-attention (head-split, full S) ============
            with tc.tile_pool(name="sa_w2", bufs=1) as w2p, \
                 tc.tile_pool(name="sa_qkv", bufs=1) as qkvp, \
                 tc.tile_pool(name="sa_e", bufs=3) as ep, \
                 tc.tile_pool(name="sa_av", bufs=2) as avp, \
                 tc.tile_pool(name="sa_f", bufs=1) as fp:
                wkv_scope = tc.tile_pool(name="sa_wkv", bufs=2)
                wkvp = wkv_scope.__enter__()
                # projections for all 4 heads; per-head JIT weight DMA
                wq_sb, wk_sb, wv_sb, wvb_sb, w28_sb, w2b_sb = {}, {}, {}, {}, {}, {}
                kt8, v8, vbf, qt8 = {}, {}, {}, {}
                for h in range(4):
                    wk_sb[h] = wkvp.tile([P, DC, D], F8, name=f"sa_wk_{h}",
                                         tag="wk")
                    nc.sync.dma_start(out=wk_sb[h][:], in_=sa_d["wk8"].ap()[h]
                                      .rearrange("(c p) f -> p c f", p=P))
                    wv_sb[h] = wkvp.tile([P, DC, D], F8, name=f"sa_wv_{h}",
                                         tag="wv")
                    nc.sync.dma_start(out=wv_sb[h][:], in_=sa_d["wv8"].ap()[h]
                                      .rearrange("(c p) f -> p c f", p=P))
                    wq_sb[h] = wkvp.tile([P, DC, D], F8, name=f"sa_wq_{h}",
                                         tag="wq")
                    nc.sync.dma_start(out=wq_sb[h][:], in_=sa_d["wq8"].ap()[h]
                                      .rearrange("(c p) f -> p c f", p=P))
                    wvb_sb[h] = wkvp.tile([P, DC, D], BF16, name=f"sa_wvb_{h}",
                                          tag="wvb")
                    nc.sync.dma_start(out=wvb_sb[h][:], in_=sa_d["wv_bf"].ap()[h]
                                      .rearrange("(c p) f -> p c f", p=P))
                    kt8[h] = qkvp.tile([P, DC, S], F8, name=f"sa_kt_{h}",
                                       tag=f"kt{h}")
                    v8[h] = qkvp.tile([P, TC, D], F8, name=f"sa_v_{h}",
                                      tag=f"v{h}")
                    qt8[h] = qkvp.tile([P, DC, S], F8, name=f"sa_qt_{h}",
                                       tag=f"qt{h}")
                    proj_head(xt8_sb, wk_sb[h], wv_sb[h], wq_sb[h],
                              bias_sb["sa", "bq"][:, h, :], kt8[h], v8[h],
                              qt8[h], 2, [nc.vector, nc.scalar])
                    # bf16 V for keys 0..255 (early-token numerics)
                    vbf[h] = qkvp.tile([P, 2, D], BF16, name=f"sa_vb_{h}",
                                       tag=f"vb{h}")
                    for tci in range(2):
                        ps = pp.tile([P, SW], F32, name=f"vbp_{h}_{tci}",
                                     tag="pp")
                        for c in range(DC):
                            nc.tensor.matmul(
                                ps[:], xbf_sb[:, c, tci * P:(tci + 1) * P],
                                wvb_sb[h][:, c, :],
                                start=(c == 0), stop=(c == DC - 1))
                        nc.vector.tensor_copy(vbf[h][:, tci, :], ps[:])
                wkv_scope.__exit__(None, None, None)
                # W2 weights arrive after the projection weights
                for h in range(4):
                    w28_sb[h] = w2p.tile([P, DC, D], F8, name=f"sa_w28_{h}",
                                         tag=f"w28{h}")
                    nc.sync.dma_start(out=w28_sb[h][:], in_=sa_d["w28"].ap()[h]
                                      .rearrange("(c p) f -> p c f", p=P))

                fbf = fp.tile([P, DC, S], BF16, name="sa_fbf")

                # attention: query-tile outer, head inner
                for qt in range(4):
                    qsl = slice(qt * QW, (qt + 1) * QW)
                    npair = qt + 1     # key-block pairs for this tile
                    bf = (qt == 0)     # bf16 AV/W2 path for queries 0..255
                    pw = pwp.tile([P, DC, QW], F32, name=f"sa_pw_{qt}", tag="pw")
                    rbs = {}
                    for h in range(4):
                        e8 = ep.tile([P, 2, QW] if bf else [P, TC, QW],
                                     BF16 if bf else F8,
                                     name=f"sa_e_{qt}_{h}",
                                     tag="ebf" if bf else "e",
                                     bufs=2 if bf else None)
                        dn = pp.tile([1, 2, QW], F32, name=f"sa_dn_{qt}_{h}",
                                     tag="pp")
                        for jp in range(npair):
                            sps = scp.tile([P, 2, QW], F32,
                                           name=f"sa_s_{qt}_{h}_{jp}", tag="sc")
                            for j2 in range(2):
                                kb = 2 * jp + j2
                                for fcp in range(2):
                                    nc.tensor.matmul(
                                        sps[:, j2, :],
                                        kt8[h][:, 2 * fcp:2 * fcp + 2,
                                               kb * P:(kb + 1) * P],
                                        qt8[h][:, 2 * fcp:2 * fcp + 2, qsl],
                                        start=(fcp == 0), stop=(fcp == 1),
                                        perf_mode=DR)
                            nc.scalar.activation(
                                e8[:, 2 * jp:2 * jp + 2, :], sps[:], AF.Exp,
                                scale=C_EXP,
                                bias=zero_col[:] if bf else ln8_col[:])
                            if jp == npair - 1:   # diagonal pair: apply mask
                                nc.vector.tensor_tensor(
                                    e8[:, 2 * jp:2 * jp + 2, :],
                                    e8[:, 2 * jp:2 * jp + 2, :],
                                    masksbf_sb[:] if bf else masks8_sb[:],
                                    OP.mult)
                        if bf:
                            for j2 in range(2):
                                nc.tensor.matmul(
                                    dn[:, 0, :], onesbf_sb[:],
                                    e8[:, j2, :],
                                    start=(j2 == 0), stop=(j2 == 1))
                        else:
                            nkb = 2 * npair
                            for kb in range(nkb):
                                nc.tensor.matmul(
                                    dn[:, 0, :], ones8_sb[:, 0, :],
                                    e8[:, kb, :],
                                    start=(kb == 0), stop=(kb == nkb - 1))
                        rc = statp.tile([1, QW], F32R, name=f"sa_rc_{qt}_{h}",
                                        tag="st")
                        with nc.allow_low_precision(reason="recip bcast"):
                            nc.vector.reciprocal(rc[:], dn[:, 0, :])
                        rbp = pp.tile([P, QW], F32, name=f"sa_rbp_{qt}_{h}",
                                      tag="pp")
                        nc.tensor.matmul(rbp[:], ones_row[:], rc[:],
                                         start=True, stop=True)
                        rb = smallp.tile([P, QW], F32, name=f"sa_rb_{qt}_{h}",
                                         tag="sm")
                        nc.scalar.activation(rb[:], rbp[:], AF.Copy)
                        rbs[h] = rb

                        avn = avp.tile([P, DC, QW], BF16 if bf else F8,
                                       name=f"sa_avn_{qt}_{h}",
                                       tag="avnbf" if bf else "avn",
                                       bufs=1 if bf else None)
                        for fp2 in range(2):
                            po = scp.tile([P, 2, QW], F32,
                                          name=f"sa_po_{qt}_{h}_{fp2}",
                                          tag="sc")
                            for f2_ in range(2):
                                fc = 2 * fp2 + f2_
                                if bf:
                                    for j in range(2):
                                        nc.tensor.matmul(
                                            po[:, f2_, :],
                                            vbf[h][:, j, fc * P:(fc + 1) * P],
                                            e8[:, j, :],
                                            start=(j == 0), stop=(j == 1))
                                else:
                                    for jp in range(npair):
                                        nc.tensor.matmul(
                                            po[:, f2_, :],
                                            v8[h][:, 2 * jp:2 * jp + 2,
                                                  fc * P:(fc + 1) * P],
                                            e8[:, 2 * jp:2 * jp + 2, :],
                                            start=(jp == 0),
                                            stop=(jp == npair - 1),
                                            perf_mode=DR)
                                nc.vector.tensor_tensor(
                                    avn[:, fc, :], po[:, f2_, :], rbs[h][:],
                                    OP.mult)
                        if bf:
                            # quantize the bf16 qt0 avn to fp8 (one ACT pass)
                            avnq = avp.tile([P, DC, QW], F8,
                                            name=f"sa_avnq_{qt}_{h}",
                                            tag="avn")
                            nc.scalar.activation(avnq[:], avn[:], AF.Copy,
                                                 scale=SV)
                            avn = avnq
                        w2s = w28_sb[h]
                        for gc in range(DC):
                            for fcp in range(2):
                                nc.tensor.matmul(
                                    pw[:, gc, :],
                                    w2s[:, 2 * fcp:2 * fcp + 2,
                                        gc * P:(gc + 1) * P],
                                    avn[:, 2 * fcp:2 * fcp + 2, :],
                                    start=(h == 0 and fcp == 0),
                                    stop=(h == 3 and fcp == 1),
                                    perf_mode=DR)
                    # epilogue: all 4 heads accumulated; + acc bias -> bf16
                    cw = C_W2
                    for gc in range(DC):
                        nc.scalar.activation(
                            fbf[:, gc, qsl], pw[:, gc, :], AF.Identity,
                            scale=cw, bias=bias_sb["sa", "acc"][:, gc:gc + 1])

                for half in range(2):
                    nc.sync.dma_start(
                        out=cc_in[half].rearrange("(c p) s -> p c s", p=P),
                        in_=fbf[:, :, half * SW:(half + 1) * SW])

            # one pair collective: reduce partial head-sums + scatter seq halves
            nc.gpsimd.collective_compute(
                "ReduceScatter", mybir.AluOpType.add, replica_groups=PAIRS,
                ins=[cc_in.opt()], outs=[cc_half.opt()])

            # ---- cross-attention K/V projections.  All 8 heads emitted into
            # a 4-deep rotating pool: the first ~3 run during the collective,
            # the rest pipeline as the head-outer attention loop frees slots.
            ln1_anchor = [None]
            ca_kt8, ca_v8 = {}, {}

            def ca_kv_proj(h, rr=None):
                wk_s = kvwp.tile([P, DC, D], F8, name=f"ca_wk_{h}", tag="wk")
                nc.sync.dma_start(out=wk_s[:], in_=ca_d["wk8"].ap()[h]
                                  .rearrange("(c p) f -> p c f", p=P))
                wv_s = kvwp.tile([P, DC, D], F8, name=f"ca_wv_{h}", tag="wv")
                nc.sync.dma_start(out=wv_s[:], in_=ca_d["wv8"].ap()[h]
                                  .rearrange("(c p) f -> p c f", p=P))
                ca_kt8[h] = kvpp.tile([P, DC, T], F8, name=f"ca_ktp_{h}",
                                      tag="cktp")
                ca_v8[h] = kvpp.tile([P, TC, D], F8, name=f"ca_vp_{h}",
                                     tag="cvp")
                return proj_head(et8_sb, wk_s, wv_s, None, None,
                                 ca_kt8[h], ca_v8[h], None, 0,
                                 rr or [nc.vector, nc.scalar])

            for h in range(4):
                last = ca_kv_proj(h)
                if h == 2:
                    ln1_anchor[0] = last

            # ---- LN1 on my sequence half ----
            cch_sb = earlyp.tile([P, DC, SW], BF16, name="cch_sb",
                                 tag="cch")
            nc.sync.dma_start(
                out=cch_sb[:],
                in_=cc_half.opt().rearrange("(c p) s -> p c s", p=P))
            x1_sb = residp.tile([P, DC, SW], F32R, name="x1_sb", tag="resid")
            from concourse.tile import add_dep_helper as _adh
            _bb = nc.main_func.blocks[-1]
            _n0 = len(_bb.instructions)
            layernorm(cch_sb, xres_sb, x1_sb, ln_sb["ln1_b"], 0)
            x18_sb = residp.tile([P, DC, SW], F8, name="x18_sb", tag="x18",
                                 bufs=1)
            for c in range(DC):
                nc.scalar.activation(x18_sb[:, c, :], x1_sb[:, c, :],
                                     AF.Copy, scale=SX)
            if ln1_anchor[0] is not None:
                for _ins in list(_bb.instructions)[_n0:]:
                    _adh(_ins, ln1_anchor[0].ins, sync=False,
                         reason="order LN1 after CA-KV precompute h2")
            early_scope.__exit__(None, None, None)

            # FFN weights prefetch (transfers overlap CA attention)
            ffn_w_scope = tc.tile_pool(name="ffn_w", bufs=1)
            fwp = ffn_w_scope.__enter__()
            fc1_sb = fwp.tile([P, DC, 2048], BF16, name="fc1_sb")
            fc2_sb = fwp.tile([P, MC, D], BF16, name="fc2_sb")

            # ============ cross-attention (seq-split, all heads) ============
            with tc.tile_pool(name="ca_w", bufs=1) as cwp, \
                 tc.tile_pool(name="ca_qt", bufs=1) as cqtp, \
                 tc.tile_pool(name="ca_e", bufs=2) as ep, \
                 tc.tile_pool(name="ca_av", bufs=2) as avp, \
                 tc.tile_pool(name="ca_f", bufs=1) as fp:
                wq_c, w2_c = {}, {}
                for h in range(H):
                    w2_c[h] = cwp.tile([P, DC, D], F8, name=f"ca_w2_{h}",
                                       tag=f"cw2{h}")
                    nc.sync.dma_start(out=w2_c[h][:], in_=ca_d["w28"].ap()[h]
                                      .rearrange("(c p) f -> p c f", p=P))
                # wq tiles 2-buffered: freed as Q projections complete
                cwqp_scope = tc.tile_pool(name="ca_wq", bufs=2)
                cwqp = cwqp_scope.__enter__()
                for h in range(H):
                    wq_c[h] = cwqp.tile([P, DC, D], F8, name=f"ca_wq_{h}",
                                        tag="cwq")
                    nc.sync.dma_start(out=wq_c[h][:], in_=ca_d["wq8"].ap()[h]
                                      .rearrange("(c p) f -> p c f", p=P))
                for mg in range(4):
                    nc.sync.dma_start(
                        out=fc1_sb[:, :, mg * SW:(mg + 1) * SW],
                        in_=fc1_d.ap().rearrange("(c p) m -> p c m", p=P)
                        [:, :, mg * SW:(mg + 1) * SW])
                nc.sync.dma_start(out=fc2_sb[:], in_=fc2_d.ap()
                                  .rearrange("(c p) g -> p c g", p=P))
                qt8_c = {}
                for h in range(H):
                    qt8_c[h] = cqtp.tile([P, DC, SW], F8, name=f"ca_qt_{h}",
                                         tag=f"cq{h}")
                    _ = wq_c[h]  # DMA emitted above; tiles rotate via pool
                    for fc in range(DC):
                        ps = pp.tile([P, SW], F32, name=f"cqp_{h}_{fc}",
                                     tag="pp")
                        for nt in range(2):
                            for cp in range(2):
                                nc.tensor.matmul(
                                    ps[:, nt * QW:(nt + 1) * QW],
                                    wq_c[h][:, 2 * cp:2 * cp + 2,
                                            fc * P:(fc + 1) * P],
                                    x18_sb[:, 2 * cp:2 * cp + 2,
                                           nt * QW:(nt + 1) * QW],
                                    start=(cp == 0), stop=(cp == 1),
                                    perf_mode=DR)
                        if h % 2:
                            nc.scalar.activation(
                                qt8_c[h][:, fc, :], ps[:], AF.Identity,
                                scale=C_Q,
                                bias=bias_sb["ca", "bq"][:, h, fc:fc + 1])
                        else:
                            nc.vector.tensor_scalar(
                                qt8_c[h][:, fc, :], ps[:], C_Q,
                                bias_sb["ca", "bq"][:, h, fc:fc + 1],
                                OP.mult, OP.add)
                cwqp_scope.__exit__(None, None, None)

                f2_sb = fp.tile([P, DC, SW], F32, name="ca_f2")
                for h in range(H):
                    e8 = ep.tile([P, 2 * TC, QW], F8, name=f"ca_e_{h}",
                                 tag="e")
                    dn = pp.tile([1, 2, QW], F32, name=f"ca_dn_{h}", tag="pp")
                    rb = pp.tile([P, 2, QW], F32, name=f"ca_rb_{h}", tag="pp")
                    for qt in range(2):
                        qsl = slice(qt * QW, (qt + 1) * QW)
                        for jp in range(4):
                            sps = scp.tile([P, 2, QW], F32,
                                           name=f"ca_s_{qt}_{h}_{jp}", tag="sc")
                            for j2 in range(2):
                                kb = 2 * jp + j2
                                for fcp in range(2):
                                    nc.tensor.matmul(
                                        sps[:, j2, :],
                                        ca_kt8[h][:, 2 * fcp:2 * fcp + 2,
                                                  kb * P:(kb + 1) * P],
                                        qt8_c[h][:, 2 * fcp:2 * fcp + 2, qsl],
                                        start=(fcp == 0), stop=(fcp == 1),
                                        perf_mode=DR)
                            nc.scalar.activation(
                                e8[:, 8 * qt + 2 * jp:8 * qt + 2 * jp + 2, :],
                                sps[:], AF.Exp, scale=C_EXP, bias=ln8_col[:])
                    # denominators after the scores loop (avoids PE
                    # head-of-line wait on each exp)
                    for qt in range(2):
                        for kb in range(8):
                            nc.tensor.matmul(
                                dn[:, qt, :], ones8_sb[:, 0, :],
                                e8[:, 8 * qt + kb, :],
                                start=(kb == 0), stop=(kb == 7))
                    rc = statp.tile([1, 2, QW], F32R, name=f"ca_rc_{h}",
                                    tag="st")
                    with nc.allow_low_precision(reason="recip bcast"):
                        nc.vector.reciprocal(rc[:], dn[:])
                    for qt in range(2):
                        nc.tensor.matmul(rb[:, qt, :], ones_row[:],
                                         rc[:, qt, :], start=True, stop=True)
                    rb_sb = smallp.tile([P, 2, QW], F32, name=f"ca_rbs_{h}",
                                        tag="sm")
                    nc.scalar.activation(rb_sb[:], rb[:], AF.Copy)

                    for qt in range(2):
                        qsl = slice(qt * QW, (qt + 1) * QW)
                        avn = avp.tile([P, DC, QW], F8,
                                       name=f"ca_avn_{qt}_{h}", tag="avn")
                        for fp2 in range(2):
                            po = scp.tile([P, 2, QW], F32,
                                          name=f"ca_po_{qt}_{h}_{fp2}",
                                          tag="sc")
                            for f2_ in range(2):
                                fc = 2 * fp2 + f2_
                                for jp in range(4):
                                    nc.tensor.matmul(
                                        po[:, f2_, :],
                                        ca_v8[h][:, 2 * jp:2 * jp + 2,
                                                 fc * P:(fc + 1) * P],
                                        e8[:, 8 * qt + 2 * jp:8 * qt + 2 * jp + 2, :],
                                        start=(jp == 0), stop=(jp == 3),
                                        perf_mode=DR)
                                nc.vector.tensor_tensor(
                                    avn[:, fc, :], po[:, f2_, :],
                                    rb_sb[:, qt, :], OP.mult)
                        pw = pwp.tile([P, DC, QW], F32, name=f"ca_pw_{qt}_{h}",
                                      tag="pw")
                        for gc in range(DC):
                            for fcp in range(2):
                                nc.tensor.matmul(
                                    pw[:, gc, :],
                                    w2_c[h][:, 2 * fcp:2 * fcp + 2,
                                            gc * P:(gc + 1) * P],
                                    avn[:, 2 * fcp:2 * fcp + 2, :],
                                    start=(fcp == 0), stop=(fcp == 1),
                                    perf_mode=DR)
                        if h == 0:
                            for gc in range(DC):
                                nc.vector.tensor_scalar(
                                    f2_sb[:, gc, qsl], pw[:, gc, :], C_W2,
                                    bias_sb["ca", "acc"][:, gc:gc + 1],
                                    OP.mult, OP.add)
                        else:
                            nc.vector.scalar_tensor_tensor(
                                f2_sb[:, :, qsl], pw[:], C_W2,
                                f2_sb[:, :, qsl], OP.mult, OP.add)
                    if h + 4 < H:
                        ca_kv_proj(h + 4, rr=[nc.vector, nc.scalar])

                # ---- LN2 ----
                x2_sb = residp.tile([P, DC, SW], F32R, name="x2_sb", tag="resid")
                layernorm(f2_sb, x1_sb, x2_sb, ln_sb["ln2_b"], 1)
                x2b_sb = residp.tile([P, DC, SW], BF16, name="x2b_sb",
                                     tag="x2b", bufs=1)
                for c in range(DC):
                    nc.gpsimd.tensor_copy(x2b_sb[:, c, :], x2_sb[:, c, :])
            # ============ FFN (bf16, seq-split) ============
            with tc.tile_pool(name="ffn_h", bufs=1) as fhp:
                h_sb = fhp.tile([P, MC, SW], BF16, name="h_sb")
                f3_sb = fhp.tile([P, DC, SW], F32, name="f3_sb")
                for mc in range(MC):
                    ps = pp.tile([P, SW], F32, name=f"f1_{mc}", tag="pp")
                    for c in range(DC):
                        nc.tensor.matmul(ps[:], fc1_sb[:, c, mc * P:(mc + 1) * P],
                                         x2b_sb[:, c, :],
                                         start=(c == 0), stop=(c == DC - 1))
                    nc.scalar.activation(h_sb[:, mc, :], ps[:], AF.Relu,
                                         bias=fc1b_sb[:, mc:mc + 1])
                for gc in range(DC):
                    ps = pp.tile([P, SW], F32, name=f"f2_{gc}", tag="pp")
                    for mc in range(MC):
                        nc.tensor.matmul(ps[:],
                                         fc2_sb[:, mc, gc * P:(gc + 1) * P],
                                         h_sb[:, mc, :],
                                         start=(mc == 0), stop=(mc == MC - 1))
                    nc.vector.tensor_scalar_add(f3_sb[:, gc, :], ps[:],
                                                ffnb_sb[:, gc:gc + 1])

                # ---- LN3 + output ----
                out_sb = residp.tile([P, DC, SW], F32R, name="out_sb",
                                     tag="resid")
                layernorm(f3_sb, x2_sb, out_sb, ln_sb["ln3_b"], 2)
                for c in range(DC):
                    nc.sync.dma_start(
                        out=outt_d.ap().rearrange("(c p) s -> p c s", p=P)[:, c, :],
                        in_=out_sb[:, c, :].bitcast(F32))

            ffn_w_scope.__exit__(None, None, None)
            ca_kvw_scope.__exit__(None, None, None)
            ca_kv_scope.__exit__(None, None, None)

    nc.compile()
    return nc


def get_program():
    if "nc" not in _CACHE:
        _CACHE["nc"] = build_program()
    return _CACHE["nc"]


def kernel(**inputs) -> np.ndarray:
    from concourse.bass_utils import run_bass_kernel_spmd
    nc = get_program()
    in_maps = _host_prep(inputs)
    res = run_bass_kernel_spmd(nc, in_maps, core_ids=list(range(NC)))
    out = np.empty((B, S, D), np.float32)
    for b in range(B):
        out[b, 0:SW] = res.results[2 * b]["outt"].T
        out[b, SW:S] = res.results[2 * b + 1]["outt"].T
    return out
